# revision 1
# baseline (speedup 1.0000x reference)
"""Trainium2 Bass kernel for a 2-layer RGCN scene-graph model (8 NeuronCores).

Sharding: node/dst-parallel. Core c owns nodes [c*1024, (c+1)*1024) (= 8 whole
graphs of 128 nodes, so pooling is local). Relation weights are replicated.
Message aggregation is mean-per-(relation, dst); we aggregate x first (the
relation transform is linear, so mean-then-transform == transform-then-mean)
which keeps all matmul work sharded. The segment mean is computed on the
TensorEngine as (gathered edge rows)^T @ one-hot, where the one-hot carries
1/cnt; edge rows are fetched with dma_gather from the all-gathered node
features in DRAM. All integer index metadata (edge sort, one-hot matrices,
counts) is precomputed on host inside kernel().
"""

import sys

sys.path.insert(0, "/opt/trn_rl_repo")

import numpy as np
import ml_dtypes

bf16 = ml_dtypes.bfloat16
fp8 = ml_dtypes.float8_e4m3
FP8 = True  # gather/one-hot path in float8_e4m3 (halves AG + gather + OH bytes)

N = 8192
E = 262144
R = 8
NCLS = 151
EMB = 256
BOX = 1024
HID = 512
OUT = 256
NT = 2
CORES = 8
NLOC = N // CORES          # 1024 nodes per core
NB = NLOC // 128           # 8 dst blocks of 128 per core
GPC = NLOC // 128          # graphs per core (nodes_per_graph == 128)

_PATCHED = False
DBG_SUB = 0  # 0=full layer, 1=gathers only, 2=+seg matmuls, 3=+transform


def _patch_tile():
    """This container's walrus rejects >2 sync-wait commands per instruction;
    TileContext's kernel-tail drain attaches one wait per active logical proc.
    Redistribute the drain's waits over event-sem instructions (2 each)."""
    global _PATCHED
    if _PATCHED:
        return
    import concourse.mybir as mybir
    import concourse.tile as tile
    from concourse.vector_clock import ScopedClock

    def _drain_and_barrier(self, tick_clock, wait_clock):
        nc = self.nc
        drain_inst = nc.sync.drain()
        wait_clock.add_sem_waits(
            drain_inst.ins, ScopedClock({None: tick_clock.global_clock})
        )
        si = drain_inst.ins.sync_info
        waits = list(si.on_wait) if si is not None else []
        if waits:
            drain_inst.ins.sync_info = mybir.SyncInfo(
                on_wait=[], on_update=list(si.on_update) if si else []
            )
            dummy_sem = nc.alloc_semaphore(f"tail_split_sem_{nc.next_id()}")
            for i in range(0, len(waits), 2):
                ev = nc.sync.wait_ge(dummy_sem, 0)
                evsi = ev.ins.sync_info
                ev.ins.sync_info = mybir.SyncInfo(
                    on_wait=waits[i : i + 2],
                    on_update=list(evsi.on_update) if evsi else [],
                )
        nc.all_engine_barrier()
        assert self.sems is not None
        popped = nc._tile_sem_poison_stack.pop()
        assert popped is self._sem_poison
        nc.clear_and_free_semaphores(list(self.sems.allocated().values()))
        nc.all_engine_barrier()

    tile.TileContext._drain_and_barrier = _drain_and_barrier
    _PATCHED = True


def _split_excess_waits(nc, max_waits=2):
    """This walrus build rejects instructions carrying more than 2 sync-wait
    commands, but Tile's wait-assignment pass can attach more. Move excess
    waits onto same-engine EventSemaphore instructions inserted just before
    the over-subscribed instruction."""
    import concourse.mybir as mybir

    counter = [0]
    for f in nc.m.functions:
        for bb in f.blocks:
            cur = list(bb.instructions)
            out = []
            changed = False
            for ins in cur:
                si = ins.sync_info
                waits = list(si.on_wait) if si is not None else []
                allowed = (
                    max_waits
                    if type(ins).__name__ == "InstEventSemaphore"
                    else 1
                )
                if len(waits) > allowed:
                    keep = waits[:allowed]
                    extra = waits[allowed:]
                    ins.sync_info = mybir.SyncInfo(
                        on_wait=keep, on_update=list(si.on_update)
                    )
                    for i in range(0, len(extra), max_waits):
                        counter[0] += 1
                        ev = mybir.InstEventSemaphore(
                            name=f"I-wsplit-{counter[0]}",
                            ins=[],
                            outs=[],
                            engine=ins.engine,
                        )
                        ev.sync_info = mybir.SyncInfo(
                            on_wait=extra[i : i + max_waits], on_update=[]
                        )
                        out.append(ev)
                    changed = True
                out.append(ins)
            if changed:
                bb.instructions = out


def prep(inputs):
    """Host preprocessing: returns (in_maps, meta). meta['chunks'] is the
    uniform [NB][R] chunk table that parameterizes the program structure."""
    box = np.asarray(inputs["box_features"], dtype=np.float32)
    lab = np.asarray(inputs["pred_labels"]).astype(np.int64).reshape(-1)
    ei = np.asarray(inputs["edge_index"]).astype(np.int64)
    et = np.asarray(inputs["edge_type"]).astype(np.int64).reshape(-1)
    emb = np.asarray(inputs["emb_table"], dtype=np.float32)
    W_lin = np.asarray(inputs["W_lin"], dtype=np.float32)
    b_lin = np.asarray(inputs["b_lin"], dtype=np.float32)
    rel_W1 = np.asarray(inputs["rel_W1"], dtype=np.float32)
    root_W1 = np.asarray(inputs["root_W1"], dtype=np.float32)
    b1 = np.asarray(inputs["b1"], dtype=np.float32)
    rel_W2 = np.asarray(inputs["rel_W2"], dtype=np.float32)
    root_W2 = np.asarray(inputs["root_W2"], dtype=np.float32)
    b2 = np.asarray(inputs["b2"], dtype=np.float32)
    Wc1 = np.asarray(inputs["Wc1"], dtype=np.float32)
    bc1 = np.asarray(inputs["bc1"], dtype=np.float32)
    Wc2 = np.asarray(inputs["Wc2"], dtype=np.float32)
    bc2 = np.asarray(inputs["bc2"], dtype=np.float32)

    src, dst = ei[0], ei[1]

    # per-(relation, dst) in-degree -> 1/cnt, folded into the one-hot values
    cnt = np.bincount(et * N + dst, minlength=R * N).reshape(R, N)
    inv = (1.0 / np.maximum(cnt, 1)).astype(np.float32)

    core_of = dst // NLOC
    blk = (dst % NLOC) // 128

    # uniform chunk counts per (block, relation): max over cores
    key = (core_of * NB + blk) * R + et
    ecnt = np.bincount(key, minlength=CORES * NB * R).reshape(CORES, NB, R)
    chunks = np.ceil(ecnt / 128).astype(np.int64).max(axis=0)  # [NB, R]
    totc = int(chunks.sum())
    tot_slots = totc * 128

    # slot offset of each (b, r) group (group-major order: b outer, r inner)
    goff = np.zeros((NB, R), dtype=np.int64)
    acc = 0
    for b in range(NB):
        for r in range(R):
            goff[b, r] = acc
            acc += int(chunks[b, r]) * 128

    # per-core gather index + one-hot construction
    order = np.lexsort((et, blk, core_of))  # sort by (core, blk, r)
    s_src = src[order]
    s_dst = dst[order]
    s_et = et[order]
    s_core = core_of[order]
    s_blk = blk[order]
    s_inv = inv[s_et, s_dst]

    gidx_all = np.zeros((CORES, tot_slots), dtype=np.int16)
    oh_all = np.zeros((CORES, 128, tot_slots), dtype=np.float32)

    # slot position of each sorted edge: group offset + rank within group
    grp_key = (s_core * NB + s_blk) * R + s_et
    # rank within group via cumcount on sorted keys
    first_idx = np.zeros(E, dtype=np.int64)
    new_grp = np.empty(E, dtype=bool)
    new_grp[0] = True
    new_grp[1:] = grp_key[1:] != grp_key[:-1]
    grp_start = np.flatnonzero(new_grp)
    start_of = np.repeat(grp_start, np.diff(np.append(grp_start, E)))
    rank = np.arange(E) - start_of
    slot = goff[s_blk, s_et] + rank  # slot within the core's stream
    chunk_of = slot // 128
    srow = slot % 128
    dcol = s_dst % 128

    c_idx = s_core
    gidx_all[c_idx, slot] = s_src.astype(np.int16)
    oh_all[c_idx, srow, chunk_of * 128 + dcol] = s_inv

    # wrapped gather indices: position i -> [i % 16, i // 16], replicated x8
    gidx_wrap = np.zeros((CORES, 128, tot_slots // 16), dtype=np.int16)
    w = gidx_all.reshape(CORES, tot_slots // 16, 16).transpose(0, 2, 1)
    for rep in range(8):
        gidx_wrap[:, rep * 16 : (rep + 1) * 16, :] = w

    # weights, host-fused and laid out for SBUF tiles
    W_A = W_lin[:BOX]                                 # [1024, 512]
    W_Bc = emb @ W_lin[BOX:]                          # [151, 512]
    W_B = np.zeros((256, HID), dtype=np.float32)
    W_B[:NCLS] = W_Bc
    W_B[NCLS] = b_lin                                 # bias as a weight row
    labT = np.zeros((CORES, 256, NLOC), dtype=np.float32)
    for c in range(CORES):
        loc = lab[c * NLOC : (c + 1) * NLOC]
        labT[c, loc, np.arange(NLOC)] = 1.0
        labT[c, NCLS, :] = 1.0                        # constant-1 bias feature

    def chunk_rows(Wm, p=128):
        # [K, O] -> [128, (K/128)*O] with [p, k*O+o] = Wm[k*128+p, o]
        K, O = Wm.shape
        return Wm.reshape(K // p, p, O).transpose(1, 0, 2).reshape(p, -1)

    W1_t = np.concatenate(
        [chunk_rows(rel_W1[r]) for r in range(R)], axis=1
    )  # [128, 8*4*512]
    RW1_t = chunk_rows(root_W1)                       # [128, 4*512]
    W2_t = np.concatenate(
        [chunk_rows(rel_W2[r]) for r in range(R)], axis=1
    )  # [128, 8*4*256]
    RW2_t = chunk_rows(root_W2)                       # [128, 4*256]
    WC1_t = chunk_rows(Wc1)                           # [128, 2*256]
    WC2_t = chunk_rows(Wc2)                           # [128, 2*2]

    boxT = box.T.copy()                               # [1024 f, 8192 n]

    shared = {
        "W_A": chunk_rows(W_A).astype(bf16),          # [128, 8*512]
        "W_B": chunk_rows(W_B).astype(bf16),          # [128, 2*512]
        "W1": W1_t.astype(bf16),
        "RW1": RW1_t.astype(bf16),
        "W2": W2_t.astype(bf16),
        "RW2": RW2_t.astype(bf16),
        "WC1": WC1_t.astype(bf16),
        "WC2": WC2_t.astype(bf16),
        "b1row": b1.reshape(1, HID).astype(bf16),
        "b2row": b2.reshape(1, OUT).astype(bf16),
        "bc2row": bc2.reshape(1, NT).astype(bf16),
        "bc1row": bc1.reshape(1, OUT).astype(bf16),
        "ones1": np.ones((1, 128), dtype=bf16),
        "ones128": np.full((128, 1), 1.0 / 128.0, dtype=bf16),
    }

    pack_order = ["W_A", "W_B", "W1", "RW1", "W2", "RW2", "WC1", "WC2",
                  "b1row", "b2row", "bc2row", "bc1row", "ones1", "ones128"]
    in_maps = []
    offsets = None
    for c in range(CORES):
        m = dict(shared)
        m["boxT"] = np.ascontiguousarray(
            boxT[:, c * NLOC : (c + 1) * NLOC]
        ).reshape(8, 128, NLOC).transpose(1, 0, 2).reshape(128, 8 * NLOC).astype(bf16)
        m["labT"] = (
            labT[c].reshape(2, 128, NLOC).transpose(1, 0, 2).reshape(128, 2 * NLOC)
        ).astype(bf16)
        m["OH"] = oh_all[c].astype(bf16)             # [128, tot_slots]
        # pack every bf16 tensor into one flat blob (single DRAM parameter:
        # keeps host-side per-parameter binding overhead out of the NEFF)
        parts = ["boxT", "labT"] + pack_order + ["OH"]
        offs = {}
        cur = 0
        bufs = []
        for name in parts:
            a = np.ascontiguousarray(m[name], dtype=bf16)
            offs[name] = (cur, a.shape)
            bufs.append(a.reshape(-1))
            cur += a.size
        blob = np.concatenate(bufs)
        if offsets is None:
            offsets = offs
        in_maps.append({"blob": blob.reshape(1, -1), "GIDX": gidx_wrap[c]})

    meta = {"chunks": tuple(tuple(int(x) for x in row) for row in chunks),
            "fp8": FP8, "offsets": offsets,
            "blob_elems": int(in_maps[0]["blob"].size)}
    return in_maps, meta


def build(meta, split_waits=True, dbg_phase=99):
    _patch_tile()
    import concourse.bass as bass
    import concourse.mybir as mybir
    import concourse.tile as tile
    from concourse import library_config
    from concourse.bass import _add_dep_helper

    dt = mybir.dt
    AF = mybir.ActivationFunctionType
    use_fp8 = meta.get("fp8", False)
    gdt = dt.float8e4 if use_fp8 else dt.bfloat16
    chunks = meta["chunks"]
    totc = sum(sum(row) for row in chunks)
    tot_slots = totc * 128

    nc = bass.Bass()

    # ---- parameters: one packed bf16 blob + the int16 gather indices ----
    offsets = meta["offsets"]
    blob = nc.declare_dram_parameter(
        "blob", [1, meta["blob_elems"]], dt.bfloat16, isOutput=False
    )

    class _ParamViews(dict):
        def __missing__(self, name):
            off, shape = offsets[name]
            p, c = shape
            ap = blob.ap()[:, off : off + p * c]
            ap = ap.rearrange("a (p c) -> (a p) c", p=p)
            v = _View(ap)
            self[name] = v
            return v

    class _View:
        def __init__(self, ap):
            self._ap = ap
        def ap(self):
            return self._ap
        def __getitem__(self, idx):
            return self._ap[idx]

    P = _ParamViews()
    P["GIDX"] = nc.declare_dram_parameter(
        "GIDX", [128, tot_slots // 16], dt.int16, isOutput=False
    )
    out_y = nc.declare_dram_parameter("out", [2, GPC, NT], dt.float32, isOutput=True)

    # ---- internal DRAM ----
    x0_loc = nc.dram_tensor("x0_loc", [NLOC, HID], dt.bfloat16)
    xg_loc = nc.dram_tensor("xg_loc", [NLOC, HID], gdt)
    xfull = nc.dram_tensor("xfull", [N, HID], gdt, addr_space="Shared")
    h1_loc = nc.dram_tensor("h1_loc", [NLOC, HID], dt.bfloat16)
    hg_loc = nc.dram_tensor("hg_loc", [NLOC, HID], gdt)
    hfull = nc.dram_tensor("hfull", [N, HID], gdt, addr_space="Shared")

    rg = [list(range(CORES))]

    with tile.TileContext(nc) as tc:
        with (
            tc.tile_pool(name="wpool", bufs=1) as wpool,
            tc.tile_pool(name="xpool", bufs=1) as xpool,
            tc.tile_pool(name="spool", bufs=2) as spool,
            tc.tile_pool(name="psA", bufs=3, space="PSUM") as psA,
            tc.tile_pool(name="psB", bufs=2, space="PSUM") as psB,
            tc.tile_pool(name="psC", bufs=1, space="PSUM") as psC,
        ):
            # GPSIMD ucode library providing DMAGatherAnt; every dma_gather
            # gets an explicit dep edge on this load.
            liblod = nc.gpsimd.load_library(library_config.mlp)

            # ---- static loads ----
            def load(name, shape, dtype=dt.bfloat16, pool=wpool):
                t = pool.tile(list(shape), dtype, tag=name)
                nc.sync.dma_start(t[:], P[name].ap())
                return t

            fpool_cm = tc.tile_pool(name="fpool", bufs=1)
            fpool = fpool_cm.__enter__()
            boxT_t = load("boxT", (128, 8 * NLOC), pool=fpool)
            labT_t = load("labT", (128, 2 * NLOC), pool=fpool)
            W_A_t = load("W_A", (128, 8 * HID), pool=fpool)
            W_B_t = load("W_B", (128, 2 * HID), pool=fpool)

            W1_t = load("W1", (128, R * 4 * HID))
            RW1_t = load("RW1", (128, 4 * HID))
            W2_t = load("W2", (128, R * 4 * OUT))
            RW2_t = load("RW2", (128, 4 * OUT))
            WC1_t = load("WC1", (128, 2 * OUT))
            WC2_t = load("WC2", (128, 2 * NT))
            b1_t = load("b1row", (1, HID))
            b2_t = load("b2row", (1, OUT))
            bc2_t = load("bc2row", (1, NT))
            bc1_t = load("bc1row", (1, OUT))
            ones1_t = load("ones1", (1, 128))
            ones128_t = load("ones128", (128, 1))
            GIDX_t = load("GIDX", (128, tot_slots // 16), dt.int16)

            if True:
                # featurize: x0 = [box, onehot(lab)] @ W (+bias via weight row)
                x0N = fpool.tile([128, NB, HID], dt.bfloat16, tag="x0N")
                x0G = fpool.tile([128, NB, HID], gdt, tag="x0G")
                for nb in range(NB):
                    ps = psA.tile([128, HID], dt.float32, tag="ps512")
                    for k in range(8):
                        nc.tensor.matmul(
                            ps[:],
                            boxT_t[:, k * NLOC + nb * 128 : k * NLOC + (nb + 1) * 128],
                            W_A_t[:, k * HID : (k + 1) * HID],
                            start=(k == 0),
                            stop=False,
                        )
                    for k in range(2):
                        nc.tensor.matmul(
                            ps[:],
                            labT_t[:, k * NLOC + nb * 128 : k * NLOC + (nb + 1) * 128],
                            W_B_t[:, k * HID : (k + 1) * HID],
                            start=False,
                            stop=(k == 1),
                        )
                    nc.scalar.activation(x0N[:, nb, :], ps[:], AF.Copy)
                    nc.vector.tensor_copy(x0G[:, nb, :], ps[:])
                nc.sync.dma_start(
                    x0_loc.ap().rearrange("(nb p) f -> p nb f", p=128), x0N[:]
                )
                nc.sync.dma_start(
                    xg_loc.ap().rearrange("(nb p) f -> p nb f", p=128), x0G[:]
                )
            fpool_cm.__exit__(None, None, None)

            if dbg_phase >= 2:
                _build_rest(nc, tc, mybir, dt, AF, chunks, P,
                            x0_loc, xfull, h1_loc, hfull, rg, out_y,
                            wpool, xpool, spool, gpool_bufs=3,
                            xg_loc=xg_loc, hg_loc=hg_loc, gdt=gdt,
                            psA=psA, psB=psB, psC=psC,
                            W1_t=W1_t, RW1_t=RW1_t, W2_t=W2_t, RW2_t=RW2_t,
                            WC1_t=WC1_t, WC2_t=WC2_t, b1_t=b1_t, b2_t=b2_t,
                            bc2_t=bc2_t, bc1_t=bc1_t, ones1_t=ones1_t,
                            ones128_t=ones128_t, GIDX_t=GIDX_t,
                            liblod=liblod, dbg_phase=dbg_phase)
            else:
                # minimal output so the kernel is well-formed
                zz = spool.tile([8, NT], dt.float32, tag="ylog")
                nc.vector.memset(zz[:], 0.0)
                nc.sync.dma_start(out_y[0], zz[:])
                nc.sync.dma_start(out_y[1], zz[:])

    mybir.codegen_inst_isa_subclasses(nc)
    if split_waits:
        _split_excess_waits(nc)
    return nc


def _build_rest(nc, tc, mybir, dt, AF, chunks, P, x0_loc, xfull, h1_loc, hfull,
                rg, out_y, wpool, xpool, spool, gpool_bufs, psA, psB, psC,
                W1_t, RW1_t, W2_t, RW2_t, WC1_t, WC2_t, b1_t, b2_t, bc2_t,
                bc1_t, ones1_t, ones128_t, GIDX_t, liblod, dbg_phase,
                xg_loc=None, hg_loc=None, gdt=None):
    from concourse.bass import _add_dep_helper
    tile = None  # unused
    if True:
        if True:
            # ---- all-gather x0 ----
            nc.gpsimd.collective_compute(
                "AllGather",
                mybir.AluOpType.bypass,
                replica_groups=rg,
                ins=[xg_loc.ap().opt()],
                outs=[xfull.ap().opt()],
            )

            # transposed local x0 (for the root term): DMA-transpose loads
            x0T_t = xpool.tile([128, 4, NLOC], dt.bfloat16, tag="x0T")
            for k in range(4):
                nc.sync.dma_start(
                    x0T_t[:, k, :],
                    x0_loc[:, k * 128 : (k + 1) * 128],
                    transpose=True,
                )

            def dummy_out():
                zz = spool.tile([8, NT], dt.float32, tag="ylog")
                nc.vector.memset(zz[:], 0.0)
                nc.sync.dma_start(out_y[0], zz[:])
                nc.sync.dma_start(out_y[1], zz[:])

            if dbg_phase < 3:
                dummy_out()
                return

            lp = tc.tile_pool(name="gpool", bufs=4)
            gpool = lp.__enter__()
            lp2 = tc.tile_pool(name="ohpool", bufs=4)
            ohpool = lp2.__enter__()
            lp3 = tc.tile_pool(name="stpool", bufs=2)
            stpool = lp3.__enter__()

            def layer(src_full, xT_t, W_t, RW_t, brow_t, odim, act, out_tile, g_out=None):
                """One RGCN conv layer. out_tile: [128, NB, odim] bf16."""
                coff = 0  # chunk offset into the global stream
                for b in range(NB):
                    chb = [chunks[b][r] for r in range(R)]
                    nch = sum(chb)
                    # gather this block's edge rows in two halves
                    halves = []
                    h0 = nch // 2
                    for h, (c0, c1) in enumerate(((0, h0), (h0, nch))):
                        ncols = c1 - c0
                        gt = gpool.tile([128, max(ncols, 1), HID], gdt,
                                        tag="gt")
                        if ncols > 0:
                            g_ins = nc.gpsimd.dma_gather(
                                gt[:, :ncols, :],
                                src_full.ap(),
                                GIDX_t[:, (coff + c0) * 8 : (coff + c1) * 8],
                                num_idxs=ncols * 128,
                                num_idxs_reg=ncols * 128,
                                elem_size=HID,
                                single_packet=False,
                            )
                            _add_dep_helper(
                                g_ins.ins, liblod.ins,
                                reason="dma_gather needs mlp library",
                            )
                        oh = ohpool.tile([128, max(ncols, 1) * 128], dt.bfloat16,
                                         tag="oh")
                        if ncols > 0:
                            nc.sync.dma_start(
                                oh[:, : ncols * 128],
                                P["OH"][:, (coff + c0) * 128 : (coff + c1) * 128],
                            )
                        halves.append((gt, oh, c0, c1))
                    # segment means, transposed: S.T[f, dst] per relation
                    st = []
                    j0 = 0
                    for r in range(R):
                        nchr = chb[r]
                        if nchr == 0 or DBG_SUB == 1:
                            st.append(None)
                            continue
                        pss = psA.tile([128, HID], dt.float32, tag="ps512")
                        # one accumulation group for the whole bank: start only
                        # on the very first matmul, stop only on the last, with
                        # explicit edges to pin starter-first / stopper-last.
                        starter = None
                        last_by_fc = {}
                        for j in range(j0, j0 + nchr):
                            gt, oh, c0, c1 = halves[0] if j < h0 else halves[1]
                            jl = j - c0
                            for fc in range(4):
                                mm = nc.tensor.matmul(
                                    pss[:, fc * 128 : (fc + 1) * 128],
                                    gt[:, jl, fc * 128 : (fc + 1) * 128],
                                    oh[:, jl * 128 : (jl + 1) * 128],
                                    start=(j == j0 and fc == 0),
                                    stop=(j == j0 + nchr - 1 and fc == 3),
                                )
                                if starter is None:
                                    starter = mm
                                elif j == j0:
                                    _add_dep_helper(
                                        mm.ins, starter.ins,
                                        reason="psum group starter first",
                                    )
                                if j == j0 + nchr - 1:
                                    last_by_fc[fc] = mm
                        for fc in range(3):
                            _add_dep_helper(
                                last_by_fc[3].ins, last_by_fc[fc].ins,
                                reason="psum group stopper last",
                            )
                        stt = stpool.tile([128, HID], dt.bfloat16, tag=f"st{r}")
                        nc.scalar.activation(stt[:], pss[:], AF.Copy)
                        st.append(stt)
                        j0 += nchr
                    coff += nch
                    if DBG_SUB in (1, 2):
                        nc.vector.memset(out_tile[:, b, :], 0.0)
                        continue
                    # transform: h[dst, :] = sum_r S_r.T.T @ W_r + x.T.T @ RW + b
                    pst = psB.tile([128, odim], dt.float32, tag="transps")
                    nc.tensor.matmul(
                        pst[:], ones1_t[:1, :], brow_t[:1, :odim],
                        start=True, stop=False,
                    )
                    for r in range(R):
                        if st[r] is None:
                            continue
                        for k in range(4):
                            nc.tensor.matmul(
                                pst[:],
                                st[r][:, k * 128 : (k + 1) * 128],
                                W_t[:, (r * 4 + k) * odim : (r * 4 + k + 1) * odim],
                                start=False,
                                stop=False,
                            )
                    for k in range(4):
                        nc.tensor.matmul(
                            pst[:],
                            xT_t[:, k, b * 128 : (b + 1) * 128],
                            RW_t[:, k * odim : (k + 1) * odim],
                            start=False,
                            stop=(k == 3),
                        )
                    nc.scalar.activation(out_tile[:, b, :], pst[:], act)
                    if g_out is not None:
                        nc.vector.tensor_copy(g_out[:, b, :], pst[:])

            # ---- layer 1 ----
            h1N = xpool.tile([128, NB, HID], dt.bfloat16, tag="h1N")
            layer(xfull, x0T_t, W1_t, RW1_t, b1_t, HID, AF.Relu, h1N)
            nc.sync.dma_start(
                h1_loc.ap().rearrange("(nb p) f -> p nb f", p=128), h1N[:]
            )
            h1G = xpool.tile([128, NB, HID], gdt, tag="h1G")
            for nb in range(NB):
                nc.vector.tensor_copy(h1G[:, nb, :], h1N[:, nb, :])
            nc.sync.dma_start(
                hg_loc.ap().rearrange("(nb p) f -> p nb f", p=128), h1G[:]
            )
            if dbg_phase < 4:
                lp3.__exit__(None, None, None)
                lp2.__exit__(None, None, None)
                lp.__exit__(None, None, None)
                dummy_out()
                return
            nc.gpsimd.collective_compute(
                "AllGather",
                mybir.AluOpType.bypass,
                replica_groups=rg,
                ins=[hg_loc.ap().opt()],
                outs=[hfull.ap().opt()],
            )
            h1T_t = xpool.tile([128, 4, NLOC], dt.bfloat16, tag="h1T")
            for k in range(4):
                nc.sync.dma_start(
                    h1T_t[:, k, :],
                    h1_loc[:, k * 128 : (k + 1) * 128],
                    transpose=True,
                )

            if dbg_phase < 5:
                lp3.__exit__(None, None, None)
                lp2.__exit__(None, None, None)
                lp.__exit__(None, None, None)
                dummy_out()
                return

            # ---- layer 2 ----
            o2N = xpool.tile([128, NB, OUT], dt.bfloat16, tag="o2N")
            layer(hfull, h1T_t, W2_t, RW2_t, b2_t, OUT, AF.Copy, o2N)

            lp3.__exit__(None, None, None)
            lp2.__exit__(None, None, None)
            lp.__exit__(None, None, None)

            if dbg_phase < 6:
                dummy_out()
                return

            # ---- mean pooling (per graph = per node-block), transposed ----
            psp = psC.tile([128, 16], dt.float32, tag="poolps")
            for g in range(GPC):
                for fc in range(2):
                    nc.tensor.matmul(
                        psp[:, fc * 8 + g : fc * 8 + g + 1],
                        o2N[:, g, fc * 128 : (fc + 1) * 128],
                        ones128_t[:],
                        start=True,
                        stop=True,
                    )
            pooledT = spool.tile([128, 16], dt.bfloat16, tag="pooledT")
            nc.vector.tensor_copy(pooledT[:], psp[:])

            # ---- classifier ----
            zT = spool.tile([128, 16], dt.bfloat16, tag="zT")
            for f2c in range(2):
                psz = psC.tile([128, 8], dt.float32, tag="zps")
                nc.tensor.matmul(
                    psz[:],
                    bc1_t[:1, f2c * 128 : (f2c + 1) * 128],
                    ones1_t[:1, :8],
                    start=True,
                    stop=False,
                )
                for f1c in range(2):
                    nc.tensor.matmul(
                        psz[:],
                        WC1_t[:, f1c * OUT + f2c * 128 : f1c * OUT + (f2c + 1) * 128],
                        pooledT[:, f1c * 8 : (f1c + 1) * 8],
                        start=False,
                        stop=(f1c == 1),
                    )
                nc.scalar.activation(zT[:, f2c * 8 : (f2c + 1) * 8], psz[:], AF.Copy)
            psy = psC.tile([8, NT], dt.float32, tag="yps")
            for f2c in range(2):
                nc.tensor.matmul(
                    psy[:],
                    zT[:, f2c * 8 : (f2c + 1) * 8],
                    WC2_t[:, f2c * NT : (f2c + 1) * NT],
                    start=(f2c == 0),
                    stop=False,
                )
            nc.tensor.matmul(
                psy[:], ones1_t[:1, :GPC], bc2_t[:1, :], start=False, stop=True
            )
            ylog = spool.tile([8, NT], dt.float32, tag="ylog")
            nc.vector.tensor_copy(ylog[:], psy[:])
            negmax = spool.tile([8, 1], dt.float32, tag="negmax")
            nc.vector.reduce_max(
                negmax[:], ylog[:], axis=mybir.AxisListType.X, negate=True
            )
            ey = spool.tile([8, NT], dt.float32, tag="ey")
            nc.scalar.activation(ey[:], ylog[:], AF.Exp, bias=negmax[:])
            ssum = spool.tile([8, 1], dt.float32, tag="ssum")
            nc.vector.reduce_sum(ssum[:], ey[:], axis=mybir.AxisListType.X)
            rinv = spool.tile([8, 1], dt.float32, tag="rinv")
            nc.vector.reciprocal(rinv[:], ssum[:])
            yprob = spool.tile([8, NT], dt.float32, tag="yprob")
            nc.vector.tensor_scalar_mul(yprob[:], ey[:], rinv[:])

            nc.sync.dma_start(out_y[0], ylog[:])
            nc.sync.dma_start(out_y[1], yprob[:])


_CACHE = {}


def _get_built(inputs):
    in_maps, meta = prep(inputs)
    key = meta["chunks"]
    if key not in _CACHE:
        _CACHE[key] = build(meta)
    return _CACHE[key], in_maps, meta


def kernel(**inputs):
    nc, in_maps, meta = _get_built(inputs)
    from concourse.bass_utils import run_bass_kernel_spmd

    res = run_bass_kernel_spmd(nc, in_maps, core_ids=list(range(CORES)))
    ylog = np.concatenate([res.results[c]["out"][0] for c in range(CORES)], axis=0)
    yprob = np.concatenate([res.results[c]["out"][1] for c in range(CORES)], axis=0)
    return ylog.astype(np.float32), yprob.astype(np.float32)



# revision 21
# speedup vs baseline: 2.2668x; 2.2668x over previous
"""Trainium2 Bass kernel for a 2-layer RGCN scene-graph model (8 NeuronCores).

Sharding: node/dst-parallel. Core c owns nodes [c*1024, (c+1)*1024) (= 8 whole
graphs of 128 nodes, so pooling is local). Relation weights are replicated.
Message aggregation is mean-per-(relation, dst); we aggregate x first (the
relation transform is linear, so mean-then-transform == transform-then-mean)
which keeps all matmul work sharded. The segment mean is computed on the
TensorEngine as (gathered edge rows)^T @ one-hot, where the one-hot carries
1/cnt; edge rows are fetched with dma_gather from the all-gathered node
features in DRAM. The root-weight term x@RW is folded into the same pipeline
by a per-block PE transpose (matmul against an identity "one-hot"), so node
features never round-trip through DRAM for transposition. All integer index
metadata (edge sort, one-hot matrices, counts) is precomputed on host inside
kernel().
"""

import sys

sys.path.insert(0, "/opt/trn_rl_repo")

import numpy as np
import ml_dtypes

bf16 = ml_dtypes.bfloat16
fp8 = ml_dtypes.float8_e4m3
FP8 = True  # gather path in float8_e4m3 (halves AG + gather bytes)

N = 8192
E = 262144
R = 8
NCLS = 151
EMB = 256
BOX = 1024
HID = 512
OUT = 256
NT = 2
CORES = 8
NLOC = N // CORES          # 1024 nodes per core
NB = NLOC // 128           # 8 dst blocks of 128 per core
GPC = NLOC // 128          # graphs per core (nodes_per_graph == 128)

_PATCHED = False


def _patch_tile():
    """This container's walrus rejects >2 sync-wait commands per instruction;
    TileContext's kernel-tail drain attaches one wait per active logical proc.
    Redistribute the drain's waits over event-sem instructions (2 each)."""
    global _PATCHED
    if _PATCHED:
        return
    import concourse.mybir as mybir
    import concourse.tile as tile
    from concourse.vector_clock import ScopedClock

    def _drain_and_barrier(self, tick_clock, wait_clock):
        nc = self.nc
        drain_inst = nc.sync.drain()
        wait_clock.add_sem_waits(
            drain_inst.ins, ScopedClock({None: tick_clock.global_clock})
        )
        si = drain_inst.ins.sync_info
        waits = list(si.on_wait) if si is not None else []
        if waits:
            drain_inst.ins.sync_info = mybir.SyncInfo(
                on_wait=[], on_update=list(si.on_update) if si else []
            )
            dummy_sem = nc.alloc_semaphore(f"tail_split_sem_{nc.next_id()}")
            for i in range(0, len(waits), 2):
                ev = nc.sync.wait_ge(dummy_sem, 0)
                evsi = ev.ins.sync_info
                ev.ins.sync_info = mybir.SyncInfo(
                    on_wait=waits[i : i + 2],
                    on_update=list(evsi.on_update) if evsi else [],
                )
        nc.all_engine_barrier()
        assert self.sems is not None
        popped = nc._tile_sem_poison_stack.pop()
        assert popped is self._sem_poison
        nc.clear_and_free_semaphores(list(self.sems.allocated().values()))
        nc.all_engine_barrier()

    tile.TileContext._drain_and_barrier = _drain_and_barrier
    _PATCHED = True


def _split_excess_waits(nc, max_waits=2):
    """This walrus build rejects instructions carrying more than 2 sync-wait
    commands, but Tile's wait-assignment pass can attach more. Move excess
    waits onto same-engine EventSemaphore instructions inserted just before
    the over-subscribed instruction."""
    import concourse.mybir as mybir

    counter = [0]
    for f in nc.m.functions:
        for bb in f.blocks:
            cur = list(bb.instructions)
            out = []
            changed = False
            for ins in cur:
                si = ins.sync_info
                waits = list(si.on_wait) if si is not None else []
                allowed = (
                    max_waits
                    if type(ins).__name__ == "InstEventSemaphore"
                    else 1
                )
                if len(waits) > allowed:
                    keep = waits[:allowed]
                    extra = waits[allowed:]
                    ins.sync_info = mybir.SyncInfo(
                        on_wait=keep, on_update=list(si.on_update)
                    )
                    for i in range(0, len(extra), max_waits):
                        counter[0] += 1
                        ev = mybir.InstEventSemaphore(
                            name=f"I-wsplit-{counter[0]}",
                            ins=[],
                            outs=[],
                            engine=ins.engine,
                        )
                        ev.sync_info = mybir.SyncInfo(
                            on_wait=extra[i : i + max_waits], on_update=[]
                        )
                        out.append(ev)
                    changed = True
                out.append(ins)
            if changed:
                bb.instructions = out


def prep(inputs):
    """Host preprocessing: returns (in_maps, meta). meta['chunks'] is the
    uniform [NB][R] chunk table that parameterizes the program structure."""
    box = np.asarray(inputs["box_features"], dtype=np.float32)
    lab = np.asarray(inputs["pred_labels"]).astype(np.int64).reshape(-1)
    ei = np.asarray(inputs["edge_index"]).astype(np.int64)
    et = np.asarray(inputs["edge_type"]).astype(np.int64).reshape(-1)
    emb = np.asarray(inputs["emb_table"], dtype=np.float32)
    W_lin = np.asarray(inputs["W_lin"], dtype=np.float32)
    b_lin = np.asarray(inputs["b_lin"], dtype=np.float32)
    rel_W1 = np.asarray(inputs["rel_W1"], dtype=np.float32)
    root_W1 = np.asarray(inputs["root_W1"], dtype=np.float32)
    b1 = np.asarray(inputs["b1"], dtype=np.float32)
    rel_W2 = np.asarray(inputs["rel_W2"], dtype=np.float32)
    root_W2 = np.asarray(inputs["root_W2"], dtype=np.float32)
    b2 = np.asarray(inputs["b2"], dtype=np.float32)
    Wc1 = np.asarray(inputs["Wc1"], dtype=np.float32)
    bc1 = np.asarray(inputs["bc1"], dtype=np.float32)
    Wc2 = np.asarray(inputs["Wc2"], dtype=np.float32)
    bc2 = np.asarray(inputs["bc2"], dtype=np.float32)

    src, dst = ei[0], ei[1]

    # per-(relation, dst) in-degree -> 1/cnt, folded into the one-hot values
    cnt = np.bincount(et * N + dst, minlength=R * N).reshape(R, N)
    inv = (1.0 / np.maximum(cnt, 1)).astype(np.float32)

    core_of = dst // NLOC
    blk = (dst % NLOC) // 128

    # uniform chunk counts per (block, relation): max over cores
    key = (core_of * NB + blk) * R + et
    ecnt = np.bincount(key, minlength=CORES * NB * R).reshape(CORES, NB, R)
    chunks = np.ceil(ecnt / 128).astype(np.int64).max(axis=0)  # [NB, R]
    totc = int(chunks.sum())
    tot_slots = totc * 128

    # slot offset of each (b, r) group (group-major order: b outer, r inner)
    goff = np.zeros((NB, R), dtype=np.int64)
    acc = 0
    for b in range(NB):
        for r in range(R):
            goff[b, r] = acc
            acc += int(chunks[b, r]) * 128

    # per-core gather index + one-hot construction
    order = np.lexsort((et, blk, core_of))  # sort by (core, blk, r)
    s_src = src[order]
    s_dst = dst[order]
    s_et = et[order]
    s_core = core_of[order]
    s_blk = blk[order]
    s_inv = inv[s_et, s_dst]

    gidx_all = np.zeros((CORES, tot_slots), dtype=np.int16)
    # compact one-hot encoding: per slot, the dst column (0..127); the binary
    # [slot, dst] one-hot matrix is built on-device by DVE as (IOTA == dcol)
    # in fp8 (exact); padding slots get dcol=255 which never matches. The
    # 1/cnt mean normalization is applied afterwards in bf16 via INVRB.
    dcol_all = np.full((CORES, 128, totc), 255.0, dtype=np.float32)

    # slot position of each sorted edge: group offset + rank within group
    grp_key = (s_core * NB + s_blk) * R + s_et
    # rank within group via cumcount on sorted keys
    new_grp = np.empty(E, dtype=bool)
    new_grp[0] = True
    new_grp[1:] = grp_key[1:] != grp_key[:-1]
    grp_start = np.flatnonzero(new_grp)
    start_of = np.repeat(grp_start, np.diff(np.append(grp_start, E)))
    rank = np.arange(E) - start_of
    slot = goff[s_blk, s_et] + rank  # slot within the core's stream
    chunk_of = slot // 128
    srow = slot % 128
    dcol = s_dst % 128

    c_idx = s_core
    gidx_all[c_idx, slot] = s_src.astype(np.int16)
    dcol_all[c_idx, srow, chunk_of] = dcol

    # 1/cnt per (block, relation, dst-col), row-replicated for DVE use:
    # invrb[c, :, (b*R+r)*128 + d] = inv[r, c*NLOC + b*128 + d]
    invrb = np.zeros((CORES, 128, NB * R * 128), dtype=np.float32)
    for c in range(CORES):
        for b in range(NB):
            for r in range(R):
                invrb[c, :, (b * R + r) * 128 : (b * R + r + 1) * 128] = inv[
                    r, c * NLOC + b * 128 : c * NLOC + (b + 1) * 128
                ][None, :]

    # wrapped gather indices: position i -> [i % 16, i // 16], replicated x8
    gidx_wrap = np.zeros((CORES, 128, tot_slots // 16), dtype=np.int16)
    w = gidx_all.reshape(CORES, tot_slots // 16, 16).transpose(0, 2, 1)
    for rep in range(8):
        gidx_wrap[:, rep * 16 : (rep + 1) * 16, :] = w

    # weights, host-fused and laid out for SBUF tiles
    W_A = W_lin[:BOX]                                 # [1024, 512]
    W_Bc = emb @ W_lin[BOX:]                          # [151, 512]
    W_B = np.zeros((256, HID), dtype=np.float32)
    W_B[:NCLS] = W_Bc
    W_B[NCLS] = b_lin                                 # bias as a weight row
    labT = np.zeros((CORES, 256, NLOC), dtype=np.float32)
    for c in range(CORES):
        loc = lab[c * NLOC : (c + 1) * NLOC]
        labT[c, loc, np.arange(NLOC)] = 1.0
        labT[c, NCLS, :] = 1.0                        # constant-1 bias feature

    def chunk_rows(Wm, p=128):
        # [K, O] -> [128, (K/128)*O] with [p, k*O+o] = Wm[k*128+p, o]
        K, O = Wm.shape
        return Wm.reshape(K // p, p, O).transpose(1, 0, 2).reshape(p, -1)

    W1_t = np.concatenate(
        [chunk_rows(rel_W1[r]) for r in range(R)], axis=1
    )  # [128, 8*4*512]
    RW1_t = chunk_rows(root_W1)                       # [128, 4*512]
    W2_t = np.concatenate(
        [chunk_rows(rel_W2[r]) for r in range(R)], axis=1
    )  # [128, 8*4*256]
    RW2_t = chunk_rows(root_W2)                       # [128, 4*256]
    WC1_t = chunk_rows(Wc1)                           # [128, 2*256]
    WC2_t = chunk_rows(Wc2)                           # [128, 2*2]

    boxT = box.T.copy()                               # [1024 f, 8192 n]

    shared = {
        "W_A": chunk_rows(W_A).astype(bf16),          # [128, 8*512]
        "W_B": chunk_rows(W_B).astype(bf16),          # [128, 2*512]
        "W1": W1_t.astype(bf16),
        "RW1": RW1_t.astype(bf16),
        "W2": W2_t.astype(bf16),
        "RW2": RW2_t.astype(bf16),
        "WC1": WC1_t.astype(bf16),
        "WC2": WC2_t.astype(bf16),
        "b1row": b1.reshape(1, HID).astype(bf16),
        "b2row": b2.reshape(1, OUT).astype(bf16),
        "bc2row": bc2.reshape(1, NT).astype(bf16),
        "bc1row": bc1.reshape(1, OUT).astype(bf16),
        "ones1": np.ones((1, 128), dtype=bf16),
        "ones128": np.full((128, 1), 1.0 / 128.0, dtype=bf16),
        "EYE": np.eye(128, dtype=bf16),
        "IOTA": np.tile(np.arange(128, dtype=bf16), (128, 1)),
    }

    pack_order = ["W_A", "W_B", "W1", "RW1", "W2", "RW2", "WC1", "WC2",
                  "b1row", "b2row", "bc2row", "bc1row", "ones1", "ones128",
                  "EYE", "IOTA"]
    in_maps = []
    offsets = None
    for c in range(CORES):
        m = dict(shared)
        m["boxT"] = np.ascontiguousarray(
            boxT[:, c * NLOC : (c + 1) * NLOC]
        ).reshape(8, 128, NLOC).transpose(1, 0, 2).reshape(128, 8 * NLOC).astype(bf16)
        m["labT"] = (
            labT[c].reshape(2, 128, NLOC).transpose(1, 0, 2).reshape(128, 2 * NLOC)
        ).astype(bf16)
        m["DCOLW"] = dcol_all[c].astype(bf16)        # [128, totc]
        m["INVRB"] = invrb[c].astype(bf16)           # [128, NB*R*128]
        # pack every bf16 tensor into one flat blob (single DRAM parameter:
        # keeps host-side per-parameter binding overhead out of the NEFF)
        parts = ["boxT", "labT", "DCOLW", "INVRB"] + pack_order
        offs = {}
        cur = 0
        bufs = []
        for name in parts:
            a = np.ascontiguousarray(m[name], dtype=bf16)
            offs[name] = (cur, a.shape)
            bufs.append(a.reshape(-1))
            cur += a.size
        blob = np.concatenate(bufs)
        if offsets is None:
            offsets = offs
        in_maps.append({"blob": blob.reshape(1, -1), "GIDX": gidx_wrap[c]})

    meta = {"chunks": tuple(tuple(int(x) for x in row) for row in chunks),
            "fp8": FP8, "offsets": offsets,
            "blob_elems": int(in_maps[0]["blob"].size)}
    return in_maps, meta


def build(meta, split_waits=True):
    _patch_tile()
    import concourse.bass as bass
    import concourse.mybir as mybir
    import concourse.tile as tile
    from concourse import library_config
    from concourse.bass import _add_dep_helper

    dt = mybir.dt
    AF = mybir.ActivationFunctionType
    use_fp8 = meta.get("fp8", False)
    gdt = dt.float8e4 if use_fp8 else dt.bfloat16
    chunks = meta["chunks"]
    totc = sum(sum(row) for row in chunks)
    tot_slots = totc * 128

    nc = bass.Bass()

    # ---- parameters: one packed bf16 blob + the int16 gather indices ----
    offsets = meta["offsets"]
    blob = nc.declare_dram_parameter(
        "blob", [1, meta["blob_elems"]], dt.bfloat16, isOutput=False
    )

    class _ParamViews(dict):
        def __missing__(self, name):
            off, shape = offsets[name]
            p, c = shape
            ap = blob.ap()[:, off : off + p * c]
            ap = ap.rearrange("a (p c) -> (a p) c", p=p)
            v = _View(ap)
            self[name] = v
            return v

    class _View:
        def __init__(self, ap):
            self._ap = ap
        def ap(self):
            return self._ap
        def __getitem__(self, idx):
            return self._ap[idx]

    P = _ParamViews()
    P["GIDX"] = nc.declare_dram_parameter(
        "GIDX", [128, tot_slots // 16], dt.int16, isOutput=False
    )
    out_y = nc.declare_dram_parameter("out", [2, GPC, NT], dt.float32, isOutput=True)

    # ---- internal DRAM ----
    xg_loc = nc.dram_tensor("xg_loc", [NLOC, HID], gdt)
    xfull = nc.dram_tensor("xfull", [N, HID], gdt, addr_space="Shared")
    hg_loc = nc.dram_tensor("hg_loc", [NLOC, HID], gdt)
    hfull = nc.dram_tensor("hfull", [N, HID], gdt, addr_space="Shared")

    rg = [list(range(CORES))]

    with tile.TileContext(nc) as tc:
        with (
            tc.tile_pool(name="wpool", bufs=1) as wpool,
            tc.tile_pool(name="xpool", bufs=1) as xpool,
            tc.tile_pool(name="spool", bufs=2) as spool,
            tc.tile_pool(name="gpool", bufs=2) as gpool,
            tc.tile_pool(name="ohpool", bufs=2) as ohpool,
            tc.tile_pool(name="invpool", bufs=2) as invpool,
            tc.tile_pool(name="stpool", bufs=2) as stpool,
            tc.tile_pool(name="psA", bufs=3, space="PSUM") as psA,
            tc.tile_pool(name="psB", bufs=2, space="PSUM") as psB,
            tc.tile_pool(name="psC", bufs=1, space="PSUM") as psC,
        ):
            # GPSIMD ucode library providing DMAGatherAnt; every dma_gather
            # gets an explicit dep edge on this load.
            liblod = nc.gpsimd.load_library(library_config.mlp)

            def load(name, shape, dtype=dt.bfloat16, pool=wpool):
                t = pool.tile(list(shape), dtype, tag=name)
                nc.sync.dma_start(t[:], P[name].ap())
                return t

            # ---- stage-1 loads: only what featurize + the first gathers
            # need, so the all-gather is issued as early as possible ----
            fpool_cm = tc.tile_pool(name="fpool", bufs=1)
            fpool = fpool_cm.__enter__()
            boxT_t = load("boxT", (128, 8 * NLOC), pool=fpool)
            labT_t = load("labT", (128, 2 * NLOC), pool=fpool)
            W_A_t = load("W_A", (128, 8 * HID), pool=fpool)
            W_B_t = load("W_B", (128, 2 * HID), pool=fpool)
            GIDX_t = load("GIDX", (128, tot_slots // 16), dt.int16)
            DCOLW_t = load("DCOLW", (128, totc))
            IOTA_t = load("IOTA", (128, 128))

            # featurize: x0 = [box, onehot(lab)] @ W (+bias via weight row)
            x0N = xpool.tile([128, NB, HID], dt.bfloat16, tag="x0N")
            x0G = xpool.tile([128, NB, HID], gdt, tag="x0G")
            for nb in range(NB):
                ps = psA.tile([128, HID], dt.float32, tag="ps512")
                for k in range(8):
                    nc.tensor.matmul(
                        ps[:],
                        boxT_t[:, k * NLOC + nb * 128 : k * NLOC + (nb + 1) * 128],
                        W_A_t[:, k * HID : (k + 1) * HID],
                        start=(k == 0),
                        stop=False,
                    )
                for k in range(2):
                    nc.tensor.matmul(
                        ps[:],
                        labT_t[:, k * NLOC + nb * 128 : k * NLOC + (nb + 1) * 128],
                        W_B_t[:, k * HID : (k + 1) * HID],
                        start=False,
                        stop=(k == 1),
                    )
                nc.scalar.activation(x0N[:, nb, :], ps[:], AF.Copy)
                nc.vector.tensor_copy(x0G[:, nb, :], ps[:])
            xg_dma = nc.sync.dma_start(
                xg_loc.ap().rearrange("(nb p) f -> p nb f", p=128), x0G[:]
            )
            fpool_cm.__exit__(None, None, None)

            # ---- all-gather x0 (issued before the heavy weight loads so the
            # loads stream in under the collective) ----
            nc.gpsimd.collective_compute(
                "AllGather",
                mybir.AluOpType.bypass,
                replica_groups=rg,
                ins=[xg_loc.ap().opt()],
                outs=[xfull.ap().opt()],
            )

            # ---- stage-2 loads: overlap the collective. Explicit dep on the
            # x0 writeout keeps these big copies from jumping ahead of it on
            # the DMA engines and delaying the all-gather start. ----
            def load2(name, shape, dtype=dt.bfloat16):
                t = wpool.tile(list(shape), dtype, tag=name)
                d = nc.sync.dma_start(t[:], P[name].ap())
                _add_dep_helper(d.ins, xg_dma.ins, reason="defer to stage 2")
                return t

            W1_t = load2("W1", (128, R * 4 * HID))
            RW1_t = load2("RW1", (128, 4 * HID))
            EYE_t = load2("EYE", (128, 128))
            b1_t = load2("b1row", (1, HID))
            ones1_t = load2("ones1", (1, 128))
            W2_t = load2("W2", (128, R * 4 * OUT))
            RW2_t = load2("RW2", (128, 4 * OUT))
            b2_t = load2("b2row", (1, OUT))
            WC1_t = load2("WC1", (128, 2 * OUT))
            WC2_t = load2("WC2", (128, 2 * NT))
            bc1_t = load2("bc1row", (1, OUT))
            bc2_t = load2("bc2row", (1, NT))
            ones128_t = load2("ones128", (128, 1))

            def layer(src_full, xN, W_t, RW_t, brow_t, odim, act, out_tile,
                      g_out=None):
                """One RGCN conv layer. out_tile: [128, NB, odim] bf16.
                xN: [128, NB, HID] node-major input (for the root term)."""
                coff = 0  # chunk offset into the global stream
                for b in range(NB):
                    chb = [chunks[b][r] for r in range(R)]
                    nch = sum(chb)
                    # gather this block's edge rows in one call
                    gt = gpool.tile([128, nch, HID], gdt, tag="gt")
                    g_ins = nc.gpsimd.dma_gather(
                        gt[:],
                        src_full.ap(),
                        GIDX_t[:, coff * 8 : (coff + nch) * 8],
                        num_idxs=nch * 128,
                        num_idxs_reg=nch * 128,
                        elem_size=HID,
                        single_packet=False,
                    )
                    _add_dep_helper(
                        g_ins.ins, liblod.ins,
                        reason="dma_gather needs mlp library",
                    )
                    # build this block's binary one-hot on DVE: (IOTA == dcol)
                    # in fp8 (1.0/0.0 exact; padding dcol=255 never matches)
                    ohb = ohpool.tile([128, nch, 128], gdt, tag="ohb")
                    dc_ap = DCOLW_t[:, coff : coff + nch].rearrange(
                        "p (c u) -> p c u", u=1
                    )
                    io_ap = IOTA_t[:].rearrange("p (u d) -> p u d", u=1)
                    dc_b, io_b = bass.broadcast_tensor_aps(dc_ap, io_ap)
                    nc.vector.tensor_tensor(
                        ohb[:], io_b, dc_b, mybir.AluOpType.is_equal
                    )
                    # per-(relation, dst) 1/cnt rows (row-replicated, bf16)
                    invb = invpool.tile([128, R * 128], dt.bfloat16, tag="invb")
                    nc.sync.dma_start(
                        invb[:], P["INVRB"][:, b * R * 128 : (b + 1) * R * 128]
                    )
                    # segment sums, transposed: S.T[f, dst] per relation; fp8
                    # DoubleRow contracts two 128-slot chunks per matmul.
                    st = []
                    j0 = 0
                    for r in range(R):
                        nchr = chb[r]
                        if nchr == 0:
                            st.append(None)
                            continue
                        pss = psA.tile([128, HID], dt.float32, tag="ps512")
                        # one accumulation group for the whole bank: start only
                        # on the very first matmul, stop only on the last, with
                        # explicit edges to pin starter-first / stopper-last.
                        starter = None
                        last_by_fc = {}
                        npair = nchr // 2
                        tail = nchr % 2
                        nsteps = npair + tail
                        for s in range(nsteps):
                            j = j0 + 2 * s
                            is_pair = s < npair
                            for fc in range(4):
                                if is_pair:
                                    mm = nc.tensor.matmul(
                                        pss[:, fc * 128 : (fc + 1) * 128],
                                        gt[:, j : j + 2, fc * 128 : (fc + 1) * 128],
                                        ohb[:, j : j + 2, :],
                                        start=(s == 0 and fc == 0),
                                        stop=(s == nsteps - 1 and fc == 3),
                                        perf_mode=mybir.MatmulPerfMode.DoubleRow,
                                    )
                                else:
                                    mm = nc.tensor.matmul(
                                        pss[:, fc * 128 : (fc + 1) * 128],
                                        gt[:, j, fc * 128 : (fc + 1) * 128],
                                        ohb[:, j, :],
                                        start=(s == 0 and fc == 0),
                                        stop=(s == nsteps - 1 and fc == 3),
                                    )
                                if starter is None:
                                    starter = mm
                                elif s == 0:
                                    _add_dep_helper(
                                        mm.ins, starter.ins,
                                        reason="psum group starter first",
                                    )
                                if s == nsteps - 1:
                                    last_by_fc[fc] = mm
                        for fc in range(3):
                            _add_dep_helper(
                                last_by_fc[3].ins, last_by_fc[fc].ins,
                                reason="psum group stopper last",
                            )
                        # st = pss * (1/cnt)[dst], bf16, on DVE (exact bf16
                        # normalization, same numerics as the old folded OH)
                        stt = stpool.tile([128, HID], dt.bfloat16, tag=f"st{r}")
                        pss3 = pss[:].rearrange("p (c d) -> p c d", d=128)
                        inv3 = invb[:, r * 128 : (r + 1) * 128].rearrange(
                            "p (u d) -> p u d", u=1
                        )
                        inv_b, _ = bass.broadcast_tensor_aps(inv3, pss3)
                        nc.vector.tensor_tensor(
                            stt[:].rearrange("p (c d) -> p c d", d=128),
                            pss3,
                            inv_b,
                            mybir.AluOpType.mult,
                        )
                        st.append(stt)
                        j0 += nchr
                    coff += nch
                    # root term via PE transpose: x_block^T in st layout
                    psr = psA.tile([128, HID], dt.float32, tag="ps512")
                    for fc in range(4):
                        nc.tensor.matmul(
                            psr[:, fc * 128 : (fc + 1) * 128],
                            xN[:, b, fc * 128 : (fc + 1) * 128],
                            EYE_t[:],
                            start=(fc == 0),
                            stop=(fc == 3),
                        )
                    str_t = stpool.tile([128, HID], dt.bfloat16, tag="stroot")
                    nc.scalar.activation(str_t[:], psr[:], AF.Copy)
                    # transform: h[dst, :] = sum_r S_r.T.T @ W_r + x.T.T @ RW + b
                    pst = psB.tile([128, odim], dt.float32, tag="transps")
                    nc.tensor.matmul(
                        pst[:], ones1_t[:1, :], brow_t[:1, :odim],
                        start=True, stop=False,
                    )
                    for r in range(R):
                        if st[r] is None:
                            continue
                        for k in range(4):
                            nc.tensor.matmul(
                                pst[:],
                                st[r][:, k * 128 : (k + 1) * 128],
                                W_t[:, (r * 4 + k) * odim : (r * 4 + k + 1) * odim],
                                start=False,
                                stop=False,
                            )
                    for k in range(4):
                        nc.tensor.matmul(
                            pst[:],
                            str_t[:, k * 128 : (k + 1) * 128],
                            RW_t[:, k * odim : (k + 1) * odim],
                            start=False,
                            stop=(k == 3),
                        )
                    nc.scalar.activation(out_tile[:, b, :], pst[:], act)
                    if g_out is not None:
                        nc.vector.tensor_copy(g_out[:, b, :], pst[:])

            # ---- layer 1 ----
            h1N = xpool.tile([128, NB, HID], dt.bfloat16, tag="h1N")
            h1G = xpool.tile([128, NB, HID], gdt, tag="h1G")
            layer(xfull, x0N, W1_t, RW1_t, b1_t, HID, AF.Relu, h1N)
            for nb in range(NB):
                nc.vector.tensor_copy(h1G[:, nb, :], h1N[:, nb, :])
            nc.sync.dma_start(
                hg_loc.ap().rearrange("(nb p) f -> p nb f", p=128), h1G[:]
            )
            nc.gpsimd.collective_compute(
                "AllGather",
                mybir.AluOpType.bypass,
                replica_groups=rg,
                ins=[hg_loc.ap().opt()],
                outs=[hfull.ap().opt()],
            )

            # ---- layer 2 ----
            o2N = xpool.tile([128, NB, OUT], dt.bfloat16, tag="o2N")
            layer(hfull, h1N, W2_t, RW2_t, b2_t, OUT, AF.Copy, o2N)

            # ---- mean pooling (per graph = per node-block), transposed ----
            psp = psC.tile([128, 16], dt.float32, tag="poolps")
            for g in range(GPC):
                for fc in range(2):
                    nc.tensor.matmul(
                        psp[:, fc * 8 + g : fc * 8 + g + 1],
                        o2N[:, g, fc * 128 : (fc + 1) * 128],
                        ones128_t[:],
                        start=True,
                        stop=True,
                    )
            pooledT = spool.tile([128, 16], dt.bfloat16, tag="pooledT")
            nc.vector.tensor_copy(pooledT[:], psp[:])

            # ---- classifier ----
            zT = spool.tile([128, 16], dt.bfloat16, tag="zT")
            for f2c in range(2):
                psz = psC.tile([128, 8], dt.float32, tag="zps")
                nc.tensor.matmul(
                    psz[:],
                    bc1_t[:1, f2c * 128 : (f2c + 1) * 128],
                    ones1_t[:1, :8],
                    start=True,
                    stop=False,
                )
                for f1c in range(2):
                    nc.tensor.matmul(
                        psz[:],
                        WC1_t[:, f1c * OUT + f2c * 128 : f1c * OUT + (f2c + 1) * 128],
                        pooledT[:, f1c * 8 : (f1c + 1) * 8],
                        start=False,
                        stop=(f1c == 1),
                    )
                nc.scalar.activation(zT[:, f2c * 8 : (f2c + 1) * 8], psz[:], AF.Copy)
            psy = psC.tile([8, NT], dt.float32, tag="yps")
            for f2c in range(2):
                nc.tensor.matmul(
                    psy[:],
                    zT[:, f2c * 8 : (f2c + 1) * 8],
                    WC2_t[:, f2c * NT : (f2c + 1) * NT],
                    start=(f2c == 0),
                    stop=False,
                )
            nc.tensor.matmul(
                psy[:], ones1_t[:1, :GPC], bc2_t[:1, :], start=False, stop=True
            )
            ylog = spool.tile([8, NT], dt.float32, tag="ylog")
            nc.vector.tensor_copy(ylog[:], psy[:])
            negmax = spool.tile([8, 1], dt.float32, tag="negmax")
            nc.vector.reduce_max(
                negmax[:], ylog[:], axis=mybir.AxisListType.X, negate=True
            )
            ey = spool.tile([8, NT], dt.float32, tag="ey")
            nc.scalar.activation(ey[:], ylog[:], AF.Exp, bias=negmax[:])
            ssum = spool.tile([8, 1], dt.float32, tag="ssum")
            nc.vector.reduce_sum(ssum[:], ey[:], axis=mybir.AxisListType.X)
            rinv = spool.tile([8, 1], dt.float32, tag="rinv")
            nc.vector.reciprocal(rinv[:], ssum[:])
            yprob = spool.tile([8, NT], dt.float32, tag="yprob")
            nc.vector.tensor_scalar_mul(yprob[:], ey[:], rinv[:])

            nc.sync.dma_start(out_y[0], ylog[:])
            nc.sync.dma_start(out_y[1], yprob[:])

    mybir.codegen_inst_isa_subclasses(nc)
    if split_waits:
        _split_excess_waits(nc)
    return nc


_CACHE = {}


def _get_built(inputs):
    in_maps, meta = prep(inputs)
    key = meta["chunks"]
    if key not in _CACHE:
        _CACHE[key] = build(meta)
    return _CACHE[key], in_maps, meta


def kernel(**inputs):
    nc, in_maps, meta = _get_built(inputs)
    from concourse.bass_utils import run_bass_kernel_spmd

    res = run_bass_kernel_spmd(nc, in_maps, core_ids=list(range(CORES)))
    ylog = np.concatenate([res.results[c]["out"][0] for c in range(CORES)], axis=0)
    yprob = np.concatenate([res.results[c]["out"][1] for c in range(CORES)], axis=0)
    return ylog.astype(np.float32), yprob.astype(np.float32)


# revision 26
# speedup vs baseline: 2.7978x; 1.2342x over previous
"""Trainium2 Bass kernel for a 2-layer RGCN scene-graph model (8 NeuronCores).

Sharding: node/dst-parallel. Core c owns nodes [c*1024, (c+1)*1024) (= 8 whole
graphs of 128 nodes, so pooling is local). Relation weights are replicated.
Message aggregation is mean-per-(relation, dst); we aggregate x first (the
relation transform is linear, so mean-then-transform == transform-then-mean)
which keeps all matmul work sharded. The segment mean is computed on the
TensorEngine as (gathered edge rows)^T @ one-hot, where the one-hot carries
1/cnt; edge rows are fetched with dma_gather from the all-gathered node
features in DRAM. The root-weight term x@RW is folded into the same pipeline
by a per-block PE transpose (matmul against an identity "one-hot"), so node
features never round-trip through DRAM for transposition. All integer index
metadata (edge sort, one-hot matrices, counts) is precomputed on host inside
kernel().
"""

import sys

sys.path.insert(0, "/opt/trn_rl_repo")

import numpy as np
import ml_dtypes

bf16 = ml_dtypes.bfloat16
fp8 = ml_dtypes.float8_e4m3
FP8 = True  # gather path in float8_e4m3 (halves AG + gather bytes)

N = 8192
E = 262144
R = 8
NCLS = 151
EMB = 256
BOX = 1024
HID = 512
OUT = 256
NT = 2
CORES = 8
NLOC = N // CORES          # 1024 nodes per core
NB = NLOC // 128           # 8 dst blocks of 128 per core
GPC = NLOC // 128          # graphs per core (nodes_per_graph == 128)

_PATCHED = False


def _patch_tile():
    """This container's walrus rejects >2 sync-wait commands per instruction;
    TileContext's kernel-tail drain attaches one wait per active logical proc.
    Redistribute the drain's waits over event-sem instructions (2 each)."""
    global _PATCHED
    if _PATCHED:
        return
    import concourse.mybir as mybir
    import concourse.tile as tile
    from concourse.vector_clock import ScopedClock

    def _drain_and_barrier(self, tick_clock, wait_clock):
        nc = self.nc
        drain_inst = nc.sync.drain()
        wait_clock.add_sem_waits(
            drain_inst.ins, ScopedClock({None: tick_clock.global_clock})
        )
        si = drain_inst.ins.sync_info
        waits = list(si.on_wait) if si is not None else []
        if waits:
            drain_inst.ins.sync_info = mybir.SyncInfo(
                on_wait=[], on_update=list(si.on_update) if si else []
            )
            dummy_sem = nc.alloc_semaphore(f"tail_split_sem_{nc.next_id()}")
            for i in range(0, len(waits), 2):
                ev = nc.sync.wait_ge(dummy_sem, 0)
                evsi = ev.ins.sync_info
                ev.ins.sync_info = mybir.SyncInfo(
                    on_wait=waits[i : i + 2],
                    on_update=list(evsi.on_update) if evsi else [],
                )
        nc.all_engine_barrier()
        assert self.sems is not None
        popped = nc._tile_sem_poison_stack.pop()
        assert popped is self._sem_poison
        nc.clear_and_free_semaphores(list(self.sems.allocated().values()))
        nc.all_engine_barrier()

    tile.TileContext._drain_and_barrier = _drain_and_barrier
    _PATCHED = True


def _split_excess_waits(nc, max_waits=2):
    """This walrus build rejects instructions carrying more than 2 sync-wait
    commands, but Tile's wait-assignment pass can attach more. Move excess
    waits onto same-engine EventSemaphore instructions inserted just before
    the over-subscribed instruction."""
    import concourse.mybir as mybir

    counter = [0]
    for f in nc.m.functions:
        for bb in f.blocks:
            cur = list(bb.instructions)
            out = []
            changed = False
            for ins in cur:
                si = ins.sync_info
                waits = list(si.on_wait) if si is not None else []
                allowed = (
                    max_waits
                    if type(ins).__name__ == "InstEventSemaphore"
                    else 1
                )
                if len(waits) > allowed:
                    keep = waits[:allowed]
                    extra = waits[allowed:]
                    ins.sync_info = mybir.SyncInfo(
                        on_wait=keep, on_update=list(si.on_update)
                    )
                    for i in range(0, len(extra), max_waits):
                        counter[0] += 1
                        ev = mybir.InstEventSemaphore(
                            name=f"I-wsplit-{counter[0]}",
                            ins=[],
                            outs=[],
                            engine=ins.engine,
                        )
                        ev.sync_info = mybir.SyncInfo(
                            on_wait=extra[i : i + max_waits], on_update=[]
                        )
                        out.append(ev)
                    changed = True
                out.append(ins)
            if changed:
                bb.instructions = out


def prep(inputs):
    """Host preprocessing: returns (in_maps, meta). meta['chunks'] is the
    uniform [NB][R] chunk table that parameterizes the program structure."""
    box = np.asarray(inputs["box_features"], dtype=np.float32)
    lab = np.asarray(inputs["pred_labels"]).astype(np.int64).reshape(-1)
    ei = np.asarray(inputs["edge_index"]).astype(np.int64)
    et = np.asarray(inputs["edge_type"]).astype(np.int64).reshape(-1)
    emb = np.asarray(inputs["emb_table"], dtype=np.float32)
    W_lin = np.asarray(inputs["W_lin"], dtype=np.float32)
    b_lin = np.asarray(inputs["b_lin"], dtype=np.float32)
    rel_W1 = np.asarray(inputs["rel_W1"], dtype=np.float32)
    root_W1 = np.asarray(inputs["root_W1"], dtype=np.float32)
    b1 = np.asarray(inputs["b1"], dtype=np.float32)
    rel_W2 = np.asarray(inputs["rel_W2"], dtype=np.float32)
    root_W2 = np.asarray(inputs["root_W2"], dtype=np.float32)
    b2 = np.asarray(inputs["b2"], dtype=np.float32)
    Wc1 = np.asarray(inputs["Wc1"], dtype=np.float32)
    bc1 = np.asarray(inputs["bc1"], dtype=np.float32)
    Wc2 = np.asarray(inputs["Wc2"], dtype=np.float32)
    bc2 = np.asarray(inputs["bc2"], dtype=np.float32)

    src, dst = ei[0], ei[1]

    # per-(relation, dst) in-degree -> 1/cnt, folded into the one-hot values
    cnt = np.bincount(et * N + dst, minlength=R * N).reshape(R, N)
    inv = (1.0 / np.maximum(cnt, 1)).astype(np.float32)

    core_of = dst // NLOC
    blk = (dst % NLOC) // 128

    # uniform chunk counts per (block, relation): max over cores
    key = (core_of * NB + blk) * R + et
    ecnt = np.bincount(key, minlength=CORES * NB * R).reshape(CORES, NB, R)
    chunks = np.ceil(ecnt / 128).astype(np.int64).max(axis=0)  # [NB, R]
    totc = int(chunks.sum())
    tot_slots = totc * 128

    # slot offset of each (b, r) group (group-major order: b outer, r inner)
    goff = np.zeros((NB, R), dtype=np.int64)
    acc = 0
    for b in range(NB):
        for r in range(R):
            goff[b, r] = acc
            acc += int(chunks[b, r]) * 128

    # per-core gather index + one-hot construction
    order = np.lexsort((et, blk, core_of))  # sort by (core, blk, r)
    s_src = src[order]
    s_dst = dst[order]
    s_et = et[order]
    s_core = core_of[order]
    s_blk = blk[order]
    s_inv = inv[s_et, s_dst]

    gidx_all = np.zeros((CORES, tot_slots), dtype=np.int16)
    # compact one-hot encoding: per slot, the dst column (0..127); the binary
    # [slot, dst] one-hot matrix is built on-device by DVE as (IOTA == dcol)
    # in fp8 (exact); padding slots get dcol=255 which never matches. The
    # 1/cnt mean normalization is applied afterwards in bf16 via INVRB.
    dcol_all = np.full((CORES, 128, totc), 255.0, dtype=np.float32)

    # slot position of each sorted edge: group offset + rank within group
    grp_key = (s_core * NB + s_blk) * R + s_et
    # rank within group via cumcount on sorted keys
    new_grp = np.empty(E, dtype=bool)
    new_grp[0] = True
    new_grp[1:] = grp_key[1:] != grp_key[:-1]
    grp_start = np.flatnonzero(new_grp)
    start_of = np.repeat(grp_start, np.diff(np.append(grp_start, E)))
    rank = np.arange(E) - start_of
    slot = goff[s_blk, s_et] + rank  # slot within the core's stream
    chunk_of = slot // 128
    srow = slot % 128
    dcol = s_dst % 128

    c_idx = s_core
    gidx_all[c_idx, slot] = s_src.astype(np.int16)
    dcol_all[c_idx, srow, chunk_of] = dcol

    # 1/cnt per (block, relation, dst-col), row-replicated for DVE use:
    # invrb[c, :, (b*R+r)*128 + d] = inv[r, c*NLOC + b*128 + d]
    invrb = np.zeros((CORES, 128, NB * R * 128), dtype=np.float32)
    for c in range(CORES):
        for b in range(NB):
            for r in range(R):
                invrb[c, :, (b * R + r) * 128 : (b * R + r + 1) * 128] = inv[
                    r, c * NLOC + b * 128 : c * NLOC + (b + 1) * 128
                ][None, :]

    # wrapped gather indices: position i -> [i % 16, i // 16], replicated x8
    gidx_wrap = np.zeros((CORES, 128, tot_slots // 16), dtype=np.int16)
    w = gidx_all.reshape(CORES, tot_slots // 16, 16).transpose(0, 2, 1)
    for rep in range(8):
        gidx_wrap[:, rep * 16 : (rep + 1) * 16, :] = w

    # weights, host-fused and laid out for SBUF tiles
    W_A = W_lin[:BOX]                                 # [1024, 512]
    W_Bc = emb @ W_lin[BOX:]                          # [151, 512]
    W_B = np.zeros((256, HID), dtype=np.float32)
    W_B[:NCLS] = W_Bc
    W_B[NCLS] = b_lin                                 # bias as a weight row
    labT = np.zeros((CORES, 256, NLOC), dtype=np.float32)
    for c in range(CORES):
        loc = lab[c * NLOC : (c + 1) * NLOC]
        labT[c, loc, np.arange(NLOC)] = 1.0
        labT[c, NCLS, :] = 1.0                        # constant-1 bias feature

    def chunk_rows(Wm, p=128):
        # [K, O] -> [128, (K/128)*O] with [p, k*O+o] = Wm[k*128+p, o]
        K, O = Wm.shape
        return Wm.reshape(K // p, p, O).transpose(1, 0, 2).reshape(p, -1)

    W1_t = np.concatenate(
        [chunk_rows(rel_W1[r]) for r in range(R)], axis=1
    )  # [128, 8*4*512]
    RW1_t = chunk_rows(root_W1)                       # [128, 4*512]
    W2_t = np.concatenate(
        [chunk_rows(rel_W2[r]) for r in range(R)], axis=1
    )  # [128, 8*4*256]
    RW2_t = chunk_rows(root_W2)                       # [128, 4*256]
    WC1_t = chunk_rows(Wc1)                           # [128, 2*256]
    WC2_t = chunk_rows(Wc2)                           # [128, 2*2]

    boxT = box.T.copy()                               # [1024 f, 8192 n]

    shared = {
        "W_A": chunk_rows(W_A).astype(bf16),          # [128, 8*512]
        "W_B": chunk_rows(W_B).astype(bf16),          # [128, 2*512]
        "W1": W1_t.astype(bf16),
        "RW1": RW1_t.astype(bf16),
        "W2": W2_t.astype(bf16),
        "RW2": RW2_t.astype(bf16),
        "WC1": WC1_t.astype(bf16),
        "WC2": WC2_t.astype(bf16),
        "b1row": b1.reshape(1, HID).astype(bf16),
        "b2row": b2.reshape(1, OUT).astype(bf16),
        "bc2row": bc2.reshape(1, NT).astype(bf16),
        "bc1row": bc1.reshape(1, OUT).astype(bf16),
        "ones1": np.ones((1, 128), dtype=bf16),
        "ones128": np.full((128, 1), 1.0 / 128.0, dtype=bf16),
        "EYE": np.eye(128, dtype=bf16),
        "IOTA": np.tile(np.arange(128, dtype=bf16), (128, 1)),
    }

    pack_order = ["W_A", "W_B", "W1", "RW1", "W2", "RW2", "WC1", "WC2",
                  "b1row", "b2row", "bc2row", "bc1row", "ones1", "ones128",
                  "EYE", "IOTA"]
    in_maps = []
    offsets = None
    for c in range(CORES):
        m = dict(shared)
        m["boxT"] = np.ascontiguousarray(
            boxT[:, c * NLOC : (c + 1) * NLOC]
        ).reshape(8, 128, NLOC).transpose(1, 0, 2).reshape(128, 8 * NLOC).astype(bf16)
        m["labT"] = (
            labT[c].reshape(2, 128, NLOC).transpose(1, 0, 2).reshape(128, 2 * NLOC)
        ).astype(bf16)
        m["DCOLW"] = dcol_all[c].astype(bf16)        # [128, totc]
        m["INVRB"] = invrb[c].astype(bf16)           # [128, NB*R*128]
        # pack every bf16 tensor into one flat blob (single DRAM parameter:
        # keeps host-side per-parameter binding overhead out of the NEFF)
        parts = ["boxT", "labT", "DCOLW", "INVRB"] + pack_order
        offs = {}
        cur = 0
        bufs = []
        for name in parts:
            a = np.ascontiguousarray(m[name], dtype=bf16)
            offs[name] = (cur, a.shape)
            bufs.append(a.reshape(-1))
            cur += a.size
        blob = np.concatenate(bufs)
        if offsets is None:
            offsets = offs
        in_maps.append({"blob": blob.reshape(1, -1), "GIDX": gidx_wrap[c]})

    meta = {"chunks": tuple(tuple(int(x) for x in row) for row in chunks),
            "fp8": FP8, "offsets": offsets,
            "blob_elems": int(in_maps[0]["blob"].size)}
    return in_maps, meta


def build(meta, split_waits=True):
    _patch_tile()
    import concourse.bass as bass
    import concourse.mybir as mybir
    import concourse.tile as tile
    from concourse import library_config
    from concourse.bass import _add_dep_helper

    dt = mybir.dt
    AF = mybir.ActivationFunctionType
    use_fp8 = meta.get("fp8", False)
    gdt = dt.float8e4 if use_fp8 else dt.bfloat16
    chunks = meta["chunks"]
    totc = sum(sum(row) for row in chunks)
    tot_slots = totc * 128

    nc = bass.Bass()

    # ---- parameters: one packed bf16 blob + the int16 gather indices ----
    offsets = meta["offsets"]
    blob = nc.declare_dram_parameter(
        "blob", [1, meta["blob_elems"]], dt.bfloat16, isOutput=False
    )

    class _ParamViews(dict):
        def __missing__(self, name):
            off, shape = offsets[name]
            p, c = shape
            ap = blob.ap()[:, off : off + p * c]
            ap = ap.rearrange("a (p c) -> (a p) c", p=p)
            v = _View(ap)
            self[name] = v
            return v

    class _View:
        def __init__(self, ap):
            self._ap = ap
        def ap(self):
            return self._ap
        def __getitem__(self, idx):
            return self._ap[idx]

    P = _ParamViews()
    P["GIDX"] = nc.declare_dram_parameter(
        "GIDX", [128, tot_slots // 16], dt.int16, isOutput=False
    )
    out_y = nc.declare_dram_parameter("out", [2, GPC, NT], dt.float32, isOutput=True)

    # ---- internal DRAM ----
    xg_loc = nc.dram_tensor("xg_loc", [NLOC, HID], gdt)
    xfull = nc.dram_tensor("xfull", [N, HID], gdt, addr_space="Shared")
    hg_loc = nc.dram_tensor("hg_loc", [NLOC, HID], gdt)
    hfull = nc.dram_tensor("hfull", [N, HID], gdt, addr_space="Shared")

    rg = [list(range(CORES))]

    with tile.TileContext(nc) as tc:
        with (
            tc.tile_pool(name="wpool", bufs=1) as wpool,
            tc.tile_pool(name="xpool", bufs=1) as xpool,
            tc.tile_pool(name="spool", bufs=2) as spool,
            tc.tile_pool(name="gpool", bufs=2) as gpool,
            tc.tile_pool(name="ohpool", bufs=2) as ohpool,
            tc.tile_pool(name="invpool", bufs=2) as invpool,
            tc.tile_pool(name="stpool", bufs=2) as stpool,
            tc.tile_pool(name="psA", bufs=3, space="PSUM") as psA,
            tc.tile_pool(name="psB", bufs=2, space="PSUM") as psB,
            tc.tile_pool(name="psC", bufs=1, space="PSUM") as psC,
        ):
            # GPSIMD ucode library providing DMAGatherAnt; every dma_gather
            # gets an explicit dep edge on this load.
            liblod = nc.gpsimd.load_library(library_config.mlp)

            def load(name, shape, dtype=dt.bfloat16, pool=wpool):
                t = pool.tile(list(shape), dtype, tag=name)
                nc.sync.dma_start(t[:], P[name].ap())
                return t

            # ---- stage-1 loads: only what featurize + the first gathers
            # need, so the all-gather is issued as early as possible ----
            fpool_cm = tc.tile_pool(name="fpool", bufs=1)
            fpool = fpool_cm.__enter__()
            boxT_t = load("boxT", (128, 8 * NLOC), pool=fpool)
            labT_t = load("labT", (128, 2 * NLOC), pool=fpool)
            W_A_t = load("W_A", (128, 8 * HID), pool=fpool)
            W_B_t = load("W_B", (128, 2 * HID), pool=fpool)
            GIDX_t = load("GIDX", (128, tot_slots // 16), dt.int16)
            DCOLW_t = load("DCOLW", (128, totc))
            IOTA_t = load("IOTA", (128, 128))

            # featurize: x0 = [box, onehot(lab)] @ W (+bias via weight row)
            x0N = xpool.tile([128, NB, HID], dt.bfloat16, tag="x0N")
            x0G = xpool.tile([128, NB, HID], gdt, tag="x0G")
            for nb in range(NB):
                ps = psA.tile([128, HID], dt.float32, tag="ps512")
                for k in range(8):
                    nc.tensor.matmul(
                        ps[:],
                        boxT_t[:, k * NLOC + nb * 128 : k * NLOC + (nb + 1) * 128],
                        W_A_t[:, k * HID : (k + 1) * HID],
                        start=(k == 0),
                        stop=False,
                    )
                for k in range(2):
                    nc.tensor.matmul(
                        ps[:],
                        labT_t[:, k * NLOC + nb * 128 : k * NLOC + (nb + 1) * 128],
                        W_B_t[:, k * HID : (k + 1) * HID],
                        start=False,
                        stop=(k == 1),
                    )
                nc.scalar.activation(x0N[:, nb, :], ps[:], AF.Copy)
                nc.vector.tensor_copy(x0G[:, nb, :], ps[:])
            xg_dma = nc.sync.dma_start(
                xg_loc.ap().rearrange("(nb p) f -> p nb f", p=128), x0G[:]
            )
            fpool_cm.__exit__(None, None, None)

            # ---- all-gather x0 (issued before the heavy weight loads so the
            # loads stream in under the collective) ----
            nc.gpsimd.collective_compute(
                "AllGather",
                mybir.AluOpType.bypass,
                replica_groups=rg,
                ins=[xg_loc.ap().opt()],
                outs=[xfull.ap().opt()],
            )

            # ---- stage-2 loads: overlap the collective. Explicit dep on the
            # x0 writeout keeps these big copies from jumping ahead of it on
            # the DMA engines and delaying the all-gather start. ----
            def load2(name, shape, dtype=dt.bfloat16):
                t = wpool.tile(list(shape), dtype, tag=name)
                d = nc.sync.dma_start(t[:], P[name].ap())
                _add_dep_helper(d.ins, xg_dma.ins, reason="defer to stage 2")
                return t

            W1_t = load2("W1", (128, R * 4 * HID))
            RW1_t = load2("RW1", (128, 4 * HID))
            EYE_t = load2("EYE", (128, 128))
            b1_t = load2("b1row", (1, HID))
            ones1_t = load2("ones1", (1, 128))
            W2_t = load2("W2", (128, R * 4 * OUT))
            RW2_t = load2("RW2", (128, 4 * OUT))
            b2_t = load2("b2row", (1, OUT))
            WC1_t = load2("WC1", (128, 2 * OUT))
            WC2_t = load2("WC2", (128, 2 * NT))
            bc1_t = load2("bc1row", (1, OUT))
            bc2_t = load2("bc2row", (1, NT))
            ones128_t = load2("ones128", (128, 1))

            # chunk offset of each block in the global stream
            boff = [0] * (NB + 1)
            for b in range(NB):
                boff[b + 1] = boff[b] + sum(chunks[b])

            def prefetch(src_full, b):
                """Issue block b's gather + one-hot build + inv load."""
                coff = boff[b]
                nch = boff[b + 1] - coff
                # gather this block's edge rows in one call
                gt = gpool.tile([128, nch, HID], gdt, tag="gt")
                g_ins = nc.gpsimd.dma_gather(
                    gt[:],
                    src_full.ap(),
                    GIDX_t[:, coff * 8 : (coff + nch) * 8],
                    num_idxs=nch * 128,
                    num_idxs_reg=nch * 128,
                    elem_size=HID,
                    single_packet=False,
                )
                _add_dep_helper(
                    g_ins.ins, liblod.ins,
                    reason="dma_gather needs mlp library",
                )
                # build this block's binary one-hot on DVE: (IOTA == dcol)
                # in fp8 (1.0/0.0 exact; padding dcol=255 never matches)
                ohb = ohpool.tile([128, nch, 128], gdt, tag="ohb")
                dc_ap = DCOLW_t[:, coff : coff + nch].rearrange(
                    "p (c u) -> p c u", u=1
                )
                io_ap = IOTA_t[:].rearrange("p (u d) -> p u d", u=1)
                dc_b, io_b = bass.broadcast_tensor_aps(dc_ap, io_ap)
                nc.vector.tensor_tensor(
                    ohb[:], io_b, dc_b, mybir.AluOpType.is_equal
                )
                # per-(relation, dst) 1/cnt rows (row-replicated, bf16)
                invb = invpool.tile([128, R * 128], dt.bfloat16, tag="invb")
                nc.sync.dma_start(
                    invb[:], P["INVRB"][:, b * R * 128 : (b + 1) * R * 128]
                )
                return gt, ohb, invb

            def layer(src_full, xN, W_t, RW_t, brow_t, odim, act, out_tile,
                      g_out=None):
                """One RGCN conv layer. out_tile: [128, NB, odim] bf16.
                xN: [128, NB, HID] node-major input (for the root term).
                Block b+1's gather/one-hot prefetch issues before block b's
                compute so the engines pipeline one block ahead."""
                pf = prefetch(src_full, 0)
                for b in range(NB):
                    gt, ohb, invb = pf
                    if b + 1 < NB:
                        pf = prefetch(src_full, b + 1)
                    chb = [chunks[b][r] for r in range(R)]
                    # segment sums, transposed: S.T[f, dst] per relation; fp8
                    # DoubleRow contracts two 128-slot chunks per matmul.
                    st = []
                    j0 = 0
                    for r in range(R):
                        nchr = chb[r]
                        if nchr == 0:
                            st.append(None)
                            continue
                        pss = psA.tile([128, HID], dt.float32, tag="ps512")
                        # one accumulation group for the whole bank: start only
                        # on the very first matmul, stop only on the last, with
                        # explicit edges to pin starter-first / stopper-last.
                        starter = None
                        last_by_fc = {}
                        npair = nchr // 2
                        tail = nchr % 2
                        nsteps = npair + tail
                        for s in range(nsteps):
                            j = j0 + 2 * s
                            is_pair = s < npair
                            for fc in range(4):
                                if is_pair:
                                    mm = nc.tensor.matmul(
                                        pss[:, fc * 128 : (fc + 1) * 128],
                                        gt[:, j : j + 2, fc * 128 : (fc + 1) * 128],
                                        ohb[:, j : j + 2, :],
                                        start=(s == 0 and fc == 0),
                                        stop=(s == nsteps - 1 and fc == 3),
                                        perf_mode=mybir.MatmulPerfMode.DoubleRow,
                                    )
                                else:
                                    mm = nc.tensor.matmul(
                                        pss[:, fc * 128 : (fc + 1) * 128],
                                        gt[:, j, fc * 128 : (fc + 1) * 128],
                                        ohb[:, j, :],
                                        start=(s == 0 and fc == 0),
                                        stop=(s == nsteps - 1 and fc == 3),
                                    )
                                if starter is None:
                                    starter = mm
                                elif s == 0:
                                    _add_dep_helper(
                                        mm.ins, starter.ins,
                                        reason="psum group starter first",
                                    )
                                if s == nsteps - 1:
                                    last_by_fc[fc] = mm
                        for fc in range(3):
                            _add_dep_helper(
                                last_by_fc[3].ins, last_by_fc[fc].ins,
                                reason="psum group stopper last",
                            )
                        # st = pss * (1/cnt)[dst], bf16, on DVE (exact bf16
                        # normalization, same numerics as the old folded OH)
                        stt = stpool.tile([128, HID], dt.bfloat16, tag=f"st{r}")
                        pss3 = pss[:].rearrange("p (c d) -> p c d", d=128)
                        inv3 = invb[:, r * 128 : (r + 1) * 128].rearrange(
                            "p (u d) -> p u d", u=1
                        )
                        inv_b, _ = bass.broadcast_tensor_aps(inv3, pss3)
                        nc.vector.tensor_tensor(
                            stt[:].rearrange("p (c d) -> p c d", d=128),
                            pss3,
                            inv_b,
                            mybir.AluOpType.mult,
                        )
                        st.append(stt)
                        j0 += nchr
                    # root term via PE transpose: x_block^T in st layout
                    psr = psA.tile([128, HID], dt.float32, tag="ps512")
                    for fc in range(4):
                        nc.tensor.matmul(
                            psr[:, fc * 128 : (fc + 1) * 128],
                            xN[:, b, fc * 128 : (fc + 1) * 128],
                            EYE_t[:],
                            start=(fc == 0),
                            stop=(fc == 3),
                        )
                    str_t = stpool.tile([128, HID], dt.bfloat16, tag="stroot")
                    nc.scalar.activation(str_t[:], psr[:], AF.Copy)
                    # transform: h[dst, :] = sum_r S_r.T.T @ W_r + x.T.T @ RW + b
                    pst = psB.tile([128, odim], dt.float32, tag="transps")
                    nc.tensor.matmul(
                        pst[:], ones1_t[:1, :], brow_t[:1, :odim],
                        start=True, stop=False,
                    )
                    for r in range(R):
                        if st[r] is None:
                            continue
                        for k in range(4):
                            nc.tensor.matmul(
                                pst[:],
                                st[r][:, k * 128 : (k + 1) * 128],
                                W_t[:, (r * 4 + k) * odim : (r * 4 + k + 1) * odim],
                                start=False,
                                stop=False,
                            )
                    for k in range(4):
                        nc.tensor.matmul(
                            pst[:],
                            str_t[:, k * 128 : (k + 1) * 128],
                            RW_t[:, k * odim : (k + 1) * odim],
                            start=False,
                            stop=(k == 3),
                        )
                    nc.scalar.activation(out_tile[:, b, :], pst[:], act)
                    if g_out is not None:
                        # post-activation fp8 copy per block (feeds the next
                        # all-gather without a post-layer copy tail)
                        nc.vector.tensor_copy(g_out[:, b, :], out_tile[:, b, :])

            # ---- layer 1 ----
            h1N = xpool.tile([128, NB, HID], dt.bfloat16, tag="h1N")
            h1G = xpool.tile([128, NB, HID], gdt, tag="h1G")
            layer(xfull, x0N, W1_t, RW1_t, b1_t, HID, AF.Relu, h1N, g_out=h1G)
            nc.sync.dma_start(
                hg_loc.ap().rearrange("(nb p) f -> p nb f", p=128), h1G[:]
            )
            nc.gpsimd.collective_compute(
                "AllGather",
                mybir.AluOpType.bypass,
                replica_groups=rg,
                ins=[hg_loc.ap().opt()],
                outs=[hfull.ap().opt()],
            )

            # ---- layer 2 ----
            o2N = xpool.tile([128, NB, OUT], dt.bfloat16, tag="o2N")
            layer(hfull, h1N, W2_t, RW2_t, b2_t, OUT, AF.Copy, o2N)

            # ---- mean pooling (per graph = per node-block), transposed ----
            psp = psC.tile([128, 16], dt.float32, tag="poolps")
            for g in range(GPC):
                for fc in range(2):
                    nc.tensor.matmul(
                        psp[:, fc * 8 + g : fc * 8 + g + 1],
                        o2N[:, g, fc * 128 : (fc + 1) * 128],
                        ones128_t[:],
                        start=True,
                        stop=True,
                    )
            pooledT = spool.tile([128, 16], dt.bfloat16, tag="pooledT")
            nc.vector.tensor_copy(pooledT[:], psp[:])

            # ---- classifier ----
            zT = spool.tile([128, 16], dt.bfloat16, tag="zT")
            for f2c in range(2):
                psz = psC.tile([128, 8], dt.float32, tag="zps")
                nc.tensor.matmul(
                    psz[:],
                    bc1_t[:1, f2c * 128 : (f2c + 1) * 128],
                    ones1_t[:1, :8],
                    start=True,
                    stop=False,
                )
                for f1c in range(2):
                    nc.tensor.matmul(
                        psz[:],
                        WC1_t[:, f1c * OUT + f2c * 128 : f1c * OUT + (f2c + 1) * 128],
                        pooledT[:, f1c * 8 : (f1c + 1) * 8],
                        start=False,
                        stop=(f1c == 1),
                    )
                nc.scalar.activation(zT[:, f2c * 8 : (f2c + 1) * 8], psz[:], AF.Copy)
            psy = psC.tile([8, NT], dt.float32, tag="yps")
            for f2c in range(2):
                nc.tensor.matmul(
                    psy[:],
                    zT[:, f2c * 8 : (f2c + 1) * 8],
                    WC2_t[:, f2c * NT : (f2c + 1) * NT],
                    start=(f2c == 0),
                    stop=False,
                )
            nc.tensor.matmul(
                psy[:], ones1_t[:1, :GPC], bc2_t[:1, :], start=False, stop=True
            )
            ylog = spool.tile([8, NT], dt.float32, tag="ylog")
            nc.vector.tensor_copy(ylog[:], psy[:])
            negmax = spool.tile([8, 1], dt.float32, tag="negmax")
            nc.vector.reduce_max(
                negmax[:], ylog[:], axis=mybir.AxisListType.X, negate=True
            )
            ey = spool.tile([8, NT], dt.float32, tag="ey")
            nc.scalar.activation(ey[:], ylog[:], AF.Exp, bias=negmax[:])
            ssum = spool.tile([8, 1], dt.float32, tag="ssum")
            nc.vector.reduce_sum(ssum[:], ey[:], axis=mybir.AxisListType.X)
            rinv = spool.tile([8, 1], dt.float32, tag="rinv")
            nc.vector.reciprocal(rinv[:], ssum[:])
            yprob = spool.tile([8, NT], dt.float32, tag="yprob")
            nc.vector.tensor_scalar_mul(yprob[:], ey[:], rinv[:])

            nc.sync.dma_start(out_y[0], ylog[:])
            nc.sync.dma_start(out_y[1], yprob[:])

    mybir.codegen_inst_isa_subclasses(nc)
    if split_waits:
        _split_excess_waits(nc)
    return nc


_CACHE = {}


def _get_built(inputs):
    in_maps, meta = prep(inputs)
    key = meta["chunks"]
    if key not in _CACHE:
        _CACHE[key] = build(meta)
    return _CACHE[key], in_maps, meta


def kernel(**inputs):
    nc, in_maps, meta = _get_built(inputs)
    from concourse.bass_utils import run_bass_kernel_spmd

    res = run_bass_kernel_spmd(nc, in_maps, core_ids=list(range(CORES)))
    ylog = np.concatenate([res.results[c]["out"][0] for c in range(CORES)], axis=0)
    yprob = np.concatenate([res.results[c]["out"][1] for c in range(CORES)], axis=0)
    return ylog.astype(np.float32), yprob.astype(np.float32)


# revision 31
# speedup vs baseline: 3.0125x; 1.0767x over previous
"""Trainium2 Bass kernel for a 2-layer RGCN scene-graph model (8 NeuronCores).

Sharding: node/dst-parallel. Core c owns nodes [c*1024, (c+1)*1024) (= 8 whole
graphs of 128 nodes, so pooling is local). Relation weights are replicated.
Message aggregation is mean-per-(relation, dst); we aggregate x first (the
relation transform is linear, so mean-then-transform == transform-then-mean)
which keeps all matmul work sharded. The segment mean is computed on the
TensorEngine as (gathered edge rows)^T @ one-hot, where the one-hot carries
1/cnt; edge rows are fetched with dma_gather from the all-gathered node
features in DRAM. The root-weight term x@RW is folded into the same pipeline
by a per-block PE transpose (matmul against an identity "one-hot"), so node
features never round-trip through DRAM for transposition. All integer index
metadata (edge sort, one-hot matrices, counts) is precomputed on host inside
kernel().
"""

import sys

sys.path.insert(0, "/opt/trn_rl_repo")

import numpy as np
import ml_dtypes

bf16 = ml_dtypes.bfloat16
fp8 = ml_dtypes.float8_e4m3
FP8 = True  # gather path in float8_e4m3 (halves AG + gather bytes)

N = 8192
E = 262144
R = 8
NCLS = 151
EMB = 256
BOX = 1024
HID = 512
OUT = 256
NT = 2
CORES = 8
NLOC = N // CORES          # 1024 nodes per core
NB = NLOC // 128           # 8 dst blocks of 128 per core
GPC = NLOC // 128          # graphs per core (nodes_per_graph == 128)

_PATCHED = False


def _patch_tile():
    """This container's walrus rejects >2 sync-wait commands per instruction;
    TileContext's kernel-tail drain attaches one wait per active logical proc.
    Redistribute the drain's waits over event-sem instructions (2 each)."""
    global _PATCHED
    if _PATCHED:
        return
    import concourse.mybir as mybir
    import concourse.tile as tile
    from concourse.vector_clock import ScopedClock

    def _drain_and_barrier(self, tick_clock, wait_clock):
        nc = self.nc
        drain_inst = nc.sync.drain()
        wait_clock.add_sem_waits(
            drain_inst.ins, ScopedClock({None: tick_clock.global_clock})
        )
        si = drain_inst.ins.sync_info
        waits = list(si.on_wait) if si is not None else []
        if waits:
            drain_inst.ins.sync_info = mybir.SyncInfo(
                on_wait=[], on_update=list(si.on_update) if si else []
            )
            dummy_sem = nc.alloc_semaphore(f"tail_split_sem_{nc.next_id()}")
            for i in range(0, len(waits), 2):
                ev = nc.sync.wait_ge(dummy_sem, 0)
                evsi = ev.ins.sync_info
                ev.ins.sync_info = mybir.SyncInfo(
                    on_wait=waits[i : i + 2],
                    on_update=list(evsi.on_update) if evsi else [],
                )
        nc.all_engine_barrier()
        assert self.sems is not None
        popped = nc._tile_sem_poison_stack.pop()
        assert popped is self._sem_poison
        nc.clear_and_free_semaphores(list(self.sems.allocated().values()))
        nc.all_engine_barrier()

    tile.TileContext._drain_and_barrier = _drain_and_barrier
    _PATCHED = True


def _split_excess_waits(nc, max_waits=2):
    """This walrus build rejects instructions carrying more than 2 sync-wait
    commands, but Tile's wait-assignment pass can attach more. Move excess
    waits onto same-engine EventSemaphore instructions inserted just before
    the over-subscribed instruction."""
    import concourse.mybir as mybir

    counter = [0]
    for f in nc.m.functions:
        for bb in f.blocks:
            cur = list(bb.instructions)
            out = []
            changed = False
            for ins in cur:
                si = ins.sync_info
                waits = list(si.on_wait) if si is not None else []
                allowed = (
                    max_waits
                    if type(ins).__name__ == "InstEventSemaphore"
                    else 1
                )
                if len(waits) > allowed:
                    keep = waits[:allowed]
                    extra = waits[allowed:]
                    ins.sync_info = mybir.SyncInfo(
                        on_wait=keep, on_update=list(si.on_update)
                    )
                    for i in range(0, len(extra), max_waits):
                        counter[0] += 1
                        ev = mybir.InstEventSemaphore(
                            name=f"I-wsplit-{counter[0]}",
                            ins=[],
                            outs=[],
                            engine=ins.engine,
                        )
                        ev.sync_info = mybir.SyncInfo(
                            on_wait=extra[i : i + max_waits], on_update=[]
                        )
                        out.append(ev)
                    changed = True
                out.append(ins)
            if changed:
                bb.instructions = out


def prep(inputs):
    """Host preprocessing: returns (in_maps, meta). meta['chunks'] is the
    uniform [NB][R] chunk table that parameterizes the program structure."""
    box = np.asarray(inputs["box_features"], dtype=np.float32)
    lab = np.asarray(inputs["pred_labels"]).astype(np.int64).reshape(-1)
    ei = np.asarray(inputs["edge_index"]).astype(np.int64)
    et = np.asarray(inputs["edge_type"]).astype(np.int64).reshape(-1)
    emb = np.asarray(inputs["emb_table"], dtype=np.float32)
    W_lin = np.asarray(inputs["W_lin"], dtype=np.float32)
    b_lin = np.asarray(inputs["b_lin"], dtype=np.float32)
    rel_W1 = np.asarray(inputs["rel_W1"], dtype=np.float32)
    root_W1 = np.asarray(inputs["root_W1"], dtype=np.float32)
    b1 = np.asarray(inputs["b1"], dtype=np.float32)
    rel_W2 = np.asarray(inputs["rel_W2"], dtype=np.float32)
    root_W2 = np.asarray(inputs["root_W2"], dtype=np.float32)
    b2 = np.asarray(inputs["b2"], dtype=np.float32)
    Wc1 = np.asarray(inputs["Wc1"], dtype=np.float32)
    bc1 = np.asarray(inputs["bc1"], dtype=np.float32)
    Wc2 = np.asarray(inputs["Wc2"], dtype=np.float32)
    bc2 = np.asarray(inputs["bc2"], dtype=np.float32)

    src, dst = ei[0], ei[1]

    # per-(relation, dst) in-degree -> 1/cnt, folded into the one-hot values
    cnt = np.bincount(et * N + dst, minlength=R * N).reshape(R, N)
    inv = (1.0 / np.maximum(cnt, 1)).astype(np.float32)

    core_of = dst // NLOC
    blk = (dst % NLOC) // 128

    # uniform chunk counts per (block, relation): max over cores
    key = (core_of * NB + blk) * R + et
    ecnt = np.bincount(key, minlength=CORES * NB * R).reshape(CORES, NB, R)
    chunks = np.ceil(ecnt / 128).astype(np.int64).max(axis=0)  # [NB, R]
    totc = int(chunks.sum())
    tot_slots = totc * 128

    # slot offset of each (b, r) group (group-major order: b outer, r inner)
    goff = np.zeros((NB, R), dtype=np.int64)
    acc = 0
    for b in range(NB):
        for r in range(R):
            goff[b, r] = acc
            acc += int(chunks[b, r]) * 128

    # per-core gather index + one-hot construction
    order = np.lexsort((et, blk, core_of))  # sort by (core, blk, r)
    s_src = src[order]
    s_dst = dst[order]
    s_et = et[order]
    s_core = core_of[order]
    s_blk = blk[order]
    s_inv = inv[s_et, s_dst]

    gidx_all = np.zeros((CORES, tot_slots), dtype=np.int16)
    # compact one-hot encoding: per slot, the dst column (0..127); the binary
    # [slot, dst] one-hot matrix is built on-device by DVE as (IOTA == dcol)
    # in fp8 (exact); padding slots get dcol=255 which never matches. The
    # 1/cnt mean normalization is applied afterwards in bf16 via INVRB.
    dcol_all = np.full((CORES, 128, totc), 255.0, dtype=np.float32)

    # slot position of each sorted edge: group offset + rank within group
    grp_key = (s_core * NB + s_blk) * R + s_et
    # rank within group via cumcount on sorted keys
    new_grp = np.empty(E, dtype=bool)
    new_grp[0] = True
    new_grp[1:] = grp_key[1:] != grp_key[:-1]
    grp_start = np.flatnonzero(new_grp)
    start_of = np.repeat(grp_start, np.diff(np.append(grp_start, E)))
    rank = np.arange(E) - start_of
    slot = goff[s_blk, s_et] + rank  # slot within the core's stream
    chunk_of = slot // 128
    srow = slot % 128
    dcol = s_dst % 128

    c_idx = s_core
    gidx_all[c_idx, slot] = s_src.astype(np.int16)
    dcol_all[c_idx, srow, chunk_of] = dcol

    # 1/cnt per (block, relation, dst-col), row-replicated for DVE use:
    # invrb[c, :, (b*R+r)*128 + d] = inv[r, c*NLOC + b*128 + d]
    invrb = np.zeros((CORES, 128, NB * R * 128), dtype=np.float32)
    for c in range(CORES):
        for b in range(NB):
            for r in range(R):
                invrb[c, :, (b * R + r) * 128 : (b * R + r + 1) * 128] = inv[
                    r, c * NLOC + b * 128 : c * NLOC + (b + 1) * 128
                ][None, :]

    # wrapped gather indices: position i -> [i % 16, i // 16], replicated x8
    gidx_wrap = np.zeros((CORES, 128, tot_slots // 16), dtype=np.int16)
    w = gidx_all.reshape(CORES, tot_slots // 16, 16).transpose(0, 2, 1)
    for rep in range(8):
        gidx_wrap[:, rep * 16 : (rep + 1) * 16, :] = w

    # weights, host-fused and laid out for SBUF tiles
    W_A = W_lin[:BOX]                                 # [1024, 512]
    W_Bc = emb @ W_lin[BOX:]                          # [151, 512]
    W_B = np.zeros((256, HID), dtype=np.float32)
    W_B[:NCLS] = W_Bc
    W_B[NCLS] = b_lin                                 # bias as a weight row
    labT = np.zeros((CORES, 256, NLOC), dtype=np.float32)
    for c in range(CORES):
        loc = lab[c * NLOC : (c + 1) * NLOC]
        labT[c, loc, np.arange(NLOC)] = 1.0
        labT[c, NCLS, :] = 1.0                        # constant-1 bias feature

    def chunk_rows(Wm, p=128):
        # [K, O] -> [128, (K/128)*O] with [p, k*O+o] = Wm[k*128+p, o]
        K, O = Wm.shape
        return Wm.reshape(K // p, p, O).transpose(1, 0, 2).reshape(p, -1)

    W1_t = np.concatenate(
        [chunk_rows(rel_W1[r]) for r in range(R)], axis=1
    )  # [128, 8*4*512]
    RW1_t = chunk_rows(root_W1)                       # [128, 4*512]
    W2_t = np.concatenate(
        [chunk_rows(rel_W2[r]) for r in range(R)], axis=1
    )  # [128, 8*4*256]
    RW2_t = chunk_rows(root_W2)                       # [128, 4*256]
    WC1_t = chunk_rows(Wc1)                           # [128, 2*256]
    WC2_t = chunk_rows(Wc2)                           # [128, 2*2]

    boxT = box.T.copy()                               # [1024 f, 8192 n]

    shared = {
        "W_A": chunk_rows(W_A).astype(bf16),          # [128, 8*512]
        "W_B": chunk_rows(W_B).astype(bf16),          # [128, 2*512]
        "W1": W1_t.astype(bf16),
        "RW1": RW1_t.astype(bf16),
        "W2": W2_t.astype(bf16),
        "RW2": RW2_t.astype(bf16),
        "WC1": WC1_t.astype(bf16),
        "WC2": WC2_t.astype(bf16),
        "b1row": b1.reshape(1, HID).astype(bf16),
        "b2row": b2.reshape(1, OUT).astype(bf16),
        "bc2row": bc2.reshape(1, NT).astype(bf16),
        "bc1row": bc1.reshape(1, OUT).astype(bf16),
        "ones1": np.ones((1, 128), dtype=bf16),
        "ones128": np.full((128, 1), 1.0 / 128.0, dtype=bf16),
        "EYE": np.eye(128, dtype=bf16),
        "IOTA": np.tile(np.arange(128, dtype=bf16), (128, 1)),
    }

    pack_order = ["W_A", "W_B", "W1", "RW1", "W2", "RW2", "WC1", "WC2",
                  "b1row", "b2row", "bc2row", "bc1row", "ones1", "ones128",
                  "EYE", "IOTA"]
    in_maps = []
    offsets = None
    for c in range(CORES):
        m = dict(shared)
        m["boxT"] = np.ascontiguousarray(
            boxT[:, c * NLOC : (c + 1) * NLOC]
        ).reshape(8, 128, NLOC).transpose(1, 0, 2).reshape(128, 8 * NLOC).astype(bf16)
        m["labT"] = (
            labT[c].reshape(2, 128, NLOC).transpose(1, 0, 2).reshape(128, 2 * NLOC)
        ).astype(bf16)
        m["DCOLW"] = dcol_all[c].astype(bf16)        # [128, totc]
        m["INVRB"] = invrb[c].astype(bf16)           # [128, NB*R*128]
        # pack every bf16 tensor into one flat blob (single DRAM parameter:
        # keeps host-side per-parameter binding overhead out of the NEFF)
        parts = ["boxT", "labT", "DCOLW", "INVRB"] + pack_order
        offs = {}
        cur = 0
        bufs = []
        for name in parts:
            a = np.ascontiguousarray(m[name], dtype=bf16)
            offs[name] = (cur, a.shape)
            bufs.append(a.reshape(-1))
            cur += a.size
        blob = np.concatenate(bufs)
        if offsets is None:
            offsets = offs
        in_maps.append({"blob": blob.reshape(1, -1), "GIDX": gidx_wrap[c]})

    meta = {"chunks": tuple(tuple(int(x) for x in row) for row in chunks),
            "fp8": FP8, "offsets": offsets,
            "blob_elems": int(in_maps[0]["blob"].size)}
    return in_maps, meta


def build(meta, split_waits=True):
    _patch_tile()
    import concourse.bass as bass
    import concourse.mybir as mybir
    import concourse.tile as tile
    from concourse import library_config
    from concourse.bass import _add_dep_helper

    dt = mybir.dt
    AF = mybir.ActivationFunctionType
    use_fp8 = meta.get("fp8", False)
    gdt = dt.float8e4 if use_fp8 else dt.bfloat16
    chunks = meta["chunks"]
    totc = sum(sum(row) for row in chunks)
    tot_slots = totc * 128

    nc = bass.Bass()

    # ---- parameters: one packed bf16 blob + the int16 gather indices ----
    offsets = meta["offsets"]
    blob = nc.declare_dram_parameter(
        "blob", [1, meta["blob_elems"]], dt.bfloat16, isOutput=False
    )

    class _ParamViews(dict):
        def __missing__(self, name):
            off, shape = offsets[name]
            p, c = shape
            ap = blob.ap()[:, off : off + p * c]
            ap = ap.rearrange("a (p c) -> (a p) c", p=p)
            v = _View(ap)
            self[name] = v
            return v

    class _View:
        def __init__(self, ap):
            self._ap = ap
        def ap(self):
            return self._ap
        def __getitem__(self, idx):
            return self._ap[idx]

    P = _ParamViews()
    P["GIDX"] = nc.declare_dram_parameter(
        "GIDX", [128, tot_slots // 16], dt.int16, isOutput=False
    )
    out_y = nc.declare_dram_parameter("out", [2, GPC, NT], dt.float32, isOutput=True)

    # ---- internal DRAM ----
    xg_loc = nc.dram_tensor("xg_loc", [NLOC, HID], gdt)
    xfull = nc.dram_tensor("xfull", [N, HID], gdt, addr_space="Shared")
    hg_loc = nc.dram_tensor("hg_loc", [NLOC, HID], gdt)
    hfull = nc.dram_tensor("hfull", [N, HID], gdt, addr_space="Shared")

    rg = [list(range(CORES))]

    with tile.TileContext(nc) as tc:
        with (
            tc.tile_pool(name="wpool", bufs=1) as wpool,
            tc.tile_pool(name="xpool", bufs=1) as xpool,
            tc.tile_pool(name="spool", bufs=2) as spool,
            tc.tile_pool(name="gpool", bufs=2) as gpool,
            tc.tile_pool(name="ohpool", bufs=2) as ohpool,
            tc.tile_pool(name="invpool", bufs=2) as invpool,
            tc.tile_pool(name="stpool", bufs=2) as stpool,
            tc.tile_pool(name="rcpool", bufs=1) as rcpool,
            tc.tile_pool(name="psA", bufs=3, space="PSUM") as psA,
            tc.tile_pool(name="psB", bufs=2, space="PSUM") as psB,
            tc.tile_pool(name="psC", bufs=1, space="PSUM") as psC,
        ):
            # GPSIMD ucode library providing DMAGatherAnt; every dma_gather
            # gets an explicit dep edge on this load.
            liblod = nc.gpsimd.load_library(library_config.mlp)

            def load(name, shape, dtype=dt.bfloat16, pool=wpool):
                t = pool.tile(list(shape), dtype, tag=name)
                nc.sync.dma_start(t[:], P[name].ap())
                return t

            # ---- stage-1 loads: only what featurize + the first gathers
            # need, so the all-gather is issued as early as possible ----
            fpool_cm = tc.tile_pool(name="fpool", bufs=1)
            fpool = fpool_cm.__enter__()
            boxT_t = load("boxT", (128, 8 * NLOC), pool=fpool)
            labT_t = load("labT", (128, 2 * NLOC), pool=fpool)
            W_A_t = load("W_A", (128, 8 * HID), pool=fpool)
            W_B_t = load("W_B", (128, 2 * HID), pool=fpool)
            GIDX_t = load("GIDX", (128, tot_slots // 16), dt.int16)
            DCOLW_t = load("DCOLW", (128, totc))
            IOTA_t = load("IOTA", (128, 128))

            # featurize: x0 = [box, onehot(lab)] @ W (+bias via weight row)
            x0N = xpool.tile([128, NB, HID], dt.bfloat16, tag="x0N")
            x0G = xpool.tile([128, NB, HID], gdt, tag="x0G")
            for nb in range(NB):
                ps = psA.tile([128, HID], dt.float32, tag="ps512")
                for k in range(8):
                    nc.tensor.matmul(
                        ps[:],
                        boxT_t[:, k * NLOC + nb * 128 : k * NLOC + (nb + 1) * 128],
                        W_A_t[:, k * HID : (k + 1) * HID],
                        start=(k == 0),
                        stop=False,
                    )
                for k in range(2):
                    nc.tensor.matmul(
                        ps[:],
                        labT_t[:, k * NLOC + nb * 128 : k * NLOC + (nb + 1) * 128],
                        W_B_t[:, k * HID : (k + 1) * HID],
                        start=False,
                        stop=(k == 1),
                    )
                nc.scalar.activation(x0N[:, nb, :], ps[:], AF.Copy)
                nc.vector.tensor_copy(x0G[:, nb, :], ps[:])
            xg_dma = nc.sync.dma_start(
                xg_loc.ap().rearrange("(nb p) f -> p nb f", p=128), x0G[:]
            )
            fpool_cm.__exit__(None, None, None)

            # ---- all-gather x0 (issued before the heavy weight loads so the
            # loads stream in under the collective) ----
            nc.gpsimd.collective_compute(
                "AllGather",
                mybir.AluOpType.bypass,
                replica_groups=rg,
                ins=[xg_loc.ap().opt()],
                outs=[xfull.ap().opt()],
            )

            # ---- stage-2 loads: overlap the collective. Explicit dep on the
            # x0 writeout keeps these big copies from jumping ahead of it on
            # the DMA engines and delaying the all-gather start. ----
            def load2(name, shape, dtype=dt.bfloat16):
                t = wpool.tile(list(shape), dtype, tag=name)
                d = nc.sync.dma_start(t[:], P[name].ap())
                _add_dep_helper(d.ins, xg_dma.ins, reason="defer to stage 2")
                return t

            W1_t = load2("W1", (128, R * 4 * HID))
            RW1_t = load2("RW1", (128, 4 * HID))
            EYE_t = load2("EYE", (128, 128))
            b1_t = load2("b1row", (1, HID))
            ones1_t = load2("ones1", (1, 128))
            W2_t = load2("W2", (128, R * 4 * OUT))
            RW2_t = load2("RW2", (128, 4 * OUT))
            b2_t = load2("b2row", (1, OUT))
            WC1_t = load2("WC1", (128, 2 * OUT))
            WC2_t = load2("WC2", (128, 2 * NT))
            bc1_t = load2("bc1row", (1, OUT))
            bc2_t = load2("bc2row", (1, NT))
            ones128_t = load2("ones128", (128, 1))

            # chunk offset of each block in the global stream
            boff = [0] * (NB + 1)
            for b in range(NB):
                boff[b + 1] = boff[b] + sum(chunks[b])

            def prefetch(src_full, b):
                """Issue block b's gather + one-hot build + inv load."""
                coff = boff[b]
                nch = boff[b + 1] - coff
                # gather this block's edge rows in one call
                gt = gpool.tile([128, nch, HID], gdt, tag="gt")
                g_ins = nc.gpsimd.dma_gather(
                    gt[:],
                    src_full.ap(),
                    GIDX_t[:, coff * 8 : (coff + nch) * 8],
                    num_idxs=nch * 128,
                    num_idxs_reg=nch * 128,
                    elem_size=HID,
                    single_packet=False,
                )
                _add_dep_helper(
                    g_ins.ins, liblod.ins,
                    reason="dma_gather needs mlp library",
                )
                # build this block's binary one-hot on DVE: (IOTA == dcol)
                # in fp8 (1.0/0.0 exact; padding dcol=255 never matches)
                ohb = ohpool.tile([128, nch, 128], gdt, tag="ohb")
                dc_ap = DCOLW_t[:, coff : coff + nch].rearrange(
                    "p (c u) -> p c u", u=1
                )
                io_ap = IOTA_t[:].rearrange("p (u d) -> p u d", u=1)
                dc_b, io_b = bass.broadcast_tensor_aps(dc_ap, io_ap)
                nc.vector.tensor_tensor(
                    ohb[:], io_b, dc_b, mybir.AluOpType.is_equal
                )
                # per-(relation, dst) 1/cnt rows (row-replicated, bf16)
                invb = invpool.tile([128, R * 128], dt.bfloat16, tag="invb")
                nc.sync.dma_start(
                    invb[:], P["INVRB"][:, b * R * 128 : (b + 1) * R * 128]
                )
                return gt, ohb, invb

            def root_prestage(xN, RW_t, brow_t, odim):
                """Compute rc[b] = x_b @ RW + b for every block while the
                all-gather runs (the tensor engine is otherwise idle there).
                Returns the rc tiles; layer() injects them into the transform
                PSUM group with one identity matmul each."""
                rcs = []
                for b in range(NB):
                    psr = psA.tile([128, HID], dt.float32, tag="ps512")
                    for fc in range(4):
                        nc.tensor.matmul(
                            psr[:, fc * 128 : (fc + 1) * 128],
                            xN[:, b, fc * 128 : (fc + 1) * 128],
                            EYE_t[:],
                            start=(fc == 0),
                            stop=(fc == 3),
                        )
                    str_t = stpool.tile([128, HID], dt.bfloat16, tag="stroot")
                    nc.scalar.activation(str_t[:], psr[:], AF.Copy)
                    pr = psB.tile([128, odim], dt.float32, tag="transps")
                    nc.tensor.matmul(
                        pr[:], ones1_t[:1, :], brow_t[:1, :odim],
                        start=True, stop=False,
                    )
                    for k in range(4):
                        nc.tensor.matmul(
                            pr[:],
                            str_t[:, k * 128 : (k + 1) * 128],
                            RW_t[:, k * odim : (k + 1) * odim],
                            start=False,
                            stop=(k == 3),
                        )
                    rc = rcpool.tile([128, odim], dt.bfloat16, tag=f"rc{b}")
                    nc.scalar.activation(rc[:], pr[:], AF.Copy)
                    rcs.append(rc)
                return rcs

            def layer(src_full, rcs, W_t, odim, act, out_tile, g_out=None):
                """One RGCN conv layer. out_tile: [128, NB, odim] bf16.
                rcs: prestaged per-block root+bias tiles.
                Block b+1's gather/one-hot prefetch issues before block b's
                compute so the engines pipeline one block ahead."""
                pf = prefetch(src_full, 0)
                for b in range(NB):
                    gt, ohb, invb = pf
                    if b + 1 < NB:
                        pf = prefetch(src_full, b + 1)
                    chb = [chunks[b][r] for r in range(R)]
                    # segment sums, transposed: S.T[f, dst] per relation; fp8
                    # DoubleRow contracts two 128-slot chunks per matmul.
                    st = []
                    j0 = 0
                    for r in range(R):
                        nchr = chb[r]
                        if nchr == 0:
                            st.append(None)
                            continue
                        pss = psA.tile([128, HID], dt.float32, tag="ps512")
                        # one accumulation group for the whole bank: start only
                        # on the very first matmul, stop only on the last, with
                        # explicit edges to pin starter-first / stopper-last.
                        starter = None
                        last_by_fc = {}
                        npair = nchr // 2
                        tail = nchr % 2
                        nsteps = npair + tail
                        for s in range(nsteps):
                            j = j0 + 2 * s
                            is_pair = s < npair
                            for fc in range(4):
                                if is_pair:
                                    mm = nc.tensor.matmul(
                                        pss[:, fc * 128 : (fc + 1) * 128],
                                        gt[:, j : j + 2, fc * 128 : (fc + 1) * 128],
                                        ohb[:, j : j + 2, :],
                                        start=(s == 0 and fc == 0),
                                        stop=(s == nsteps - 1 and fc == 3),
                                        perf_mode=mybir.MatmulPerfMode.DoubleRow,
                                    )
                                else:
                                    mm = nc.tensor.matmul(
                                        pss[:, fc * 128 : (fc + 1) * 128],
                                        gt[:, j, fc * 128 : (fc + 1) * 128],
                                        ohb[:, j, :],
                                        start=(s == 0 and fc == 0),
                                        stop=(s == nsteps - 1 and fc == 3),
                                    )
                                if starter is None:
                                    starter = mm
                                elif s == 0:
                                    _add_dep_helper(
                                        mm.ins, starter.ins,
                                        reason="psum group starter first",
                                    )
                                if s == nsteps - 1:
                                    last_by_fc[fc] = mm
                        for fc in range(3):
                            _add_dep_helper(
                                last_by_fc[3].ins, last_by_fc[fc].ins,
                                reason="psum group stopper last",
                            )
                        # st = pss * (1/cnt)[dst], bf16, on DVE (exact bf16
                        # normalization, same numerics as the old folded OH)
                        stt = stpool.tile([128, HID], dt.bfloat16, tag=f"st{r}")
                        pss3 = pss[:].rearrange("p (c d) -> p c d", d=128)
                        inv3 = invb[:, r * 128 : (r + 1) * 128].rearrange(
                            "p (u d) -> p u d", u=1
                        )
                        inv_b, _ = bass.broadcast_tensor_aps(inv3, pss3)
                        nc.vector.tensor_tensor(
                            stt[:].rearrange("p (c d) -> p c d", d=128),
                            pss3,
                            inv_b,
                            mybir.AluOpType.mult,
                        )
                        st.append(stt)
                        j0 += nchr
                    # transform: h[dst,:] = sum_r S_r.T.T @ W_r + rc[b]
                    # (rc = root term + bias, prestaged under the all-gather;
                    # injected via one identity matmul that opens the group)
                    pst = psB.tile([128, odim], dt.float32, tag="transps")
                    inj = nc.tensor.matmul(
                        pst[:], EYE_t[:], rcs[b][:],
                        start=True, stop=False,
                    )
                    rk = [
                        (r, k)
                        for r in range(R)
                        if st[r] is not None
                        for k in range(4)
                    ]
                    first = True
                    for r, k in rk:
                        mm = nc.tensor.matmul(
                            pst[:],
                            st[r][:, k * 128 : (k + 1) * 128],
                            W_t[:, (r * 4 + k) * odim : (r * 4 + k + 1) * odim],
                            start=False,
                            stop=((r, k) == rk[-1]),
                        )
                        if first:
                            _add_dep_helper(
                                mm.ins, inj.ins,
                                reason="psum group starter first",
                            )
                            first = False
                    nc.scalar.activation(out_tile[:, b, :], pst[:], act)
                    if g_out is not None:
                        # post-activation fp8 copy per block (feeds the next
                        # all-gather without a post-layer copy tail)
                        nc.vector.tensor_copy(g_out[:, b, :], out_tile[:, b, :])

            # ---- layer 1 (root terms prestaged under all-gather #1) ----
            rcs1 = root_prestage(x0N, RW1_t, b1_t, HID)
            h1N = xpool.tile([128, NB, HID], dt.bfloat16, tag="h1N")
            h1G = xpool.tile([128, NB, HID], gdt, tag="h1G")
            layer(xfull, rcs1, W1_t, HID, AF.Relu, h1N, g_out=h1G)
            nc.sync.dma_start(
                hg_loc.ap().rearrange("(nb p) f -> p nb f", p=128), h1G[:]
            )
            nc.gpsimd.collective_compute(
                "AllGather",
                mybir.AluOpType.bypass,
                replica_groups=rg,
                ins=[hg_loc.ap().opt()],
                outs=[hfull.ap().opt()],
            )

            # ---- layer 2 (root terms prestaged under all-gather #2) ----
            rcs2 = root_prestage(h1N, RW2_t, b2_t, OUT)
            o2N = xpool.tile([128, NB, OUT], dt.bfloat16, tag="o2N")
            layer(hfull, rcs2, W2_t, OUT, AF.Copy, o2N)

            # ---- mean pooling (per graph = per node-block), transposed ----
            psp = psC.tile([128, 16], dt.float32, tag="poolps")
            for g in range(GPC):
                for fc in range(2):
                    nc.tensor.matmul(
                        psp[:, fc * 8 + g : fc * 8 + g + 1],
                        o2N[:, g, fc * 128 : (fc + 1) * 128],
                        ones128_t[:],
                        start=True,
                        stop=True,
                    )
            pooledT = spool.tile([128, 16], dt.bfloat16, tag="pooledT")
            nc.vector.tensor_copy(pooledT[:], psp[:])

            # ---- classifier ----
            zT = spool.tile([128, 16], dt.bfloat16, tag="zT")
            for f2c in range(2):
                psz = psC.tile([128, 8], dt.float32, tag="zps")
                nc.tensor.matmul(
                    psz[:],
                    bc1_t[:1, f2c * 128 : (f2c + 1) * 128],
                    ones1_t[:1, :8],
                    start=True,
                    stop=False,
                )
                for f1c in range(2):
                    nc.tensor.matmul(
                        psz[:],
                        WC1_t[:, f1c * OUT + f2c * 128 : f1c * OUT + (f2c + 1) * 128],
                        pooledT[:, f1c * 8 : (f1c + 1) * 8],
                        start=False,
                        stop=(f1c == 1),
                    )
                nc.scalar.activation(zT[:, f2c * 8 : (f2c + 1) * 8], psz[:], AF.Copy)
            psy = psC.tile([8, NT], dt.float32, tag="yps")
            for f2c in range(2):
                nc.tensor.matmul(
                    psy[:],
                    zT[:, f2c * 8 : (f2c + 1) * 8],
                    WC2_t[:, f2c * NT : (f2c + 1) * NT],
                    start=(f2c == 0),
                    stop=False,
                )
            nc.tensor.matmul(
                psy[:], ones1_t[:1, :GPC], bc2_t[:1, :], start=False, stop=True
            )
            ylog = spool.tile([8, NT], dt.float32, tag="ylog")
            nc.vector.tensor_copy(ylog[:], psy[:])
            negmax = spool.tile([8, 1], dt.float32, tag="negmax")
            nc.vector.reduce_max(
                negmax[:], ylog[:], axis=mybir.AxisListType.X, negate=True
            )
            ey = spool.tile([8, NT], dt.float32, tag="ey")
            nc.scalar.activation(ey[:], ylog[:], AF.Exp, bias=negmax[:])
            ssum = spool.tile([8, 1], dt.float32, tag="ssum")
            nc.vector.reduce_sum(ssum[:], ey[:], axis=mybir.AxisListType.X)
            rinv = spool.tile([8, 1], dt.float32, tag="rinv")
            nc.vector.reciprocal(rinv[:], ssum[:])
            yprob = spool.tile([8, NT], dt.float32, tag="yprob")
            nc.vector.tensor_scalar_mul(yprob[:], ey[:], rinv[:])

            nc.sync.dma_start(out_y[0], ylog[:])
            nc.sync.dma_start(out_y[1], yprob[:])

    mybir.codegen_inst_isa_subclasses(nc)
    if split_waits:
        _split_excess_waits(nc)
    return nc


_CACHE = {}


def _get_built(inputs):
    in_maps, meta = prep(inputs)
    key = meta["chunks"]
    if key not in _CACHE:
        _CACHE[key] = build(meta)
    return _CACHE[key], in_maps, meta


def kernel(**inputs):
    nc, in_maps, meta = _get_built(inputs)
    from concourse.bass_utils import run_bass_kernel_spmd

    res = run_bass_kernel_spmd(nc, in_maps, core_ids=list(range(CORES)))
    ylog = np.concatenate([res.results[c]["out"][0] for c in range(CORES)], axis=0)
    yprob = np.concatenate([res.results[c]["out"][1] for c in range(CORES)], axis=0)
    return ylog.astype(np.float32), yprob.astype(np.float32)


# revision 53
# speedup vs baseline: 3.8539x; 1.2793x over previous
"""Trainium2 Bass kernel for a 2-layer RGCN scene-graph model (8 NeuronCores).

Sharding: node/dst-parallel. Core c owns nodes [c*1024, (c+1)*1024) (= 8 whole
graphs of 128 nodes, so pooling is local). Relation weights are replicated.
Message aggregation is mean-per-(relation, dst); we aggregate x first (the
relation transform is linear, so mean-then-transform == transform-then-mean)
which keeps all matmul work sharded. The segment mean is computed on the
TensorEngine as (gathered edge rows)^T @ one-hot, where the one-hot carries
1/cnt; edge rows are fetched with dma_gather from the all-gathered node
features in DRAM. The root-weight term x@RW is folded into the same pipeline
by a per-block PE transpose (matmul against an identity "one-hot"), so node
features never round-trip through DRAM for transposition. All integer index
metadata (edge sort, one-hot matrices, counts) is precomputed on host inside
kernel().
"""

import sys

sys.path.insert(0, "/opt/trn_rl_repo")

import numpy as np
import ml_dtypes

bf16 = ml_dtypes.bfloat16
fp8 = ml_dtypes.float8_e4m3
FP8 = True  # gather path in float8_e4m3 (halves AG + gather bytes)

N = 8192
E = 262144
R = 8
NCLS = 151
EMB = 256
BOX = 1024
HID = 512
OUT = 256
NT = 2
CORES = 8
NLOC = N // CORES          # 1024 nodes per core
NB = NLOC // 128           # 8 dst blocks of 128 per core
GPC = NLOC // 128          # graphs per core (nodes_per_graph == 128)

_PATCHED = False


def _patch_tile():
    """This container's walrus rejects >2 sync-wait commands per instruction;
    TileContext's kernel-tail drain attaches one wait per active logical proc.
    Redistribute the drain's waits over event-sem instructions (2 each)."""
    global _PATCHED
    if _PATCHED:
        return
    import concourse.mybir as mybir
    import concourse.tile as tile
    from concourse.vector_clock import ScopedClock

    def _drain_and_barrier(self, tick_clock, wait_clock):
        nc = self.nc
        drain_inst = nc.sync.drain()
        wait_clock.add_sem_waits(
            drain_inst.ins, ScopedClock({None: tick_clock.global_clock})
        )
        si = drain_inst.ins.sync_info
        waits = list(si.on_wait) if si is not None else []
        if waits:
            drain_inst.ins.sync_info = mybir.SyncInfo(
                on_wait=[], on_update=list(si.on_update) if si else []
            )
            dummy_sem = nc.alloc_semaphore(f"tail_split_sem_{nc.next_id()}")
            for i in range(0, len(waits), 2):
                ev = nc.sync.wait_ge(dummy_sem, 0)
                evsi = ev.ins.sync_info
                ev.ins.sync_info = mybir.SyncInfo(
                    on_wait=waits[i : i + 2],
                    on_update=list(evsi.on_update) if evsi else [],
                )
        nc.all_engine_barrier()
        assert self.sems is not None
        popped = nc._tile_sem_poison_stack.pop()
        assert popped is self._sem_poison
        nc.clear_and_free_semaphores(list(self.sems.allocated().values()))
        nc.all_engine_barrier()

    tile.TileContext._drain_and_barrier = _drain_and_barrier
    _PATCHED = True


def _split_excess_waits(nc, max_waits=2):
    """This walrus build rejects instructions carrying more than 2 sync-wait
    commands, but Tile's wait-assignment pass can attach more. Move excess
    waits onto same-engine EventSemaphore instructions inserted just before
    the over-subscribed instruction."""
    import concourse.mybir as mybir

    counter = [0]
    for f in nc.m.functions:
        for bb in f.blocks:
            cur = list(bb.instructions)
            out = []
            changed = False
            for ins in cur:
                si = ins.sync_info
                waits = list(si.on_wait) if si is not None else []
                allowed = (
                    max_waits
                    if type(ins).__name__ == "InstEventSemaphore"
                    else 1
                )
                if len(waits) > allowed:
                    keep = waits[:allowed]
                    extra = waits[allowed:]
                    ins.sync_info = mybir.SyncInfo(
                        on_wait=keep, on_update=list(si.on_update)
                    )
                    for i in range(0, len(extra), max_waits):
                        counter[0] += 1
                        ev = mybir.InstEventSemaphore(
                            name=f"I-wsplit-{counter[0]}",
                            ins=[],
                            outs=[],
                            engine=ins.engine,
                        )
                        ev.sync_info = mybir.SyncInfo(
                            on_wait=extra[i : i + max_waits], on_update=[]
                        )
                        out.append(ev)
                    changed = True
                out.append(ins)
            if changed:
                bb.instructions = out


def prep(inputs):
    """Host preprocessing: returns (in_maps, meta). meta['chunks'] is the
    uniform [NB][R] chunk table that parameterizes the program structure."""
    box = np.asarray(inputs["box_features"], dtype=np.float32)
    lab = np.asarray(inputs["pred_labels"]).astype(np.int64).reshape(-1)
    ei = np.asarray(inputs["edge_index"]).astype(np.int64)
    et = np.asarray(inputs["edge_type"]).astype(np.int64).reshape(-1)
    emb = np.asarray(inputs["emb_table"], dtype=np.float32)
    W_lin = np.asarray(inputs["W_lin"], dtype=np.float32)
    b_lin = np.asarray(inputs["b_lin"], dtype=np.float32)
    rel_W1 = np.asarray(inputs["rel_W1"], dtype=np.float32)
    root_W1 = np.asarray(inputs["root_W1"], dtype=np.float32)
    b1 = np.asarray(inputs["b1"], dtype=np.float32)
    rel_W2 = np.asarray(inputs["rel_W2"], dtype=np.float32)
    root_W2 = np.asarray(inputs["root_W2"], dtype=np.float32)
    b2 = np.asarray(inputs["b2"], dtype=np.float32)
    Wc1 = np.asarray(inputs["Wc1"], dtype=np.float32)
    bc1 = np.asarray(inputs["bc1"], dtype=np.float32)
    Wc2 = np.asarray(inputs["Wc2"], dtype=np.float32)
    bc2 = np.asarray(inputs["bc2"], dtype=np.float32)

    src, dst = ei[0], ei[1]

    # per-(relation, dst) in-degree -> 1/cnt, folded into the one-hot values
    cnt = np.bincount(et * N + dst, minlength=R * N).reshape(R, N)
    inv = (1.0 / np.maximum(cnt, 1)).astype(np.float32)

    core_of = dst // NLOC
    blk = (dst % NLOC) // 128

    # uniform chunk counts per (block, relation): max over cores
    key = (core_of * NB + blk) * R + et
    ecnt = np.bincount(key, minlength=CORES * NB * R).reshape(CORES, NB, R)
    chunks = np.ceil(ecnt / 128).astype(np.int64).max(axis=0)  # [NB, R]
    totc = int(chunks.sum())
    tot_slots = totc * 128

    # slot offset of each (b, r) group (group-major order: b outer, r inner)
    goff = np.zeros((NB, R), dtype=np.int64)
    acc = 0
    for b in range(NB):
        for r in range(R):
            goff[b, r] = acc
            acc += int(chunks[b, r]) * 128

    # per-core gather index + one-hot construction
    order = np.lexsort((et, blk, core_of))  # sort by (core, blk, r)
    s_src = src[order]
    s_dst = dst[order]
    s_et = et[order]
    s_core = core_of[order]
    s_blk = blk[order]
    s_inv = inv[s_et, s_dst]

    gidx_all = np.zeros((CORES, tot_slots), dtype=np.int16)
    # compact one-hot encoding: per slot, the dst column (0..127); the binary
    # [slot, dst] one-hot matrix is built on-device by DVE as (IOTA == dcol)
    # in fp8 (exact); padding slots get dcol=255 which never matches. The
    # 1/cnt mean normalization is applied afterwards in bf16 via INVRB.
    dcol_all = np.full((CORES, 128, totc), 255.0, dtype=np.float32)

    # slot position of each sorted edge: group offset + rank within group
    grp_key = (s_core * NB + s_blk) * R + s_et
    # rank within group via cumcount on sorted keys
    new_grp = np.empty(E, dtype=bool)
    new_grp[0] = True
    new_grp[1:] = grp_key[1:] != grp_key[:-1]
    grp_start = np.flatnonzero(new_grp)
    start_of = np.repeat(grp_start, np.diff(np.append(grp_start, E)))
    rank = np.arange(E) - start_of
    slot = goff[s_blk, s_et] + rank  # slot within the core's stream
    chunk_of = slot // 128
    srow = slot % 128
    dcol = s_dst % 128

    c_idx = s_core
    gidx_all[c_idx, slot] = s_src.astype(np.int16)
    dcol_all[c_idx, srow, chunk_of] = dcol

    # 1/cnt per (block, relation, dst-col), row-replicated for DVE use:
    # invrb[c, :, (b*R+r)*128 + d] = inv[r, c*NLOC + b*128 + d]
    invrb = np.zeros((CORES, 128, NB * R * 128), dtype=np.float32)
    for c in range(CORES):
        for b in range(NB):
            for r in range(R):
                invrb[c, :, (b * R + r) * 128 : (b * R + r + 1) * 128] = inv[
                    r, c * NLOC + b * 128 : c * NLOC + (b + 1) * 128
                ][None, :]

    # ---- layer 2 collapses to graph level: the pooled mean of the second
    # conv is linear, so pooled[g] = sum_rt P[(g,rt)] @ W2_rt + b2 with
    # P[(g,rt)] = sum_src h1[src] * omega[src, (g,rt)] (rt==R is the root
    # term). Each core computes its own graphs' P rows directly from the
    # all-gathered h1 — no per-node layer-2 output is ever materialized.
    NG = N // 128                    # 64 graphs
    GRP = 96                         # padded (rt, gl) columns per core
    OM = np.zeros((N, NG, R + 1), dtype=np.float64)
    np.add.at(OM, (src, dst // 128, et), inv[et, dst] / 128.0)
    nodes = np.arange(N)
    OM[nodes, nodes // 128, R] += 1.0 / 128.0
    OM = OM.astype(np.float32)

    b2colT = np.zeros((128, 2), dtype=np.float32)
    b2colT[:, 0] = b2[:128]
    b2colT[:, 1] = b2[128:]

    # wrapped gather indices: position i -> [i % 16, i // 16], replicated x8
    gidx_wrap = np.zeros((CORES, 128, tot_slots // 16), dtype=np.int16)
    w = gidx_all.reshape(CORES, tot_slots // 16, 16).transpose(0, 2, 1)
    for rep in range(8):
        gidx_wrap[:, rep * 16 : (rep + 1) * 16, :] = w

    # weights, host-fused and laid out for SBUF tiles
    W_A = W_lin[:BOX]                                 # [1024, 512]
    W_Bc = emb @ W_lin[BOX:]                          # [151, 512]
    W_B = np.zeros((256, HID), dtype=np.float32)
    W_B[:NCLS] = W_Bc
    W_B[NCLS] = b_lin                                 # bias as a weight row
    labT = np.zeros((CORES, 256, NLOC), dtype=np.float32)
    for c in range(CORES):
        loc = lab[c * NLOC : (c + 1) * NLOC]
        labT[c, loc, np.arange(NLOC)] = 1.0
        labT[c, NCLS, :] = 1.0                        # constant-1 bias feature

    def chunk_rows(Wm, p=128):
        # [K, O] -> [128, (K/128)*O] with [p, k*O+o] = Wm[k*128+p, o]
        K, O = Wm.shape
        return Wm.reshape(K // p, p, O).transpose(1, 0, 2).reshape(p, -1)

    W1_t = np.concatenate(
        [chunk_rows(rel_W1[r]) for r in range(R)], axis=1
    )  # [128, 8*4*512]
    RW1_t = chunk_rows(root_W1)                       # [128, 4*512]
    W2_t = np.concatenate(
        [chunk_rows(rel_W2[r]) for r in range(R)], axis=1
    )  # [128, 8*4*256]
    RW2_t = chunk_rows(root_W2)                       # [128, 4*256]
    WC1_t = chunk_rows(Wc1)                           # [128, 2*256]
    WC2_t = chunk_rows(Wc2)                           # [128, 2*2]

    boxT = box.T.copy()                               # [1024 f, 8192 n]

    shared = {
        "W_A": chunk_rows(W_A).astype(bf16),          # [128, 8*512]
        "W_B": chunk_rows(W_B).astype(bf16),          # [128, 2*512]
        "W1": W1_t.astype(bf16),
        "RW1": RW1_t.astype(bf16),
        "W2": W2_t.astype(bf16),
        "RW2": RW2_t.astype(bf16),
        "WC1": WC1_t.astype(bf16),
        "WC2": WC2_t.astype(bf16),
        "b1row": b1.reshape(1, HID).astype(bf16),
        "b2row": b2.reshape(1, OUT).astype(bf16),
        "bc2row": bc2.reshape(1, NT).astype(bf16),
        "bc1row": bc1.reshape(1, OUT).astype(bf16),
        "ones1": np.ones((1, 128), dtype=bf16),
        "ones128": np.full((128, 1), 1.0 / 128.0, dtype=bf16),
        "EYE": np.eye(128, dtype=bf16),
        "IOTA": np.tile(np.arange(128, dtype=bf16), (128, 1)),
    }

    pack_order = ["W_A", "W_B", "W1", "RW1", "W2", "RW2", "WC1", "WC2",
                  "b1row", "b2row", "bc2row", "bc1row", "ones1", "ones128",
                  "EYE", "IOTA"]
    in_maps = []
    offsets = None
    for c in range(CORES):
        m = dict(shared)
        m["boxT"] = np.ascontiguousarray(
            boxT[:, c * NLOC : (c + 1) * NLOC]
        ).reshape(8, 128, NLOC).transpose(1, 0, 2).reshape(128, 8 * NLOC).astype(bf16)
        m["labT"] = (
            labT[c].reshape(2, 128, NLOC).transpose(1, 0, 2).reshape(128, 2 * NLOC)
        ).astype(bf16)
        m["DCOLW"] = dcol_all[c].astype(bf16)        # [128, totc]
        m["INVRB"] = invrb[c].astype(bf16)           # [128, NB*R*128]
        # omega columns for core c's graphs, (rt, gl)-ordered, padded to GRP
        omc = OM[:, c * GPC : (c + 1) * GPC, :].transpose(0, 2, 1)
        omp = np.zeros((N, GRP), dtype=np.float32)
        omp[:, : (R + 1) * GPC] = omc.reshape(N, (R + 1) * GPC)
        m["OMG"] = chunk_rows(omp).astype(bf16)      # [128, 64*GRP]
        m["B2CT"] = b2colT.astype(bf16)              # [128, 2]
        # pack every bf16 tensor into one flat blob (single DRAM parameter:
        # keeps host-side per-parameter binding overhead out of the NEFF)
        parts = ["boxT", "labT", "DCOLW", "INVRB", "OMG", "B2CT"] + pack_order
        offs = {}
        cur = 0
        bufs = []
        for name in parts:
            a = np.ascontiguousarray(m[name], dtype=bf16)
            offs[name] = (cur, a.shape)
            bufs.append(a.reshape(-1))
            cur += a.size
        blob = np.concatenate(bufs)
        if offsets is None:
            offsets = offs
        in_maps.append({"blob": blob.reshape(1, -1), "GIDX": gidx_wrap[c]})

    meta = {"chunks": tuple(tuple(int(x) for x in row) for row in chunks),
            "fp8": FP8, "offsets": offsets,
            "blob_elems": int(in_maps[0]["blob"].size)}
    return in_maps, meta


def build(meta, split_waits=True):
    _patch_tile()
    import concourse.bass as bass
    import concourse.mybir as mybir
    import concourse.tile as tile
    from concourse import library_config
    from concourse.bass import _add_dep_helper

    dt = mybir.dt
    AF = mybir.ActivationFunctionType
    use_fp8 = meta.get("fp8", False)
    gdt = dt.float8e4 if use_fp8 else dt.bfloat16
    chunks = meta["chunks"]
    totc = sum(sum(row) for row in chunks)
    tot_slots = totc * 128

    nc = bass.Bass()

    # ---- parameters: one packed bf16 blob + the int16 gather indices ----
    offsets = meta["offsets"]
    blob = nc.declare_dram_parameter(
        "blob", [1, meta["blob_elems"]], dt.bfloat16, isOutput=False
    )

    class _ParamViews(dict):
        def __missing__(self, name):
            off, shape = offsets[name]
            p, c = shape
            ap = blob.ap()[:, off : off + p * c]
            ap = ap.rearrange("a (p c) -> (a p) c", p=p)
            v = _View(ap)
            self[name] = v
            return v

    class _View:
        def __init__(self, ap):
            self._ap = ap
        def ap(self):
            return self._ap
        def __getitem__(self, idx):
            return self._ap[idx]

    P = _ParamViews()
    P["GIDX"] = nc.declare_dram_parameter(
        "GIDX", [128, tot_slots // 16], dt.int16, isOutput=False
    )
    out_y = nc.declare_dram_parameter("out", [2, GPC, NT], dt.float32, isOutput=True)

    # ---- internal DRAM ----
    xg_loc = nc.dram_tensor("xg_loc", [NLOC, HID], gdt)
    xfull = nc.dram_tensor("xfull", [N, HID], gdt, addr_space="Shared")
    hg_loc = nc.dram_tensor("hg_loc", [NLOC, HID], gdt)
    hfull = nc.dram_tensor("hfull", [N, HID], gdt, addr_space="Shared")

    rg = [list(range(CORES))]

    with tile.TileContext(nc) as tc:
        with (
            tc.tile_pool(name="wpool", bufs=1) as wpool,
            tc.tile_pool(name="xpool", bufs=1) as xpool,
            tc.tile_pool(name="spool", bufs=2) as spool,
            tc.tile_pool(name="gpool", bufs=2) as gpool,
            tc.tile_pool(name="ohpool", bufs=2) as ohpool,
            tc.tile_pool(name="invpool", bufs=2) as invpool,
            tc.tile_pool(name="stpool", bufs=2) as stpool,
            tc.tile_pool(name="rcpool", bufs=1) as rcpool,
            tc.tile_pool(name="psA", bufs=3, space="PSUM") as psA,
            tc.tile_pool(name="psB", bufs=2, space="PSUM") as psB,
            tc.tile_pool(name="psC", bufs=1, space="PSUM") as psC,
        ):
            # GPSIMD ucode library providing DMAGatherAnt; every dma_gather
            # gets an explicit dep edge on this load.
            liblod = nc.gpsimd.load_library(library_config.mlp)

            def load(name, shape, dtype=dt.bfloat16, pool=wpool):
                t = pool.tile(list(shape), dtype, tag=name)
                nc.sync.dma_start(t[:], P[name].ap())
                return t

            # ---- stage-1 loads: only what featurize + the first gathers
            # need, so the all-gather is issued as early as possible ----
            fpool_cm = tc.tile_pool(name="fpool", bufs=1)
            fpool = fpool_cm.__enter__()
            boxT_t = load("boxT", (128, 8 * NLOC), pool=fpool)
            labT_t = load("labT", (128, 2 * NLOC), pool=fpool)
            W_A_t = load("W_A", (128, 8 * HID), pool=fpool)
            W_B_t = load("W_B", (128, 2 * HID), pool=fpool)
            GIDX_t = load("GIDX", (128, tot_slots // 16), dt.int16)
            DCOLW_t = load("DCOLW", (128, totc))
            IOTA_t = load("IOTA", (128, 128))

            # featurize: x0 = [box, onehot(lab)] @ W (+bias via weight row)
            x0N = xpool.tile([128, NB, HID], dt.bfloat16, tag="x0N")
            x0G = xpool.tile([128, NB, HID], gdt, tag="x0G")
            for nb in range(NB):
                ps = psA.tile([128, HID], dt.float32, tag="ps512")
                for k in range(8):
                    nc.tensor.matmul(
                        ps[:],
                        boxT_t[:, k * NLOC + nb * 128 : k * NLOC + (nb + 1) * 128],
                        W_A_t[:, k * HID : (k + 1) * HID],
                        start=(k == 0),
                        stop=False,
                    )
                for k in range(2):
                    nc.tensor.matmul(
                        ps[:],
                        labT_t[:, k * NLOC + nb * 128 : k * NLOC + (nb + 1) * 128],
                        W_B_t[:, k * HID : (k + 1) * HID],
                        start=False,
                        stop=(k == 1),
                    )
                nc.scalar.activation(x0N[:, nb, :], ps[:], AF.Copy)
                nc.vector.tensor_copy(x0G[:, nb, :], ps[:])
            xg_dma = nc.sync.dma_start(
                xg_loc.ap().rearrange("(nb p) f -> p nb f", p=128), x0G[:]
            )
            fpool_cm.__exit__(None, None, None)

            # ---- all-gather x0 (issued before the heavy weight loads so the
            # loads stream in under the collective) ----
            nc.gpsimd.collective_compute(
                "AllGather",
                mybir.AluOpType.bypass,
                replica_groups=rg,
                ins=[xg_loc.ap().opt()],
                outs=[xfull.ap().opt()],
            )

            # ---- stage-2 loads: overlap the collective. Explicit dep on the
            # x0 writeout keeps these big copies from jumping ahead of it on
            # the DMA engines and delaying the all-gather start. ----
            def load2(name, shape, dtype=dt.bfloat16):
                t = wpool.tile(list(shape), dtype, tag=name)
                d = nc.sync.dma_start(t[:], P[name].ap())
                _add_dep_helper(d.ins, xg_dma.ins, reason="defer to stage 2")
                return t

            W1_t = load2("W1", (128, R * 4 * HID))
            RW1_t = load2("RW1", (128, 4 * HID))
            EYE_t = load2("EYE", (128, 128))
            b1_t = load2("b1row", (1, HID))
            ones1_t = load2("ones1", (1, 128))
            W2_t = load2("W2", (128, R * 4 * OUT))
            RW2_t = load2("RW2", (128, 4 * OUT))
            GRP = 96
            # OMG is only read after the second all-gather; a pool opened
            # after fpool's exit reuses the featurize tiles' freed space
            ompool_cm = tc.tile_pool(name="ompool", bufs=1)
            ompool = ompool_cm.__enter__()
            OMG_t = ompool.tile([128, 64 * GRP], dt.bfloat16, tag="OMG")
            omg_dma = nc.sync.dma_start(OMG_t[:], P["OMG"].ap())
            _add_dep_helper(omg_dma.ins, xg_dma.ins, reason="defer to stage 2")
            B2CT_t = load2("B2CT", (128, 2))
            WC1_t = load2("WC1", (128, 2 * OUT))
            WC2_t = load2("WC2", (128, 2 * NT))
            bc1_t = load2("bc1row", (1, OUT))
            bc2_t = load2("bc2row", (1, NT))

            # chunk offset of each block in the global stream
            boff = [0] * (NB + 1)
            for b in range(NB):
                boff[b + 1] = boff[b] + sum(chunks[b])

            def prefetch(src_full, b):
                """Issue block b's gather + one-hot build + inv load."""
                coff = boff[b]
                nch = boff[b + 1] - coff
                # gather this block's edge rows in one call
                gt = gpool.tile([128, nch, HID], gdt, tag="gt")
                g_ins = nc.gpsimd.dma_gather(
                    gt[:],
                    src_full.ap(),
                    GIDX_t[:, coff * 8 : (coff + nch) * 8],
                    num_idxs=nch * 128,
                    num_idxs_reg=nch * 128,
                    elem_size=HID,
                    single_packet=False,
                )
                _add_dep_helper(
                    g_ins.ins, liblod.ins,
                    reason="dma_gather needs mlp library",
                )
                # build this block's binary one-hot on DVE: (IOTA == dcol)
                # in fp8 (1.0/0.0 exact; padding dcol=255 never matches)
                ohb = ohpool.tile([128, nch, 128], gdt, tag="ohb")
                dc_ap = DCOLW_t[:, coff : coff + nch].rearrange(
                    "p (c u) -> p c u", u=1
                )
                io_ap = IOTA_t[:].rearrange("p (u d) -> p u d", u=1)
                dc_b, io_b = bass.broadcast_tensor_aps(dc_ap, io_ap)
                nc.vector.tensor_tensor(
                    ohb[:], io_b, dc_b, mybir.AluOpType.is_equal
                )
                # per-(relation, dst) 1/cnt rows (row-replicated, bf16)
                invb = invpool.tile([128, R * 128], dt.bfloat16, tag="invb")
                nc.sync.dma_start(
                    invb[:], P["INVRB"][:, b * R * 128 : (b + 1) * R * 128]
                )
                return gt, ohb, invb

            def root_prestage(xN, RW_t, brow_t, odim):
                """Compute rc[b] = x_b @ RW + b for every block while the
                all-gather runs (the tensor engine is otherwise idle there).
                Returns the rc tiles; layer() injects them into the transform
                PSUM group with one identity matmul each."""
                rcs = []
                for b in range(NB):
                    psr = psA.tile([128, HID], dt.float32, tag="ps512")
                    for fc in range(4):
                        nc.tensor.matmul(
                            psr[:, fc * 128 : (fc + 1) * 128],
                            xN[:, b, fc * 128 : (fc + 1) * 128],
                            EYE_t[:],
                            start=(fc == 0),
                            stop=(fc == 3),
                        )
                    str_t = stpool.tile([128, HID], dt.bfloat16, tag="stroot")
                    nc.scalar.activation(str_t[:], psr[:], AF.Copy)
                    pr = psB.tile([128, odim], dt.float32, tag="transps")
                    nc.tensor.matmul(
                        pr[:], ones1_t[:1, :], brow_t[:1, :odim],
                        start=True, stop=False,
                    )
                    for k in range(4):
                        nc.tensor.matmul(
                            pr[:],
                            str_t[:, k * 128 : (k + 1) * 128],
                            RW_t[:, k * odim : (k + 1) * odim],
                            start=False,
                            stop=(k == 3),
                        )
                    rc = rcpool.tile([128, odim], dt.bfloat16, tag=f"rc{b}")
                    nc.scalar.activation(rc[:], pr[:], AF.Copy)
                    rcs.append(rc)
                return rcs

            def layer(src_full, rcs, W_t, odim, act, out_tile, g_out=None):
                """One RGCN conv layer. out_tile: [128, NB, odim] bf16.
                rcs: prestaged per-block root+bias tiles.
                Block b+1's gather/one-hot prefetch issues before block b's
                compute so the engines pipeline one block ahead."""
                pf = prefetch(src_full, 0)
                for b in range(NB):
                    gt, ohb, invb = pf
                    if b + 1 < NB:
                        pf = prefetch(src_full, b + 1)
                    chb = [chunks[b][r] for r in range(R)]
                    # segment sums, transposed: S.T[f, dst] per relation; fp8
                    # DoubleRow contracts two 128-slot chunks per matmul.
                    st = []
                    j0 = 0
                    for r in range(R):
                        nchr = chb[r]
                        if nchr == 0:
                            st.append(None)
                            continue
                        pss = psA.tile([128, HID], dt.float32, tag="ps512")
                        # one accumulation group for the whole bank: start only
                        # on the very first matmul, stop only on the last, with
                        # explicit edges to pin starter-first / stopper-last.
                        starter = None
                        last_by_fc = {}
                        npair = nchr // 2
                        tail = nchr % 2
                        nsteps = npair + tail
                        for s in range(nsteps):
                            j = j0 + 2 * s
                            is_pair = s < npair
                            for fc in range(4):
                                if is_pair:
                                    mm = nc.tensor.matmul(
                                        pss[:, fc * 128 : (fc + 1) * 128],
                                        gt[:, j : j + 2, fc * 128 : (fc + 1) * 128],
                                        ohb[:, j : j + 2, :],
                                        start=(s == 0 and fc == 0),
                                        stop=(s == nsteps - 1 and fc == 3),
                                        perf_mode=mybir.MatmulPerfMode.DoubleRow,
                                    )
                                else:
                                    mm = nc.tensor.matmul(
                                        pss[:, fc * 128 : (fc + 1) * 128],
                                        gt[:, j, fc * 128 : (fc + 1) * 128],
                                        ohb[:, j, :],
                                        start=(s == 0 and fc == 0),
                                        stop=(s == nsteps - 1 and fc == 3),
                                    )
                                if starter is None:
                                    starter = mm
                                elif s == 0:
                                    _add_dep_helper(
                                        mm.ins, starter.ins,
                                        reason="psum group starter first",
                                    )
                                if s == nsteps - 1:
                                    last_by_fc[fc] = mm
                        for fc in range(3):
                            _add_dep_helper(
                                last_by_fc[3].ins, last_by_fc[fc].ins,
                                reason="psum group stopper last",
                            )
                        # st = pss * (1/cnt)[dst], bf16, on DVE (exact bf16
                        # normalization, same numerics as the old folded OH)
                        stt = stpool.tile([128, HID], dt.bfloat16, tag=f"st{r}")
                        pss3 = pss[:].rearrange("p (c d) -> p c d", d=128)
                        inv3 = invb[:, r * 128 : (r + 1) * 128].rearrange(
                            "p (u d) -> p u d", u=1
                        )
                        inv_b, _ = bass.broadcast_tensor_aps(inv3, pss3)
                        nc.vector.tensor_tensor(
                            stt[:].rearrange("p (c d) -> p c d", d=128),
                            pss3,
                            inv_b,
                            mybir.AluOpType.mult,
                        )
                        st.append(stt)
                        j0 += nchr
                    # transform: h[dst,:] = sum_r S_r.T.T @ W_r + rc[b]
                    # (rc = root term + bias, prestaged under the all-gather;
                    # injected via one identity matmul that opens the group)
                    pst = psB.tile([128, odim], dt.float32, tag="transps")
                    inj = nc.tensor.matmul(
                        pst[:], EYE_t[:], rcs[b][:],
                        start=True, stop=False,
                    )
                    rk = [
                        (r, k)
                        for r in range(R)
                        if st[r] is not None
                        for k in range(4)
                    ]
                    first = True
                    for r, k in rk:
                        mm = nc.tensor.matmul(
                            pst[:],
                            st[r][:, k * 128 : (k + 1) * 128],
                            W_t[:, (r * 4 + k) * odim : (r * 4 + k + 1) * odim],
                            start=False,
                            stop=((r, k) == rk[-1]),
                        )
                        if first:
                            _add_dep_helper(
                                mm.ins, inj.ins,
                                reason="psum group starter first",
                            )
                            first = False
                    nc.scalar.activation(out_tile[:, b, :], pst[:], act)
                    if g_out is not None:
                        # post-activation fp8 copy per block (feeds the next
                        # all-gather without a post-layer copy tail). NOTE:
                        # must stay on DVE — an Act-engine bf16->fp8 copy
                        # produces NaN on hardware (sim does not catch it).
                        nc.vector.tensor_copy(g_out[:, b, :], out_tile[:, b, :])

            # ---- layer 1 (root terms prestaged under all-gather #1) ----
            rcs1 = root_prestage(x0N, RW1_t, b1_t, HID)
            h1N = xpool.tile([128, NB, HID], dt.bfloat16, tag="h1N")
            h1G = xpool.tile([128, NB, HID], gdt, tag="h1G")
            layer(xfull, rcs1, W1_t, HID, AF.Relu, h1N, g_out=h1G)
            nc.sync.dma_start(
                hg_loc.ap().rearrange("(nb p) f -> p nb f", p=128), h1G[:]
            )
            nc.gpsimd.collective_compute(
                "AllGather",
                mybir.AluOpType.bypass,
                replica_groups=rg,
                ins=[hg_loc.ap().opt()],
                outs=[hfull.ap().opt()],
            )

            # ---- layer 2 + pooling, collapsed to graph level ----
            # pooled mean of the second conv is linear, so each core computes
            # only its graphs' P[(g,rt)] = sum_src h1[src]*omega[src,(g,rt)]
            # straight from the all-gathered h1 (64 accumulating matmuls over
            # streamed [128,512] chunks), then a tiny transform. The whole
            # per-node layer-2 phase (gathers, one-hots, transforms, pooling)
            # disappears.
            psP = psA.tile([128, HID], dt.float32, tag="ps512")
            hch = gpool.tile([128, 2, HID], gdt, tag="hch")
            nc.sync.dma_start(hch[:, 0, :], hfull.ap()[0:128, :])
            for k in range(64):
                if k + 1 < 64:
                    nc.sync.dma_start(
                        hch[:, (k + 1) % 2, :],
                        hfull.ap()[(k + 1) * 128 : (k + 2) * 128, :],
                    )
                nc.tensor.matmul(
                    psP[:GRP, :],
                    OMG_t[:, k * GRP : (k + 1) * GRP],
                    hch[:, k % 2, :],
                    start=(k == 0),
                    stop=(k == 63),
                )
            ompool_cm.__exit__(None, None, None)
            Psb = xpool.tile([GRP, HID], dt.bfloat16, tag="Psb")
            nc.scalar.activation(Psb[:], psP[:GRP, :], AF.Copy)
            # transpose the P rows: pt[f, (rt, gl)]
            pt = xpool.tile([128, 4, GRP], dt.bfloat16, tag="pt")
            for fc in range(4):
                psT = psA.tile([128, HID], dt.float32, tag="ps512")
                nc.tensor.matmul(
                    psT[:, :GRP],
                    Psb[:, fc * 128 : (fc + 1) * 128],
                    EYE_t[:GRP, :GRP],
                    start=True,
                    stop=True,
                )
                nc.scalar.activation(pt[:, fc, :], psT[:, :GRP], AF.Copy)
            # pooled^T[o, g] = sum_rt sum_fc W2_rt^T P^T in one PSUM group
            psF = psC.tile([128, 16], dt.float32, tag="poolps")
            starter = None
            lastmm = None
            for rt in range(R + 1):
                for fc in range(4):
                    for oc in range(2):
                        if rt < R:
                            wsl = W2_t[
                                :,
                                (rt * 4 + fc) * OUT + oc * 128 :
                                (rt * 4 + fc) * OUT + (oc + 1) * 128,
                            ]
                        else:
                            wsl = RW2_t[
                                :, fc * OUT + oc * 128 : fc * OUT + (oc + 1) * 128
                            ]
                        first = rt == 0 and fc == 0 and oc == 0
                        last = rt == R and fc == 3 and oc == 1
                        mm = nc.tensor.matmul(
                            psF[:, oc * 8 : (oc + 1) * 8],
                            wsl,
                            pt[:, fc, rt * GPC : (rt + 1) * GPC],
                            start=first,
                            stop=last,
                        )
                        if starter is None:
                            starter = mm
                        elif rt == 0 and fc == 0:
                            _add_dep_helper(
                                mm.ins, starter.ins,
                                reason="psum group starter first",
                            )
                        if last:
                            _add_dep_helper(
                                mm.ins, lastmm.ins,
                                reason="psum group stopper last",
                            )
                        lastmm = mm
            # pooledT = psF + b2 (bias varies along the feature partitions)
            pooledT = spool.tile([128, 16], dt.bfloat16, tag="pooledT")
            for oc in range(2):
                bc3 = B2CT_t[:, oc : oc + 1].rearrange("p (u d) -> p u d", u=1)
                ps3 = psF[:, oc * 8 : (oc + 1) * 8].rearrange(
                    "p (u d) -> p u d", u=1
                )
                bc_b, _ = bass.broadcast_tensor_aps(bc3, ps3)
                nc.vector.tensor_tensor(
                    pooledT[:, oc * 8 : (oc + 1) * 8].rearrange(
                        "p (u d) -> p u d", u=1
                    ),
                    ps3,
                    bc_b,
                    mybir.AluOpType.add,
                )

            # ---- classifier ----
            zT = spool.tile([128, 16], dt.bfloat16, tag="zT")
            for f2c in range(2):
                psz = psC.tile([128, 8], dt.float32, tag="zps")
                nc.tensor.matmul(
                    psz[:],
                    bc1_t[:1, f2c * 128 : (f2c + 1) * 128],
                    ones1_t[:1, :8],
                    start=True,
                    stop=False,
                )
                for f1c in range(2):
                    nc.tensor.matmul(
                        psz[:],
                        WC1_t[:, f1c * OUT + f2c * 128 : f1c * OUT + (f2c + 1) * 128],
                        pooledT[:, f1c * 8 : (f1c + 1) * 8],
                        start=False,
                        stop=(f1c == 1),
                    )
                nc.scalar.activation(zT[:, f2c * 8 : (f2c + 1) * 8], psz[:], AF.Copy)
            psy = psC.tile([8, NT], dt.float32, tag="yps")
            for f2c in range(2):
                nc.tensor.matmul(
                    psy[:],
                    zT[:, f2c * 8 : (f2c + 1) * 8],
                    WC2_t[:, f2c * NT : (f2c + 1) * NT],
                    start=(f2c == 0),
                    stop=False,
                )
            nc.tensor.matmul(
                psy[:], ones1_t[:1, :GPC], bc2_t[:1, :], start=False, stop=True
            )
            ylog = spool.tile([8, NT], dt.float32, tag="ylog")
            nc.vector.tensor_copy(ylog[:], psy[:])
            negmax = spool.tile([8, 1], dt.float32, tag="negmax")
            nc.vector.reduce_max(
                negmax[:], ylog[:], axis=mybir.AxisListType.X, negate=True
            )
            ey = spool.tile([8, NT], dt.float32, tag="ey")
            nc.scalar.activation(ey[:], ylog[:], AF.Exp, bias=negmax[:])
            ssum = spool.tile([8, 1], dt.float32, tag="ssum")
            nc.vector.reduce_sum(ssum[:], ey[:], axis=mybir.AxisListType.X)
            rinv = spool.tile([8, 1], dt.float32, tag="rinv")
            nc.vector.reciprocal(rinv[:], ssum[:])
            yprob = spool.tile([8, NT], dt.float32, tag="yprob")
            nc.vector.tensor_scalar_mul(yprob[:], ey[:], rinv[:])

            nc.sync.dma_start(out_y[0], ylog[:])
            nc.sync.dma_start(out_y[1], yprob[:])

    mybir.codegen_inst_isa_subclasses(nc)
    if split_waits:
        _split_excess_waits(nc)
    return nc


_CACHE = {}


def _get_built(inputs):
    in_maps, meta = prep(inputs)
    key = meta["chunks"]
    if key not in _CACHE:
        _CACHE[key] = build(meta)
    return _CACHE[key], in_maps, meta


def kernel(**inputs):
    nc, in_maps, meta = _get_built(inputs)
    from concourse.bass_utils import run_bass_kernel_spmd

    res = run_bass_kernel_spmd(nc, in_maps, core_ids=list(range(CORES)))
    ylog = np.concatenate([res.results[c]["out"][0] for c in range(CORES)], axis=0)
    yprob = np.concatenate([res.results[c]["out"][1] for c in range(CORES)], axis=0)
    return ylog.astype(np.float32), yprob.astype(np.float32)


# revision 55
# speedup vs baseline: 3.9123x; 1.0151x over previous
"""Trainium2 Bass kernel for a 2-layer RGCN scene-graph model (8 NeuronCores).

Sharding: node/dst-parallel. Core c owns nodes [c*1024, (c+1)*1024) (= 8 whole
graphs of 128 nodes, so pooling is local). Relation weights are replicated.
Message aggregation is mean-per-(relation, dst); we aggregate x first (the
relation transform is linear, so mean-then-transform == transform-then-mean)
which keeps all matmul work sharded. The segment mean is computed on the
TensorEngine as (gathered edge rows)^T @ one-hot, where the one-hot carries
1/cnt; edge rows are fetched with dma_gather from the all-gathered node
features in DRAM. The root-weight term x@RW is folded into the same pipeline
by a per-block PE transpose (matmul against an identity "one-hot"), so node
features never round-trip through DRAM for transposition. All integer index
metadata (edge sort, one-hot matrices, counts) is precomputed on host inside
kernel().
"""

import sys

sys.path.insert(0, "/opt/trn_rl_repo")

import numpy as np
import ml_dtypes

bf16 = ml_dtypes.bfloat16
fp8 = ml_dtypes.float8_e4m3
FP8 = True  # gather path in float8_e4m3 (halves AG + gather bytes)

N = 8192
E = 262144
R = 8
NCLS = 151
EMB = 256
BOX = 1024
HID = 512
OUT = 256
NT = 2
CORES = 8
NLOC = N // CORES          # 1024 nodes per core
NB = NLOC // 128           # 8 dst blocks of 128 per core
GPC = NLOC // 128          # graphs per core (nodes_per_graph == 128)

_PATCHED = False


def _patch_tile():
    """This container's walrus rejects >2 sync-wait commands per instruction;
    TileContext's kernel-tail drain attaches one wait per active logical proc.
    Redistribute the drain's waits over event-sem instructions (2 each)."""
    global _PATCHED
    if _PATCHED:
        return
    import concourse.mybir as mybir
    import concourse.tile as tile
    from concourse.vector_clock import ScopedClock

    def _drain_and_barrier(self, tick_clock, wait_clock):
        nc = self.nc
        drain_inst = nc.sync.drain()
        wait_clock.add_sem_waits(
            drain_inst.ins, ScopedClock({None: tick_clock.global_clock})
        )
        si = drain_inst.ins.sync_info
        waits = list(si.on_wait) if si is not None else []
        if waits:
            drain_inst.ins.sync_info = mybir.SyncInfo(
                on_wait=[], on_update=list(si.on_update) if si else []
            )
            dummy_sem = nc.alloc_semaphore(f"tail_split_sem_{nc.next_id()}")
            for i in range(0, len(waits), 2):
                ev = nc.sync.wait_ge(dummy_sem, 0)
                evsi = ev.ins.sync_info
                ev.ins.sync_info = mybir.SyncInfo(
                    on_wait=waits[i : i + 2],
                    on_update=list(evsi.on_update) if evsi else [],
                )
        nc.all_engine_barrier()
        assert self.sems is not None
        popped = nc._tile_sem_poison_stack.pop()
        assert popped is self._sem_poison
        nc.clear_and_free_semaphores(list(self.sems.allocated().values()))
        nc.all_engine_barrier()

    tile.TileContext._drain_and_barrier = _drain_and_barrier
    _PATCHED = True


def _split_excess_waits(nc, max_waits=2):
    """This walrus build rejects instructions carrying more than 2 sync-wait
    commands, but Tile's wait-assignment pass can attach more. Move excess
    waits onto same-engine EventSemaphore instructions inserted just before
    the over-subscribed instruction."""
    import concourse.mybir as mybir

    counter = [0]
    for f in nc.m.functions:
        for bb in f.blocks:
            cur = list(bb.instructions)
            out = []
            changed = False
            for ins in cur:
                si = ins.sync_info
                waits = list(si.on_wait) if si is not None else []
                allowed = (
                    max_waits
                    if type(ins).__name__ == "InstEventSemaphore"
                    else 1
                )
                if len(waits) > allowed:
                    keep = waits[:allowed]
                    extra = waits[allowed:]
                    ins.sync_info = mybir.SyncInfo(
                        on_wait=keep, on_update=list(si.on_update)
                    )
                    for i in range(0, len(extra), max_waits):
                        counter[0] += 1
                        ev = mybir.InstEventSemaphore(
                            name=f"I-wsplit-{counter[0]}",
                            ins=[],
                            outs=[],
                            engine=ins.engine,
                        )
                        ev.sync_info = mybir.SyncInfo(
                            on_wait=extra[i : i + max_waits], on_update=[]
                        )
                        out.append(ev)
                    changed = True
                out.append(ins)
            if changed:
                bb.instructions = out


def prep(inputs):
    """Host preprocessing: returns (in_maps, meta). meta['chunks'] is the
    uniform [NB][R] chunk table that parameterizes the program structure."""
    box = np.asarray(inputs["box_features"], dtype=np.float32)
    lab = np.asarray(inputs["pred_labels"]).astype(np.int64).reshape(-1)
    ei = np.asarray(inputs["edge_index"]).astype(np.int64)
    et = np.asarray(inputs["edge_type"]).astype(np.int64).reshape(-1)
    emb = np.asarray(inputs["emb_table"], dtype=np.float32)
    W_lin = np.asarray(inputs["W_lin"], dtype=np.float32)
    b_lin = np.asarray(inputs["b_lin"], dtype=np.float32)
    rel_W1 = np.asarray(inputs["rel_W1"], dtype=np.float32)
    root_W1 = np.asarray(inputs["root_W1"], dtype=np.float32)
    b1 = np.asarray(inputs["b1"], dtype=np.float32)
    rel_W2 = np.asarray(inputs["rel_W2"], dtype=np.float32)
    root_W2 = np.asarray(inputs["root_W2"], dtype=np.float32)
    b2 = np.asarray(inputs["b2"], dtype=np.float32)
    Wc1 = np.asarray(inputs["Wc1"], dtype=np.float32)
    bc1 = np.asarray(inputs["bc1"], dtype=np.float32)
    Wc2 = np.asarray(inputs["Wc2"], dtype=np.float32)
    bc2 = np.asarray(inputs["bc2"], dtype=np.float32)

    src, dst = ei[0], ei[1]

    # per-(relation, dst) in-degree -> 1/cnt, folded into the one-hot values
    cnt = np.bincount(et * N + dst, minlength=R * N).reshape(R, N)
    inv = (1.0 / np.maximum(cnt, 1)).astype(np.float32)

    core_of = dst // NLOC
    blk = (dst % NLOC) // 128

    # uniform chunk counts per (block, relation): max over cores
    key = (core_of * NB + blk) * R + et
    ecnt = np.bincount(key, minlength=CORES * NB * R).reshape(CORES, NB, R)
    chunks = np.ceil(ecnt / 128).astype(np.int64).max(axis=0)  # [NB, R]
    totc = int(chunks.sum())
    tot_slots = totc * 128

    # slot offset of each (b, r) group (group-major order: b outer, r inner)
    goff = np.zeros((NB, R), dtype=np.int64)
    acc = 0
    for b in range(NB):
        for r in range(R):
            goff[b, r] = acc
            acc += int(chunks[b, r]) * 128

    # per-core gather index + one-hot construction
    order = np.lexsort((et, blk, core_of))  # sort by (core, blk, r)
    s_src = src[order]
    s_dst = dst[order]
    s_et = et[order]
    s_core = core_of[order]
    s_blk = blk[order]
    s_inv = inv[s_et, s_dst]

    gidx_all = np.zeros((CORES, tot_slots), dtype=np.int16)
    # compact one-hot encoding: per slot, the dst column (0..127); the binary
    # [slot, dst] one-hot matrix is built on-device by DVE as (IOTA == dcol)
    # in fp8 (exact); padding slots get dcol=255 which never matches. The
    # 1/cnt mean normalization is applied afterwards in bf16 via INVRB.
    dcol_all = np.full((CORES, 128, totc), 255.0, dtype=np.float32)

    # slot position of each sorted edge: group offset + rank within group
    grp_key = (s_core * NB + s_blk) * R + s_et
    # rank within group via cumcount on sorted keys
    new_grp = np.empty(E, dtype=bool)
    new_grp[0] = True
    new_grp[1:] = grp_key[1:] != grp_key[:-1]
    grp_start = np.flatnonzero(new_grp)
    start_of = np.repeat(grp_start, np.diff(np.append(grp_start, E)))
    rank = np.arange(E) - start_of
    slot = goff[s_blk, s_et] + rank  # slot within the core's stream
    chunk_of = slot // 128
    srow = slot % 128
    dcol = s_dst % 128

    c_idx = s_core
    gidx_all[c_idx, slot] = s_src.astype(np.int16)
    dcol_all[c_idx, srow, chunk_of] = dcol

    # 1/cnt per (block, relation, dst-col), row-replicated for DVE use:
    # invrb[c, :, (b*R+r)*128 + d] = inv[r, c*NLOC + b*128 + d]
    invrb = np.zeros((CORES, 128, NB * R * 128), dtype=np.float32)
    for c in range(CORES):
        for b in range(NB):
            for r in range(R):
                invrb[c, :, (b * R + r) * 128 : (b * R + r + 1) * 128] = inv[
                    r, c * NLOC + b * 128 : c * NLOC + (b + 1) * 128
                ][None, :]

    # ---- layer 2 collapses to graph level: the pooled mean of the second
    # conv is linear, so pooled[g] = sum_rt P[(g,rt)] @ W2_rt + b2 with
    # P[(g,rt)] = sum_src h1[src] * omega[src, (g,rt)] (rt==R is the root
    # term). Each core computes its own graphs' P rows directly from the
    # all-gathered h1 — no per-node layer-2 output is ever materialized.
    NG = N // 128                    # 64 graphs
    GRP = 96                         # padded (rt, gl) columns per core
    OM = np.zeros((N, NG, R + 1), dtype=np.float64)
    np.add.at(OM, (src, dst // 128, et), inv[et, dst] / 128.0)
    nodes = np.arange(N)
    OM[nodes, nodes // 128, R] += 1.0 / 128.0
    OM = OM.astype(np.float32)

    b2colT = np.zeros((128, 2), dtype=np.float32)
    b2colT[:, 0] = b2[:128]
    b2colT[:, 1] = b2[128:]

    # wrapped gather indices: position i -> [i % 16, i // 16], replicated x8
    gidx_wrap = np.zeros((CORES, 128, tot_slots // 16), dtype=np.int16)
    w = gidx_all.reshape(CORES, tot_slots // 16, 16).transpose(0, 2, 1)
    for rep in range(8):
        gidx_wrap[:, rep * 16 : (rep + 1) * 16, :] = w

    # weights, host-fused and laid out for SBUF tiles
    W_A = W_lin[:BOX]                                 # [1024, 512]
    W_Bc = emb @ W_lin[BOX:]                          # [151, 512]
    W_B = np.zeros((256, HID), dtype=np.float32)
    W_B[:NCLS] = W_Bc
    W_B[NCLS] = b_lin                                 # bias as a weight row
    labT = np.zeros((CORES, 256, NLOC), dtype=np.float32)
    for c in range(CORES):
        loc = lab[c * NLOC : (c + 1) * NLOC]
        labT[c, loc, np.arange(NLOC)] = 1.0
        labT[c, NCLS, :] = 1.0                        # constant-1 bias feature

    def chunk_rows(Wm, p=128):
        # [K, O] -> [128, (K/128)*O] with [p, k*O+o] = Wm[k*128+p, o]
        K, O = Wm.shape
        return Wm.reshape(K // p, p, O).transpose(1, 0, 2).reshape(p, -1)

    W1_t = np.concatenate(
        [chunk_rows(rel_W1[r]) for r in range(R)], axis=1
    )  # [128, 8*4*512]
    RW1_t = chunk_rows(root_W1)                       # [128, 4*512]
    W2_t = np.concatenate(
        [chunk_rows(rel_W2[r]) for r in range(R)], axis=1
    )  # [128, 8*4*256]
    RW2_t = chunk_rows(root_W2)                       # [128, 4*256]
    WC1_t = chunk_rows(Wc1)                           # [128, 2*256]
    WC2_t = chunk_rows(Wc2)                           # [128, 2*2]

    boxT = box.T.copy()                               # [1024 f, 8192 n]

    shared = {
        "W_A": chunk_rows(W_A).astype(bf16),          # [128, 8*512]
        "W_B": chunk_rows(W_B).astype(bf16),          # [128, 2*512]
        "W1": W1_t.astype(bf16),
        "RW1": RW1_t.astype(bf16),
        "W2": W2_t.astype(bf16),
        "RW2": RW2_t.astype(bf16),
        "WC1": WC1_t.astype(bf16),
        "WC2": WC2_t.astype(bf16),
        "b1row": b1.reshape(1, HID).astype(bf16),
        "b2row": b2.reshape(1, OUT).astype(bf16),
        "bc2row": bc2.reshape(1, NT).astype(bf16),
        "bc1row": bc1.reshape(1, OUT).astype(bf16),
        "ones1": np.ones((1, 128), dtype=bf16),
        "ones128": np.full((128, 1), 1.0 / 128.0, dtype=bf16),
        "EYE": np.eye(128, dtype=bf16),
        "IOTA": np.tile(np.arange(128, dtype=bf16), (128, 1)),
    }

    pack_order = ["W_A", "W_B", "W1", "RW1", "W2", "RW2", "WC1", "WC2",
                  "b1row", "b2row", "bc2row", "bc1row", "ones1", "ones128",
                  "EYE", "IOTA"]
    in_maps = []
    offsets = None
    for c in range(CORES):
        m = dict(shared)
        m["boxT"] = np.ascontiguousarray(
            boxT[:, c * NLOC : (c + 1) * NLOC]
        ).reshape(8, 128, NLOC).transpose(1, 0, 2).reshape(128, 8 * NLOC).astype(bf16)
        m["labT"] = (
            labT[c].reshape(2, 128, NLOC).transpose(1, 0, 2).reshape(128, 2 * NLOC)
        ).astype(bf16)
        m["DCOLW"] = dcol_all[c].astype(bf16)        # [128, totc]
        m["INVRB"] = invrb[c].astype(bf16)           # [128, NB*R*128]
        # omega columns for core c's graphs, (rt, gl)-ordered, padded to GRP
        omc = OM[:, c * GPC : (c + 1) * GPC, :].transpose(0, 2, 1)
        omp = np.zeros((N, GRP), dtype=np.float32)
        omp[:, : (R + 1) * GPC] = omc.reshape(N, (R + 1) * GPC)
        m["OMG"] = chunk_rows(omp).astype(bf16)      # [128, 64*GRP]
        m["B2CT"] = b2colT.astype(bf16)              # [128, 2]
        # pack every bf16 tensor into one flat blob (single DRAM parameter:
        # keeps host-side per-parameter binding overhead out of the NEFF)
        parts = ["boxT", "labT", "DCOLW", "INVRB", "OMG", "B2CT"] + pack_order
        offs = {}
        cur = 0
        bufs = []
        for name in parts:
            a = np.ascontiguousarray(m[name], dtype=bf16)
            offs[name] = (cur, a.shape)
            bufs.append(a.reshape(-1))
            cur += a.size
        blob = np.concatenate(bufs)
        if offsets is None:
            offsets = offs
        in_maps.append({"blob": blob.reshape(1, -1), "GIDX": gidx_wrap[c]})

    meta = {"chunks": tuple(tuple(int(x) for x in row) for row in chunks),
            "fp8": FP8, "offsets": offsets,
            "blob_elems": int(in_maps[0]["blob"].size)}
    return in_maps, meta


def build(meta, split_waits=True):
    _patch_tile()
    import concourse.bass as bass
    import concourse.mybir as mybir
    import concourse.tile as tile
    from concourse import library_config
    from concourse.bass import _add_dep_helper

    dt = mybir.dt
    AF = mybir.ActivationFunctionType
    use_fp8 = meta.get("fp8", False)
    gdt = dt.float8e4 if use_fp8 else dt.bfloat16
    chunks = meta["chunks"]
    totc = sum(sum(row) for row in chunks)
    tot_slots = totc * 128

    nc = bass.Bass()

    # ---- parameters: one packed bf16 blob + the int16 gather indices ----
    offsets = meta["offsets"]
    blob = nc.declare_dram_parameter(
        "blob", [1, meta["blob_elems"]], dt.bfloat16, isOutput=False
    )

    class _ParamViews(dict):
        def __missing__(self, name):
            off, shape = offsets[name]
            p, c = shape
            ap = blob.ap()[:, off : off + p * c]
            ap = ap.rearrange("a (p c) -> (a p) c", p=p)
            v = _View(ap)
            self[name] = v
            return v

    class _View:
        def __init__(self, ap):
            self._ap = ap
        def ap(self):
            return self._ap
        def __getitem__(self, idx):
            return self._ap[idx]

    P = _ParamViews()
    P["GIDX"] = nc.declare_dram_parameter(
        "GIDX", [128, tot_slots // 16], dt.int16, isOutput=False
    )
    out_y = nc.declare_dram_parameter("out", [2, GPC, NT], dt.float32, isOutput=True)

    # ---- internal DRAM ----
    xg_loc = nc.dram_tensor("xg_loc", [NLOC, HID], gdt)
    xfull = nc.dram_tensor("xfull", [N, HID], gdt, addr_space="Shared")
    hg_loc = nc.dram_tensor("hg_loc", [NLOC, HID], gdt)
    hfull = nc.dram_tensor("hfull", [N, HID], gdt, addr_space="Shared")

    rg = [list(range(CORES))]

    with tile.TileContext(nc) as tc:
        with (
            tc.tile_pool(name="wpool", bufs=1) as wpool,
            tc.tile_pool(name="xpool", bufs=1) as xpool,
            tc.tile_pool(name="spool", bufs=2) as spool,
            tc.tile_pool(name="gpool", bufs=2) as gpool,
            tc.tile_pool(name="ohpool", bufs=2) as ohpool,
            tc.tile_pool(name="invpool", bufs=2) as invpool,
            tc.tile_pool(name="stpool", bufs=2) as stpool,
            tc.tile_pool(name="rcpool", bufs=1) as rcpool,
            tc.tile_pool(name="psA", bufs=3, space="PSUM") as psA,
            tc.tile_pool(name="psB", bufs=2, space="PSUM") as psB,
            tc.tile_pool(name="psC", bufs=1, space="PSUM") as psC,
        ):
            # GPSIMD ucode library providing DMAGatherAnt; every dma_gather
            # gets an explicit dep edge on this load.
            liblod = nc.gpsimd.load_library(library_config.mlp)

            def load(name, shape, dtype=dt.bfloat16, pool=wpool):
                t = pool.tile(list(shape), dtype, tag=name)
                nc.sync.dma_start(t[:], P[name].ap())
                return t

            # ---- stage-1 loads: only what featurize + the first gathers
            # need, so the all-gather is issued as early as possible ----
            fpool_cm = tc.tile_pool(name="fpool", bufs=1)
            fpool = fpool_cm.__enter__()
            boxT_t = load("boxT", (128, 8 * NLOC), pool=fpool)
            labT_t = load("labT", (128, 2 * NLOC), pool=fpool)
            W_A_t = load("W_A", (128, 8 * HID), pool=fpool)
            W_B_t = load("W_B", (128, 2 * HID), pool=fpool)
            GIDX_t = load("GIDX", (128, tot_slots // 16), dt.int16)
            DCOLW_t = load("DCOLW", (128, totc))
            IOTA_t = load("IOTA", (128, 128))

            # featurize: x0 = [box, onehot(lab)] @ W (+bias via weight row)
            x0N = xpool.tile([128, NB, HID], dt.bfloat16, tag="x0N")
            x0G = xpool.tile([128, NB, HID], gdt, tag="x0G")
            for nb in range(NB):
                ps = psA.tile([128, HID], dt.float32, tag="ps512")
                for k in range(8):
                    nc.tensor.matmul(
                        ps[:],
                        boxT_t[:, k * NLOC + nb * 128 : k * NLOC + (nb + 1) * 128],
                        W_A_t[:, k * HID : (k + 1) * HID],
                        start=(k == 0),
                        stop=False,
                    )
                for k in range(2):
                    nc.tensor.matmul(
                        ps[:],
                        labT_t[:, k * NLOC + nb * 128 : k * NLOC + (nb + 1) * 128],
                        W_B_t[:, k * HID : (k + 1) * HID],
                        start=False,
                        stop=(k == 1),
                    )
                nc.scalar.activation(x0N[:, nb, :], ps[:], AF.Copy)
                nc.vector.tensor_copy(x0G[:, nb, :], ps[:])
            xg_dma = nc.sync.dma_start(
                xg_loc.ap().rearrange("(nb p) f -> p nb f", p=128), x0G[:]
            )
            fpool_cm.__exit__(None, None, None)

            # ---- all-gather x0 (issued before the heavy weight loads so the
            # loads stream in under the collective) ----
            nc.gpsimd.collective_compute(
                "AllGather",
                mybir.AluOpType.bypass,
                replica_groups=rg,
                ins=[xg_loc.ap().opt()],
                outs=[xfull.ap().opt()],
            )

            # ---- stage-2 loads: overlap the collective. Explicit dep on the
            # x0 writeout keeps these big copies from jumping ahead of it on
            # the DMA engines and delaying the all-gather start. ----
            def load2(name, shape, dtype=dt.bfloat16):
                t = wpool.tile(list(shape), dtype, tag=name)
                d = nc.sync.dma_start(t[:], P[name].ap())
                _add_dep_helper(d.ins, xg_dma.ins, reason="defer to stage 2")
                return t

            W1_t = load2("W1", (128, R * 4 * HID))
            RW1_t = load2("RW1", (128, 4 * HID))
            EYE_t = load2("EYE", (128, 128))
            b1_t = load2("b1row", (1, HID))
            ones1_t = load2("ones1", (1, 128))
            W2_t = load2("W2", (128, R * 4 * OUT))
            RW2_t = load2("RW2", (128, 4 * OUT))
            GRP = 96
            # OMG is only read after the second all-gather; a pool opened
            # after fpool's exit reuses the featurize tiles' freed space
            ompool_cm = tc.tile_pool(name="ompool", bufs=1)
            ompool = ompool_cm.__enter__()
            OMG_t = ompool.tile([128, 64 * GRP], dt.bfloat16, tag="OMG")
            omg_dma = nc.sync.dma_start(OMG_t[:], P["OMG"].ap())
            _add_dep_helper(omg_dma.ins, xg_dma.ins, reason="defer to stage 2")
            B2CT_t = load2("B2CT", (128, 2))
            WC1_t = load2("WC1", (128, 2 * OUT))
            WC2_t = load2("WC2", (128, 2 * NT))
            bc1_t = load2("bc1row", (1, OUT))
            bc2_t = load2("bc2row", (1, NT))

            # chunk offset of each block in the global stream
            boff = [0] * (NB + 1)
            for b in range(NB):
                boff[b + 1] = boff[b] + sum(chunks[b])

            def prefetch(src_full, b):
                """Issue block b's gather + one-hot build + inv load."""
                coff = boff[b]
                nch = boff[b + 1] - coff
                # gather this block's edge rows in one call
                gt = gpool.tile([128, nch, HID], gdt, tag="gt")
                g_ins = nc.gpsimd.dma_gather(
                    gt[:],
                    src_full.ap(),
                    GIDX_t[:, coff * 8 : (coff + nch) * 8],
                    num_idxs=nch * 128,
                    num_idxs_reg=nch * 128,
                    elem_size=HID,
                    single_packet=False,
                )
                _add_dep_helper(
                    g_ins.ins, liblod.ins,
                    reason="dma_gather needs mlp library",
                )
                # build this block's binary one-hot on DVE: (IOTA == dcol)
                # in fp8 (1.0/0.0 exact; padding dcol=255 never matches)
                ohb = ohpool.tile([128, nch, 128], gdt, tag="ohb")
                dc_ap = DCOLW_t[:, coff : coff + nch].rearrange(
                    "p (c u) -> p c u", u=1
                )
                io_ap = IOTA_t[:].rearrange("p (u d) -> p u d", u=1)
                dc_b, io_b = bass.broadcast_tensor_aps(dc_ap, io_ap)
                nc.vector.tensor_tensor(
                    ohb[:], io_b, dc_b, mybir.AluOpType.is_equal
                )
                # per-(relation, dst) 1/cnt rows (row-replicated, bf16)
                invb = invpool.tile([128, R * 128], dt.bfloat16, tag="invb")
                nc.sync.dma_start(
                    invb[:], P["INVRB"][:, b * R * 128 : (b + 1) * R * 128]
                )
                return gt, ohb, invb

            def root_prestage(xN, RW_t, brow_t, odim):
                """Compute rc[b] = x_b @ RW + b for every block while the
                all-gather runs (the tensor engine is otherwise idle there).
                Returns the rc tiles; layer() injects them into the transform
                PSUM group with one identity matmul each."""
                rcs = []
                for b in range(NB):
                    psr = psA.tile([128, HID], dt.float32, tag="ps512")
                    for fc in range(4):
                        nc.tensor.matmul(
                            psr[:, fc * 128 : (fc + 1) * 128],
                            xN[:, b, fc * 128 : (fc + 1) * 128],
                            EYE_t[:],
                            start=(fc == 0),
                            stop=(fc == 3),
                        )
                    str_t = stpool.tile([128, HID], dt.bfloat16, tag="stroot")
                    nc.scalar.activation(str_t[:], psr[:], AF.Copy)
                    pr = psB.tile([128, odim], dt.float32, tag="transps")
                    nc.tensor.matmul(
                        pr[:], ones1_t[:1, :], brow_t[:1, :odim],
                        start=True, stop=False,
                    )
                    for k in range(4):
                        nc.tensor.matmul(
                            pr[:],
                            str_t[:, k * 128 : (k + 1) * 128],
                            RW_t[:, k * odim : (k + 1) * odim],
                            start=False,
                            stop=(k == 3),
                        )
                    rc = rcpool.tile([128, odim], dt.bfloat16, tag=f"rc{b}")
                    nc.scalar.activation(rc[:], pr[:], AF.Copy)
                    rcs.append(rc)
                return rcs

            def layer(src_full, rcs, W_t, odim, act, out_tile, g_out=None):
                """One RGCN conv layer. out_tile: [128, NB, odim] bf16.
                rcs: prestaged per-block root+bias tiles.
                Block b+1's gather/one-hot prefetch issues before block b's
                compute so the engines pipeline one block ahead."""
                pf = prefetch(src_full, 0)
                for b in range(NB):
                    gt, ohb, invb = pf
                    if b + 1 < NB:
                        pf = prefetch(src_full, b + 1)
                    chb = [chunks[b][r] for r in range(R)]
                    # segment sums, transposed: S.T[f, dst] per relation; fp8
                    # DoubleRow contracts two 128-slot chunks per matmul.
                    st = []
                    j0 = 0
                    for r in range(R):
                        nchr = chb[r]
                        if nchr == 0:
                            st.append(None)
                            continue
                        pss = psA.tile([128, HID], dt.float32, tag="ps512")
                        # one accumulation group for the whole bank: start only
                        # on the very first matmul, stop only on the last, with
                        # explicit edges to pin starter-first / stopper-last.
                        starter = None
                        last_by_fc = {}
                        npair = nchr // 2
                        tail = nchr % 2
                        nsteps = npair + tail
                        for s in range(nsteps):
                            j = j0 + 2 * s
                            is_pair = s < npair
                            for fc in range(4):
                                if is_pair:
                                    mm = nc.tensor.matmul(
                                        pss[:, fc * 128 : (fc + 1) * 128],
                                        gt[:, j : j + 2, fc * 128 : (fc + 1) * 128],
                                        ohb[:, j : j + 2, :],
                                        start=(s == 0 and fc == 0),
                                        stop=(s == nsteps - 1 and fc == 3),
                                        perf_mode=mybir.MatmulPerfMode.DoubleRow,
                                    )
                                else:
                                    mm = nc.tensor.matmul(
                                        pss[:, fc * 128 : (fc + 1) * 128],
                                        gt[:, j, fc * 128 : (fc + 1) * 128],
                                        ohb[:, j, :],
                                        start=(s == 0 and fc == 0),
                                        stop=(s == nsteps - 1 and fc == 3),
                                    )
                                if starter is None:
                                    starter = mm
                                elif s == 0:
                                    _add_dep_helper(
                                        mm.ins, starter.ins,
                                        reason="psum group starter first",
                                    )
                                if s == nsteps - 1:
                                    last_by_fc[fc] = mm
                        for fc in range(3):
                            _add_dep_helper(
                                last_by_fc[3].ins, last_by_fc[fc].ins,
                                reason="psum group stopper last",
                            )
                        # st = pss * (1/cnt)[dst], bf16, on DVE (exact bf16
                        # normalization, same numerics as the old folded OH)
                        stt = stpool.tile([128, HID], dt.bfloat16, tag=f"st{r}")
                        pss3 = pss[:].rearrange("p (c d) -> p c d", d=128)
                        inv3 = invb[:, r * 128 : (r + 1) * 128].rearrange(
                            "p (u d) -> p u d", u=1
                        )
                        inv_b, _ = bass.broadcast_tensor_aps(inv3, pss3)
                        nc.vector.tensor_tensor(
                            stt[:].rearrange("p (c d) -> p c d", d=128),
                            pss3,
                            inv_b,
                            mybir.AluOpType.mult,
                        )
                        st.append(stt)
                        j0 += nchr
                    # transform: h[dst,:] = sum_r S_r.T.T @ W_r + rc[b]
                    # (rc = root term + bias, prestaged under the all-gather;
                    # injected via one identity matmul that opens the group)
                    pst = psB.tile([128, odim], dt.float32, tag="transps")
                    inj = nc.tensor.matmul(
                        pst[:], EYE_t[:], rcs[b][:],
                        start=True, stop=False,
                    )
                    rk = [
                        (r, k)
                        for r in range(R)
                        if st[r] is not None
                        for k in range(4)
                    ]
                    first = True
                    for r, k in rk:
                        mm = nc.tensor.matmul(
                            pst[:],
                            st[r][:, k * 128 : (k + 1) * 128],
                            W_t[:, (r * 4 + k) * odim : (r * 4 + k + 1) * odim],
                            start=False,
                            stop=((r, k) == rk[-1]),
                        )
                        if first:
                            _add_dep_helper(
                                mm.ins, inj.ins,
                                reason="psum group starter first",
                            )
                            first = False
                    nc.scalar.activation(out_tile[:, b, :], pst[:], act)
                    if g_out is not None:
                        # post-activation fp8 copy per block (feeds the next
                        # all-gather without a post-layer copy tail). NOTE:
                        # must stay on DVE — an Act-engine bf16->fp8 copy
                        # produces NaN on hardware (sim does not catch it).
                        nc.vector.tensor_copy(g_out[:, b, :], out_tile[:, b, :])

            # ---- layer 1 (root terms prestaged under all-gather #1) ----
            rcs1 = root_prestage(x0N, RW1_t, b1_t, HID)
            h1N = xpool.tile([128, NB, HID], dt.bfloat16, tag="h1N")
            h1G = xpool.tile([128, NB, HID], gdt, tag="h1G")
            layer(xfull, rcs1, W1_t, HID, AF.Relu, h1N, g_out=h1G)
            nc.sync.dma_start(
                hg_loc.ap().rearrange("(nb p) f -> p nb f", p=128), h1G[:]
            )
            nc.gpsimd.collective_compute(
                "AllGather",
                mybir.AluOpType.bypass,
                replica_groups=rg,
                ins=[hg_loc.ap().opt()],
                outs=[hfull.ap().opt()],
            )

            # ---- layer 2 + pooling, collapsed to graph level ----
            # pooled mean of the second conv is linear, so each core computes
            # only its graphs' P[(g,rt)] = sum_src h1[src]*omega[src,(g,rt)]
            # straight from the all-gathered h1 (64 accumulating matmuls over
            # streamed [128,512] chunks), then a tiny transform. The whole
            # per-node layer-2 phase (gathers, one-hots, transforms, pooling)
            # disappears.
            psP = psA.tile([128, HID], dt.float32, tag="ps512")
            hch = gpool.tile([128, 4, HID], gdt, tag="hch")
            for k0 in range(3):
                nc.sync.dma_start(
                    hch[:, k0, :], hfull.ap()[k0 * 128 : (k0 + 1) * 128, :]
                )
            for k in range(64):
                if k + 3 < 64:
                    nc.sync.dma_start(
                        hch[:, (k + 3) % 4, :],
                        hfull.ap()[(k + 3) * 128 : (k + 4) * 128, :],
                    )
                nc.tensor.matmul(
                    psP[:GRP, :],
                    OMG_t[:, k * GRP : (k + 1) * GRP],
                    hch[:, k % 4, :],
                    start=(k == 0),
                    stop=(k == 63),
                )
            ompool_cm.__exit__(None, None, None)
            Psb = xpool.tile([GRP, HID], dt.bfloat16, tag="Psb")
            nc.scalar.activation(Psb[:], psP[:GRP, :], AF.Copy)
            # transpose the P rows: pt[f, (rt, gl)]
            pt = xpool.tile([128, 4, GRP], dt.bfloat16, tag="pt")
            for fc in range(4):
                psT = psA.tile([128, HID], dt.float32, tag="ps512")
                nc.tensor.matmul(
                    psT[:, :GRP],
                    Psb[:, fc * 128 : (fc + 1) * 128],
                    EYE_t[:GRP, :GRP],
                    start=True,
                    stop=True,
                )
                nc.scalar.activation(pt[:, fc, :], psT[:, :GRP], AF.Copy)
            # pooled^T[o, g] = sum_rt sum_fc W2_rt^T P^T in one PSUM group
            psF = psC.tile([128, 16], dt.float32, tag="poolps")
            starter = None
            lastmm = None
            for rt in range(R + 1):
                for fc in range(4):
                    for oc in range(2):
                        if rt < R:
                            wsl = W2_t[
                                :,
                                (rt * 4 + fc) * OUT + oc * 128 :
                                (rt * 4 + fc) * OUT + (oc + 1) * 128,
                            ]
                        else:
                            wsl = RW2_t[
                                :, fc * OUT + oc * 128 : fc * OUT + (oc + 1) * 128
                            ]
                        first = rt == 0 and fc == 0 and oc == 0
                        last = rt == R and fc == 3 and oc == 1
                        mm = nc.tensor.matmul(
                            psF[:, oc * 8 : (oc + 1) * 8],
                            wsl,
                            pt[:, fc, rt * GPC : (rt + 1) * GPC],
                            start=first,
                            stop=last,
                        )
                        if starter is None:
                            starter = mm
                        elif rt == 0 and fc == 0:
                            _add_dep_helper(
                                mm.ins, starter.ins,
                                reason="psum group starter first",
                            )
                        if last:
                            _add_dep_helper(
                                mm.ins, lastmm.ins,
                                reason="psum group stopper last",
                            )
                        lastmm = mm
            # pooledT = psF + b2 (bias varies along the feature partitions)
            pooledT = spool.tile([128, 16], dt.bfloat16, tag="pooledT")
            for oc in range(2):
                bc3 = B2CT_t[:, oc : oc + 1].rearrange("p (u d) -> p u d", u=1)
                ps3 = psF[:, oc * 8 : (oc + 1) * 8].rearrange(
                    "p (u d) -> p u d", u=1
                )
                bc_b, _ = bass.broadcast_tensor_aps(bc3, ps3)
                nc.vector.tensor_tensor(
                    pooledT[:, oc * 8 : (oc + 1) * 8].rearrange(
                        "p (u d) -> p u d", u=1
                    ),
                    ps3,
                    bc_b,
                    mybir.AluOpType.add,
                )

            # ---- classifier ----
            zT = spool.tile([128, 16], dt.bfloat16, tag="zT")
            for f2c in range(2):
                psz = psC.tile([128, 8], dt.float32, tag="zps")
                nc.tensor.matmul(
                    psz[:],
                    bc1_t[:1, f2c * 128 : (f2c + 1) * 128],
                    ones1_t[:1, :8],
                    start=True,
                    stop=False,
                )
                for f1c in range(2):
                    nc.tensor.matmul(
                        psz[:],
                        WC1_t[:, f1c * OUT + f2c * 128 : f1c * OUT + (f2c + 1) * 128],
                        pooledT[:, f1c * 8 : (f1c + 1) * 8],
                        start=False,
                        stop=(f1c == 1),
                    )
                nc.scalar.activation(zT[:, f2c * 8 : (f2c + 1) * 8], psz[:], AF.Copy)
            psy = psC.tile([8, NT], dt.float32, tag="yps")
            for f2c in range(2):
                nc.tensor.matmul(
                    psy[:],
                    zT[:, f2c * 8 : (f2c + 1) * 8],
                    WC2_t[:, f2c * NT : (f2c + 1) * NT],
                    start=(f2c == 0),
                    stop=False,
                )
            nc.tensor.matmul(
                psy[:], ones1_t[:1, :GPC], bc2_t[:1, :], start=False, stop=True
            )
            ylog = spool.tile([8, NT], dt.float32, tag="ylog")
            nc.vector.tensor_copy(ylog[:], psy[:])
            negmax = spool.tile([8, 1], dt.float32, tag="negmax")
            nc.vector.reduce_max(
                negmax[:], ylog[:], axis=mybir.AxisListType.X, negate=True
            )
            ey = spool.tile([8, NT], dt.float32, tag="ey")
            nc.scalar.activation(ey[:], ylog[:], AF.Exp, bias=negmax[:])
            ssum = spool.tile([8, 1], dt.float32, tag="ssum")
            nc.vector.reduce_sum(ssum[:], ey[:], axis=mybir.AxisListType.X)
            rinv = spool.tile([8, 1], dt.float32, tag="rinv")
            nc.vector.reciprocal(rinv[:], ssum[:])
            yprob = spool.tile([8, NT], dt.float32, tag="yprob")
            nc.vector.tensor_scalar_mul(yprob[:], ey[:], rinv[:])

            nc.sync.dma_start(out_y[0], ylog[:])
            nc.sync.dma_start(out_y[1], yprob[:])

    mybir.codegen_inst_isa_subclasses(nc)
    if split_waits:
        _split_excess_waits(nc)
    return nc


_CACHE = {}


def _get_built(inputs):
    in_maps, meta = prep(inputs)
    key = meta["chunks"]
    if key not in _CACHE:
        _CACHE[key] = build(meta)
    return _CACHE[key], in_maps, meta


def kernel(**inputs):
    nc, in_maps, meta = _get_built(inputs)
    from concourse.bass_utils import run_bass_kernel_spmd

    res = run_bass_kernel_spmd(nc, in_maps, core_ids=list(range(CORES)))
    ylog = np.concatenate([res.results[c]["out"][0] for c in range(CORES)], axis=0)
    yprob = np.concatenate([res.results[c]["out"][1] for c in range(CORES)], axis=0)
    return ylog.astype(np.float32), yprob.astype(np.float32)
